# revision 1
# baseline (speedup 1.0000x reference)
"""Trainium2 Bass kernel for nn_LongAttention (holographic long-attention block).

Computation (see reference):
  raw = x @ W_in.T -> split [c_phase | c_mag | q_re | q_im] per hd channel
  key = sigmoid(c_mag) * exp(i*(pi*tanh(c_phase) + pos_phase))
  state = cumsum_t(key);  ret = state * conj(q)
  ret_real = interleave(Re, Im) -> LayerNorm(2*hd) -> @ W_out.T

Distribution: hd (8192) split across 8 NeuronCores (1024 ch each); every core
handles both batches and all tokens; cores are fully independent. gamma is
folded into W_out on the host and the LayerNorm is algebraically deferred:
each core returns P = ret @ (W_out*gamma).T partials plus per-token
S1 = sum_f ret, S2 = sum_f ret^2; the host combines
out = istd * (sum_c P_c - mu * (W_out @ gamma)) + W_out @ beta.

Active implementation (build_program_v2, CN=256-token chunks):
 - f16 datapath end to end (matmul inputs, elementwise, scan output) --
   same speed as bf16 everywhere but ~8x finer mantissa, plus 2x DVE
   perf-modes on the 16-bit elementwise ops.
 - The magnitude-channel GEMM runs in fp8(e4m3) with perf_mode=DoubleRow
   (2 k-planes per instruction); the quantization scales (x*16, W*64) are
   folded into the on-chip tanh input scale. Sigmoid's 1/4 slope damps the
   fp8 noise; measured end-to-end rel err 0.011 < 2e-2.
 - sin/cos via the half-angle identity: th = (pi/2)*tanh(ph) + pos/2 with
   pos pre-wrapped to [-pi, pi) on the host, so |th| <= pi stays inside
   the ACT Sin LUT range; cos(2th) = 1-2*sin^2(th) gives the real part
   without a second LUT pass over an out-of-range argument.
 - The cumsum runs channel-major on the DVE as a prefix scan along the free
   (time) axis (fp32 internal state), carried across token chunks.
 - Per-token LN stats are folded on DVE/ACT (tree adds + squares) into one
   [128, 2*CN] tile and reduced across partitions by a single pair of
   ones-matmuls -- instead of 32 PE matmuls per chunk.
 - stats + proj_out for chunk n-1 are emitted during chunk n (software
   pipelining) so the in-order PE queue never waits on the chunk's serial
   ACT<->DVE elementwise chain; all hot pools are double-buffered.
"""

import sys
import numpy as np
import ml_dtypes

for _p in ("/opt/trn_rl_repo", "/root/.axon_site/_ro/trn_rl_repo"):
    if _p not in sys.path:
        sys.path.append(_p)

import bass_rust
import concourse.bass as bass
import concourse.tile as tile
import concourse.mybir as mybir
from concourse.bass_utils import run_bass_kernel_spmd

F32 = mybir.dt.float32
F8 = mybir.dt.float8e4
F16 = mybir.dt.float16
BF16 = mybir.dt.bfloat16
AF = mybir.ActivationFunctionType
ALU = mybir.AluOpType
PI = float(np.pi)

N_CORES = 8
LN_EPS = 1e-5


# --------------------------------------------------------------------------
# Workaround: this container's walrus rejects >1 semaphore wait per
# instruction ("Too many sync wait commands"). Split the extras onto
# same-engine NoOps inserted just before (engine FIFO keeps semantics).
# --------------------------------------------------------------------------
_nop_counter = [0]


def split_multiwait(nc):
    n_split = 0
    for f in nc.m.functions:
        for bb in f.blocks:
            il = bb.instructions
            i = 0
            while i < len(il):
                ins = il[i]
                si = ins.sync_info
                waits = list(si.on_wait) if si is not None and si.on_wait else []
                if len(waits) > 1:
                    for w in waits[:-1]:
                        _nop_counter[0] += 1
                        nop = bass_rust.InstNoOp(
                            name=f"mw_nop_{_nop_counter[0]}",
                            engine=ins.engine,
                            ins=[],
                            outs=[],
                        )
                        nop.sync_info = mybir.SyncInfo(on_wait=[w], on_update=[])
                        il.insert(i, nop)
                        i += 1
                    si.on_wait = [waits[-1]]
                    n_split += 1
                i += 1
    return n_split


# --------------------------------------------------------------------------
# Device program (SPMD: identical on all cores; per-core data differs)
# --------------------------------------------------------------------------
class Cfg:
    def __init__(self, B=2, T=2048, DIM=1024, NCH=1024, CN=256):
        self.B, self.T, self.DIM, self.NCH, self.CN = B, T, DIM, NCH, CN
        self.NTOK = B * T
        self.CT = NCH // 128          # channel tiles per core
        self.KT1 = DIM // 128         # contraction tiles for proj_in
        self.KT2 = 2 * self.CT        # contraction tiles for proj_out (re+im)
        self.DT = DIM // 128          # output dim tiles
        self.NCHUNK = self.NTOK // CN
        self.CPB = T // CN            # chunks per batch


def build_program(cfg: Cfg, reps: int = 1, hw_reps: int = 1,
                  probe: str | None = None):
    c = cfg
    assert c.CT % 4 == 0 or c.CT == 2
    SEGS = 4 if c.CT % 4 == 0 else 2   # channel tiles per wide tile
    NH = c.CT // SEGS                  # wide halves per chunk
    W = SEGS * c.CN                    # wide tile width
    nc = bass.Bass()

    class _Dup:
        def __init__(self, eng, on):
            self._eng, self._on = eng, on

        def __getattr__(self, n):
            f = getattr(self._eng, n)
            if not self._on:
                return f

            def g(*a, **k):
                r = f(*a, **k)
                f(*a, **k)
                return r
            return g

    pset = set(probe.split(",")) if probe else set()
    vec = _Dup(nc.vector, "dve2" in pset)
    sca = _Dup(nc.scalar, "act2" in pset)
    kt1_lim = c.KT1 // 2 if "pein_half" in pset else c.KT1
    kt2_lim = c.KT2 // 2 if "peout_half" in pset else c.KT2
    stats_on = "stats_off" not in pset

    w1 = nc.dram_tensor("w1", [128, c.KT1, 4 * c.NCH], BF16, kind="ExternalInput")
    w2 = nc.dram_tensor("w2", [128, c.KT2, c.DIM], BF16, kind="ExternalInput")
    xt = nc.dram_tensor("xt", [128, c.KT1, c.NTOK], BF16, kind="ExternalInput")
    cp = nc.dram_tensor("cp", [128, c.CT, c.T], F16, kind="ExternalInput")
    sp = nc.dram_tensor("sp", [128, c.CT, c.T], F16, kind="ExternalInput")
    outp = nc.dram_tensor("outp", [128, c.DT, c.NTOK], F32, kind="ExternalOutput")
    stats = nc.dram_tensor("stats", [2, c.NTOK], F32, kind="ExternalOutput")

    from contextlib import ExitStack
    with tile.TileContext(nc) as tc, ExitStack() as es:
        consts = es.enter_context(tc.tile_pool(name="consts", bufs=1))
        stream = es.enter_context(tc.tile_pool(name="stream", bufs=2))
        wide = es.enter_context(tc.tile_pool(name="wide", bufs=1))
        retp = es.enter_context(tc.tile_pool(name="retp", bufs=2))
        obp = es.enter_context(tc.tile_pool(name="obp", bufs=1))
        stc = es.enter_context(tc.tile_pool(name="stc", bufs=2))
        praw = es.enter_context(tc.tile_pool(name="praw", bufs=4, space="PSUM"))
        pstat = es.enter_context(tc.tile_pool(name="pstat", bufs=1, space="PSUM"))
        pstat2 = es.enter_context(tc.tile_pool(name="pstat2", bufs=1, space="PSUM"))
        pout = es.enter_context(tc.tile_pool(name="pout", bufs=2, space="PSUM"))

        w1_sb = consts.tile([128, c.KT1, 4 * c.NCH], BF16, tag="w1_sb")
        nc.sync.dma_start(out=w1_sb[:], in_=w1[:])
        w2_sb = consts.tile([128, c.KT2, c.DIM], BF16, tag="w2_sb")
        nc.sync.dma_start(out=w2_sb[:], in_=w2[:])
        w1_t = [w1_sb[:, k, :] for k in range(c.KT1)]
        w2_t = [w2_sb[:, k, :] for k in range(c.KT2)]

        ones_bf = consts.tile([128, 1], BF16, tag="ones")
        vec.memset(ones_bf[:], 1.0)
        half_pi = consts.tile([128, 1], F32, tag="half_pi")
        vec.memset(half_pi[:], PI / 2)
        car = {}
        for h in range(NH):
            for pl in ("re", "im"):
                car[(h, pl)] = consts.tile([128, SEGS], F32, tag=f"car_{h}_{pl}",
                                           name=f"car_{h}_{pl}")

        if hw_reps > 1:
            es.enter_context(tc.For_i(0, hw_reps))

        for n in [nn_ for _ in range(reps) for nn_ in range(c.NCHUNK)]:
            t0 = (n % c.CPB) * c.CN
            first_in_batch = t0 == 0
            tok = slice(n * c.CN, (n + 1) * c.CN)

            xcb = stream.tile([128, c.KT1, c.CN], BF16, tag="xcb")
            nc.sync.dma_start(out=xcb[:], in_=xt[:, :, tok])
            xc = [xcb[:, k, :] for k in range(c.KT1)]
            cpb = stream.tile([128, c.CT, c.CN], F16, tag="cpb")
            nc.sync.dma_start(out=cpb[:], in_=cp[:, :, t0:t0 + c.CN])
            spb = stream.tile([128, c.CT, c.CN], F16, tag="spb")
            nc.sync.dma_start(out=spb[:], in_=sp[:, :, t0:t0 + c.CN])

            ret_w = {}
            for h in range(NH):
                i0 = h * SEGS
                # ---- proj_in: 4 groups x SEGS channel tiles -> psum pairs ----
                # psum tile [128, 2*CN] holds channel tiles (j, j+1) of a group
                th_ph = wide.tile([128, W], F32, tag="th_ph", name="th_ph")
                th_mg = wide.tile([128, W], F32, tag="th_mg", name="th_mg")
                qre = wide.tile([128, W], F32, tag="qre", name="qre")
                qim = wide.tile([128, W], F32, tag="qim", name="qim")
                dest = {"ph": th_ph, "mg": th_mg, "qr": qre, "qi": qim}
                for j in range(0, SEGS, 2):
                    for gi, g in enumerate(("ph", "mg", "qr", "qi")):
                        p = praw.tile([128, 2 * c.CN], F32, tag="praw")
                        for half in range(2):
                            m = gi * c.CT + i0 + j + half
                            cols = slice(half * c.CN, (half + 1) * c.CN)
                            for k in range(kt1_lim):
                                nc.tensor.matmul(
                                    p[:, cols],
                                    w1_t[k][:, m * 128:(m + 1) * 128], xc[k],
                                    start=(k == 0), stop=(k == kt1_lim - 1))
                        wcols = slice(j * c.CN, (j + 2) * c.CN)
                        if g == "ph" or g == "mg":
                            sc = 1.0 if g == "ph" else 0.5
                            sca.activation(dest[g][:, wcols], p[:],
                                                 AF.Tanh, scale=sc)
                        elif "qdve" in pset:
                            vec.tensor_copy(dest[g][:, wcols], p[:])
                        else:
                            sca.copy(dest[g][:, wcols], p[:])

                # ---- content phasor (wide) ----
                sinp = wide.tile([128, W], F32, tag="sinp", name="sinp")
                sca.activation(sinp[:], th_ph[:], AF.Sin, scale=PI)
                tabs = wide.tile([128, W], F32, tag="tabs", name="tabs")
                sca.activation(tabs[:], th_ph[:], AF.Abs)
                cosp = wide.tile([128, W], F32, tag="th_ph", name="cosp")
                sca.activation(cosp[:], tabs[:], AF.Sin,
                                     bias=half_pi[:], scale=-PI)
                # 2*sigma = th_mg + 1 ; the 0.5 is folded into cp/sp on host
                ssin = wide.tile([128, W], F32, tag="tabs", name="ssin")
                vec.scalar_tensor_tensor(ssin[:], th_mg[:], 1.0, sinp[:],
                                               ALU.add, ALU.mult)
                scos = wide.tile([128, W], F32, tag="sinp", name="scos")
                vec.scalar_tensor_tensor(scos[:], th_mg[:], 1.0, cosp[:],
                                               ALU.add, ALU.mult)

                # ---- key = content * pos phasor (wide, cp/sp pre-halved) ----
                cps = cpb[:, i0:i0 + SEGS, :]
                sps = spb[:, i0:i0 + SEGS, :]
                ta = wide.tile([128, W], F32, tag="tmp1", name="ta")
                vec.tensor_mul(ta[:], scos[:], cps)
                tb = wide.tile([128, W], F32, tag="tmp2", name="tb")
                vec.tensor_mul(tb[:], ssin[:], sps)
                kre = wide.tile([128, W], F32, tag="kre", name="kre")
                vec.tensor_sub(kre[:], ta[:], tb[:])
                tc_ = wide.tile([128, W], F32, tag="tmp1", name="tc_")
                vec.tensor_mul(tc_[:], ssin[:], cps)
                td = wide.tile([128, W], F32, tag="tmp2", name="td")
                vec.tensor_mul(td[:], scos[:], sps)
                kim = wide.tile([128, W], F32, tag="kim", name="kim")
                vec.tensor_add(kim[:], tc_[:], td[:])

                # ---- prefix scan per channel tile segment ----
                mre = wide.tile([128, W], F32, tag="mre", name="mre")
                mim = wide.tile([128, W], F32, tag="mim", name="mim")
                for s in range(SEGS):
                    seg = slice(s * c.CN, (s + 1) * c.CN)
                    init_re = 0.0 if first_in_batch else car[(h, "re")][:, s:s + 1]
                    vec.tensor_tensor_scan(mre[:, seg], kre[:, seg],
                                                 kre[:, seg], init_re,
                                                 ALU.add, ALU.bypass)
                    init_im = 0.0 if first_in_batch else car[(h, "im")][:, s:s + 1]
                    vec.tensor_tensor_scan(mim[:, seg], kim[:, seg],
                                                 kim[:, seg], init_im,
                                                 ALU.add, ALU.bypass)
                if (n % c.CPB) != c.CPB - 1:
                    cre = mre.rearrange("p (s t) -> p s t", s=SEGS)[:, :, c.CN - 1]
                    vec.tensor_copy(car[(h, "re")][:], cre)
                    cim = mim.rearrange("p (s t) -> p s t", s=SEGS)[:, :, c.CN - 1]
                    vec.tensor_copy(car[(h, "im")][:], cim)

                # ---- retrieval = state * conj(q) (wide) ----
                r1 = wide.tile([128, W], F32, tag="tmp1", name="r1")
                vec.tensor_mul(r1[:], mre[:], qre[:])
                r2 = wide.tile([128, W], F32, tag="tmp2", name="r2")
                vec.tensor_mul(r2[:], mim[:], qim[:])
                rre = retp.tile([128, W], BF16, tag=f"ret_re_{h}",
                                name=f"ret_re_{h}")
                vec.tensor_add(rre[:], r1[:], r2[:])
                r3 = wide.tile([128, W], F32, tag="tmp1", name="r3")
                vec.tensor_mul(r3[:], mim[:], qre[:])
                r4 = wide.tile([128, W], F32, tag="tmp2", name="r4")
                vec.tensor_mul(r4[:], mre[:], qim[:])
                rim = retp.tile([128, W], BF16, tag=f"ret_im_{h}",
                                name=f"ret_im_{h}")
                vec.tensor_sub(rim[:], r3[:], r4[:])
                ret_w[(h, "re")] = rre
                ret_w[(h, "im")] = rim

            # ---- per-token stats via ones-matmuls ----
            ps1 = pstat.tile([1, c.CN], F32, tag="ps1")
            ps2 = pstat2.tile([1, c.CN], F32, tag="ps2")
            n_st = 2 * c.CT
            idx = 0
            for h in range(NH):
                for pl in ("re", "im"):
                    rw = ret_w[(h, pl)]
                    sq = wide.tile([128, W], BF16, tag="sq", name="sq")
                    vec.tensor_mul(sq[:], rw[:], rw[:])
                    if not stats_on:
                        continue
                    for s in range(SEGS):
                        seg = slice(s * c.CN, (s + 1) * c.CN)
                        nc.tensor.matmul(ps1[:], ones_bf[:], rw[:, seg],
                                         start=(idx == 0), stop=(idx == n_st - 1))
                        nc.tensor.matmul(ps2[:], ones_bf[:], sq[:, seg],
                                         start=(idx == 0), stop=(idx == n_st - 1))
                        idx += 1
            if not stats_on:
                nc.tensor.matmul(ps1[:], ones_bf[:], ret_w[(0, "re")][:, 0:c.CN],
                                 start=True, stop=True)
                nc.tensor.matmul(ps2[:], ones_bf[:], ret_w[(0, "im")][:, 0:c.CN],
                                 start=True, stop=True)
            s1c = stc.tile([1, c.CN], F32, tag="s1c", name="s1c")
            sca.copy(s1c[:], ps1[:])
            nc.sync.dma_start(out=stats[0:1, tok], in_=s1c[:])
            s2c = stc.tile([1, c.CN], F32, tag="s2c", name="s2c")
            sca.copy(s2c[:], ps2[:])
            nc.sync.dma_start(out=stats[1:2, tok], in_=s2c[:])

            # ---- proj_out partial (accumulate over all chpl tiles) ----
            ob = obp.tile([128, c.DT, c.CN], F32, tag="ob", name="ob")
            for d in range(c.DT):
                po = pout.tile([128, c.CN], F32, tag="pout")
                for k in range(kt2_lim):
                    if k < c.CT:
                        h, s, pl = k // SEGS, k % SEGS, "re"
                    else:
                        h, s, pl = (k - c.CT) // SEGS, (k - c.CT) % SEGS, "im"
                    rt = ret_w[(h, pl)][:, s * c.CN:(s + 1) * c.CN]
                    nc.tensor.matmul(po[:], w2_t[k][:, d * 128:(d + 1) * 128],
                                     rt, start=(k == 0), stop=(k == kt2_lim - 1))
                sca.copy(ob[:, d, :], po[:])
            nc.sync.dma_start(out=outp[:, :, tok], in_=ob[:])

    return nc


def build_program_v2(cfg: Cfg, reps: int = 1, hw_reps: int = 1,
                     probe: str | None = None):
    """v2: f16 datapath, CN=512, double-angle sin/cos (one pos tensor),
    stats folded on DVE (single ones-matmul per chunk), engine-balanced.

    key = sigma(mg)*exp(i*theta), theta = pi*tanh(ph) + pos.
    With th = theta/2 = (pi/2)*tanh(ph) + pos/2 (|th| <= pi, LUT-valid):
      sh = sin(th), ch = cos(th) = sin(pi/2 - |th|)
      kreN = (sh^2 - 0.5)*mg2 = -sigma*cos(theta)   (mg2 = tanh(mg/2)+1 = 2*sigma)
      kim  = sh*ch*mg2        =  sigma*sin(theta)
    The negated real part flows through the scan (SreN = -Sre); retrieval
    compensates: rre = mim*qim - mreN*qre ; rim = mim*qre + mreN*qim.
    """
    c = cfg
    assert c.CT % 4 == 0
    SEGS = 4
    NH = c.CT // SEGS
    W = SEGS * c.CN
    nc = bass.Bass()

    class _Dup:
        def __init__(self, eng, on):
            self._eng, self._on = eng, on

        def __getattr__(self, n):
            f = getattr(self._eng, n)
            if not self._on:
                return f

            def g(*a, **k):
                r = f(*a, **k)
                f(*a, **k)
                return r
            return g

    pset = set(probe.split(",")) if probe else set()
    vec = _Dup(nc.vector, "dve2" in pset)
    sca = _Dup(nc.scalar, "act2" in pset)
    kt1_lim = c.KT1 // 2 if "pein_half" in pset else c.KT1
    kt2_lim = c.KT2 // 2 if "peout_half" in pset else c.KT2

    fp8mag = "fp8mag" in pset
    pin2 = "pin512" in pset
    if fp8mag:
        w13 = nc.dram_tensor("w13", [128, c.KT1, 3 * c.NCH], F16,
                             kind="ExternalInput")
        w1m8 = nc.dram_tensor("w1m8", [128, c.KT1 // 2, 2, c.NCH], F8,
                              kind="ExternalInput")
        x8 = nc.dram_tensor("x8", [128, c.KT1, c.NTOK], F8,
                            kind="ExternalInput")
    else:
        w1 = nc.dram_tensor("w1", [128, c.KT1, 4 * c.NCH], F16,
                            kind="ExternalInput")
    w2 = nc.dram_tensor("w2", [128, c.KT2, c.DIM], F16, kind="ExternalInput")
    xt = nc.dram_tensor("xt", [128, c.KT1, c.NTOK], F16, kind="ExternalInput")
    pos = nc.dram_tensor("pos", [128, c.CT, c.T], F16, kind="ExternalInput")
    outp = nc.dram_tensor("outp", [128, c.DT, c.NTOK], BF16,
                          kind="ExternalOutput")
    stats = nc.dram_tensor("stats", [2, c.NTOK], F32, kind="ExternalOutput")

    from contextlib import ExitStack
    with tile.TileContext(nc) as tc, ExitStack() as es:
        small = c.CN <= 256
        praw_bufs = 4 if (small and not pin2) else 2
        pout_bufs = 2
        if "praw3" in pset:
            praw_bufs, pout_bufs = (6, 2) if small else (3, 1)
        consts = es.enter_context(tc.tile_pool(name="consts", bufs=1))
        stream = es.enter_context(tc.tile_pool(name="stream", bufs=2))
        wide_bufs = 2 if small else 1
        for p_ in pset:
            if p_.startswith("wb"):
                wide_bufs = int(p_[2:])
        wide = es.enter_context(tc.tile_pool(name="wide", bufs=wide_bufs))
        retp = es.enter_context(tc.tile_pool(name="retp", bufs=2 if small else 1))
        obp = es.enter_context(tc.tile_pool(name="obp",
                                            bufs=1 if (pin2 or not small) else 2))
        stb = es.enter_context(tc.tile_pool(name="stb", bufs=1 if pin2 else 2))
        stc = es.enter_context(tc.tile_pool(name="stc", bufs=1 if pin2 else 2))
        praw = es.enter_context(tc.tile_pool(name="praw", bufs=praw_bufs,
                                             space="PSUM"))
        pstat = es.enter_context(tc.tile_pool(name="pstat", bufs=1, space="PSUM"))
        pout = es.enter_context(tc.tile_pool(name="pout", bufs=pout_bufs,
                                             space="PSUM"))

        if fp8mag:
            w1_sb = consts.tile([128, c.KT1, 3 * c.NCH], F16, tag="w1_sb")
            nc.sync.dma_start(out=w1_sb[:], in_=w13[:])
            w1m8_sb = consts.tile([128, c.KT1 // 2, 2, c.NCH], F8,
                                  tag="w1m8_sb")
            nc.sync.dma_start(out=w1m8_sb[:], in_=w1m8[:])
        else:
            w1_sb = consts.tile([128, c.KT1, 4 * c.NCH], F16, tag="w1_sb")
            nc.sync.dma_start(out=w1_sb[:], in_=w1[:])
        w2_sb = consts.tile([128, c.KT2, c.DIM], F16, tag="w2_sb")
        nc.sync.dma_start(out=w2_sb[:], in_=w2[:])
        w1_t = [w1_sb[:, k, :] for k in range(c.KT1)]
        w2_t = [w2_sb[:, k, :] for k in range(c.KT2)]

        ones_bf = consts.tile([128, 1], BF16, tag="ones")
        vec.memset(ones_bf[:], 1.0)
        one_f = consts.tile([128, 1], F32, tag="one_f")
        vec.memset(one_f[:], 1.0)
        half_pi = consts.tile([128, 1], F32, tag="half_pi")
        vec.memset(half_pi[:], PI / 2)
        car = {}
        for h in range(NH):
            for pl in ("re", "im"):
                car[(h, pl)] = consts.tile([128, SEGS], F16, tag=f"car_{h}_{pl}",
                                           name=f"car_{h}_{pl}")

        if hw_reps > 1:
            es.enter_context(tc.For_i(0, hw_reps))

        h2 = W // 2

        def emit_tail_pre(st):
            """DVE folds + ACT squares for the previous chunk's stats."""
            rw = st["ret"]
            a = wide.tile([128, W], F16, tag="stA", bufs=1, name="a")
            vec.tensor_add(a[:], rw[(0, "re")][:], rw[(0, "im")][:])
            b = wide.tile([128, W], F16, tag="stB", bufs=1, name="b")
            vec.tensor_add(b[:], rw[(1, "re")][:], rw[(1, "im")][:])
            rs = wide.tile([128, W], F16, tag="stC", bufs=1, name="rs")
            vec.tensor_add(rs[:], a[:], b[:])
            f = wide.tile([128, h2], F16, tag="stD", bufs=1, name="f")
            vec.tensor_add(f[:], rs[:, 0:h2], rs[:, h2:W])
            stt = stb.tile([128, 2 * c.CN], BF16, tag="stt", name="stt")
            vec.tensor_add(stt[:, 0:c.CN], f[:, 0:c.CN], f[:, c.CN:h2])
            def _sq(dst, src):
                if "sqdve" in pset:
                    vec.tensor_mul(dst, src, src)
                else:
                    sca.activation(dst, src, AF.Square)
            s0 = wide.tile([128, W], BF16, tag="stA", bufs=1, name="s0")
            _sq(s0[:], rw[(0, "re")][:])
            s1 = wide.tile([128, W], BF16, tag="stB", bufs=1, name="s1")
            _sq(s1[:], rw[(0, "im")][:])
            a2 = wide.tile([128, W], BF16, tag="stC", bufs=1, name="a2")
            vec.tensor_add(a2[:], s0[:], s1[:])
            s2 = wide.tile([128, W], BF16, tag="stA", bufs=1, name="s2")
            _sq(s2[:], rw[(1, "re")][:])
            s3 = wide.tile([128, W], BF16, tag="stB", bufs=1, name="s3")
            _sq(s3[:], rw[(1, "im")][:])
            b2 = wide.tile([128, W], BF16, tag="stD", bufs=1, name="b2")
            vec.tensor_add(b2[:], s2[:], s3[:])
            ss = wide.tile([128, W], BF16, tag="stA", bufs=1, name="ss")
            vec.tensor_add(ss[:], a2[:], b2[:])
            f2 = wide.tile([128, h2], BF16, tag="stB", bufs=1, name="f2")
            vec.tensor_add(f2[:], ss[:, 0:h2], ss[:, h2:W])
            vec.tensor_add(stt[:, c.CN:2 * c.CN], f2[:, 0:c.CN],
                           f2[:, c.CN:h2])
            st["stt"] = stt

        def emit_tail_mm(st):
            """Stats matmul + proj_out for the previous chunk."""
            stt, tok_p, rw = st["stt"], st["tok"], st["ret"]
            ps = pstat.tile([1, 2 * c.CN], F32, tag="ps")
            nc.tensor.matmul(ps[:, 0:c.CN], ones_bf[:], stt[:, 0:c.CN],
                             start=True, stop=True)
            nc.tensor.matmul(ps[:, c.CN:2 * c.CN], ones_bf[:],
                             stt[:, c.CN:2 * c.CN], start=True, stop=True)
            sc = stc.tile([1, 2 * c.CN], F32, tag="sc", name="sc")
            sca.copy(sc[:], ps[:])
            nc.sync.dma_start(out=stats[0:1, tok_p], in_=sc[:, 0:c.CN])
            nc.sync.dma_start(out=stats[1:2, tok_p], in_=sc[:, c.CN:2 * c.CN])
            ob = obp.tile([128, c.DT, c.CN], BF16, tag="ob", name="ob")
            for d in range(c.DT):
                po = pout.tile([128, c.CN], F32, tag="pout")
                for k in range(kt2_lim):
                    if k < c.CT:
                        h, s, pl = k // SEGS, k % SEGS, "re"
                    else:
                        h, s, pl = (k - c.CT) // SEGS, (k - c.CT) % SEGS, "im"
                    rt = rw[(h, pl)][:, s * c.CN:(s + 1) * c.CN]
                    nc.tensor.matmul(po[:], w2_t[k][:, d * 128:(d + 1) * 128],
                                     rt, start=(k == 0), stop=(k == kt2_lim - 1))
                if "obdve" in pset:
                    vec.tensor_copy(ob[:, d, :], po[:])
                else:
                    sca.copy(ob[:, d, :], po[:])
            nc.sync.dma_start(out=outp[:, :, tok_p], in_=ob[:])

        prev = None
        pend_in = {}
        for n in [nn_ for _ in range(reps) for nn_ in range(c.NCHUNK)]:
            t0 = (n % c.CPB) * c.CN
            first_in_batch = t0 == 0
            tok = slice(n * c.CN, (n + 1) * c.CN)

            if prev is not None:
                emit_tail_pre(prev)

            posb = stream.tile([128, c.CT, c.CN], F16, tag="posb")
            nc.sync.dma_start(out=posb[:], in_=pos[:, :, t0:t0 + c.CN])

            gnames = {"ph": "th_ph", "mg": "th_mg", "qr": "qre", "qi": "qim"}
            if not pin2 or n % 2 == 0:
                ntin = 2 * c.CN if pin2 else c.CN
                itok = slice(n * c.CN, n * c.CN + ntin)
                sb = 1 if pin2 else None
                xcb = stream.tile([128, c.KT1, ntin], F16, tag="xcb", bufs=sb)
                nc.sync.dma_start(out=xcb[:], in_=xt[:, :, itok])
                if fp8mag:
                    x8cb = stream.tile([128, c.KT1, ntin], F8, tag="x8cb",
                                       bufs=sb)
                    nc.sync.dma_start(out=x8cb[:], in_=x8[:, :, itok])
                ib = 4 if pin2 else None
                cur_in, nxt_in = {}, {}
                for h in range(NH):
                    for nm in ("th_ph", "th_mg", "qre", "qim"):
                        cur_in[(h, nm)] = wide.tile([128, W], F16, tag=nm,
                                                    name=nm, bufs=ib)
                        if pin2:
                            nxt_in[(h, nm)] = wide.tile([128, W], F16, tag=nm,
                                                        name=nm + "b", bufs=ib)
                dmaps = [(cur_in, 0)] + ([(nxt_in, 1)] if pin2 else [])
                for h in range(NH):
                    i0 = h * SEGS
                    for j in range(0, SEGS, 2):
                        for g in ("ph", "mg", "qr", "qi"):
                            p = praw.tile([128, 2 * ntin], F32, tag="praw")
                            pv = p.rearrange("p (c t) -> p c t", c=2)
                            nm = gnames[g]
                            wcols = slice(j * c.CN, (j + 2) * c.CN)
                            if g == "mg" and fp8mag:
                                nk = c.KT1 // 2
                                for half in range(2):
                                    mch = i0 + j + half
                                    for kk in range(nk):
                                        nc.tensor.matmul(
                                            pv[:, half, :],
                                            w1m8_sb[:, kk, :,
                                                    mch * 128:(mch + 1) * 128],
                                            x8cb[:, 2 * kk:2 * kk + 2, :],
                                            start=(kk == 0),
                                            stop=(kk == nk - 1),
                                            perf_mode=mybir.MatmulPerfMode.DoubleRow)
                                for dmap, cc in dmaps:
                                    src = pv[:, :, cc * c.CN:(cc + 1) * c.CN]
                                    sca.activation(dmap[(h, nm)][:, wcols],
                                                   src, AF.Tanh,
                                                   scale=0.5 / 1024.0)
                                continue
                            if fp8mag:
                                gi = {"ph": 0, "qr": 1, "qi": 2}[g]
                            else:
                                gi = {"ph": 0, "mg": 1, "qr": 2, "qi": 3}[g]
                            for half in range(2):
                                m = gi * c.CT + i0 + j + half
                                for k in range(kt1_lim):
                                    nc.tensor.matmul(
                                        pv[:, half, :],
                                        w1_t[k][:, m * 128:(m + 1) * 128],
                                        xcb[:, k, :],
                                        start=(k == 0),
                                        stop=(k == kt1_lim - 1))
                            for dmap, cc in dmaps:
                                src = pv[:, :, cc * c.CN:(cc + 1) * c.CN]
                                dst = dmap[(h, nm)][:, wcols]
                                if g == "ph":
                                    sca.activation(dst, src, AF.Tanh)
                                elif g == "mg":
                                    sca.activation(dst, src, AF.Tanh, scale=0.5)
                                elif "qdve" in pset:
                                    vec.tensor_copy(dst, src)
                                else:
                                    sca.copy(dst, src)
                if pin2:
                    pend_in.clear()
                    pend_in.update(nxt_in)
            else:
                cur_in = dict(pend_in)

            ret_w = {}
            for h in range(NH):
                i0 = h * SEGS
                th_ph = cur_in[(h, "th_ph")]
                th_mg = cur_in[(h, "th_mg")]
                qre = cur_in[(h, "qre")]
                qim = cur_in[(h, "qim")]

                # th = (pi/2)*tanh(ph) + pos/2 ; th_ph already scaled by pi/2
                pos_h = posb[:, i0:i0 + SEGS, :]
                theta = wide.tile([128, W], F16, tag="theta", name="theta")
                vec.scalar_tensor_tensor(theta[:], th_ph[:], PI / 2, pos_h,
                                         ALU.mult, ALU.add)
                sh = wide.tile([128, W], F16, tag="sh", name="sh")
                sca.activation(sh[:], theta[:], AF.Sin)
                ab = wide.tile([128, W], F16, tag="ab", name="ab")
                if "abdve" in pset:
                    vec.tensor_scalar(ab[:], theta[:], 0.0, None, ALU.abs_max)
                else:
                    sca.activation(ab[:], theta[:], AF.Abs)
                ch = wide.tile([128, W], F16, tag="theta", name="ch")
                sca.activation(ch[:], ab[:], AF.Sin, bias=half_pi[:], scale=-1.0)
                sqh = wide.tile([128, W], F16, tag="ab", name="sqh")
                if "sqhdve" in pset:
                    vec.tensor_mul(sqh[:], sh[:], sh[:])
                else:
                    sca.activation(sqh[:], sh[:], AF.Square)
                mg2 = wide.tile([128, W], F16, tag="mg2", name="mg2")
                if "mg2dve" in pset:
                    vec.tensor_scalar(mg2[:], th_mg[:], 1.0, None, ALU.add)
                else:
                    sca.activation(mg2[:], th_mg[:], AF.Identity, bias=one_f[:])

                kreN = wide.tile([128, W], F16,
                                 tag="kreN" if pin2 else "th_ph", name="kreN")
                vec.scalar_tensor_tensor(kreN[:], sqh[:], 0.5, mg2[:],
                                         ALU.subtract, ALU.mult)
                tt = wide.tile([128, W], F16,
                               tag="tt" if pin2 else "th_mg", name="tt")
                vec.tensor_mul(tt[:], sh[:], ch[:])
                kim = wide.tile([128, W], F16, tag="sh", name="kim")
                vec.tensor_mul(kim[:], tt[:], mg2[:])

                mre = wide.tile([128, W], F16, tag="mre", name="mre")
                mim = wide.tile([128, W], F16, tag="mim", name="mim")
                for s in range(SEGS):
                    seg = slice(s * c.CN, (s + 1) * c.CN)
                    init_re = 0.0 if first_in_batch else car[(h, "re")][:, s:s + 1]
                    vec.tensor_tensor_scan(mre[:, seg], kreN[:, seg],
                                           kreN[:, seg], init_re,
                                           ALU.add, ALU.bypass)
                    init_im = 0.0 if first_in_batch else car[(h, "im")][:, s:s + 1]
                    vec.tensor_tensor_scan(mim[:, seg], kim[:, seg],
                                           kim[:, seg], init_im,
                                           ALU.add, ALU.bypass)
                if (n % c.CPB) != c.CPB - 1:
                    cre = mre.rearrange("p (s t) -> p s t", s=SEGS)[:, :, c.CN - 1]
                    vec.tensor_copy(car[(h, "re")][:], cre)
                    cim = mim.rearrange("p (s t) -> p s t", s=SEGS)[:, :, c.CN - 1]
                    vec.tensor_copy(car[(h, "im")][:], cim)

                # retrieval (mreN = -Sre):
                #   rre = mim*qim - mreN*qre ; rim = mim*qre + mreN*qim
                r1 = wide.tile([128, W], F16, tag="theta", name="r1")
                vec.tensor_mul(r1[:], mre[:], qre[:])
                r2 = wide.tile([128, W], F16, tag="ab", name="r2")
                vec.tensor_mul(r2[:], mim[:], qim[:])
                rre = retp.tile([128, W], F16, tag=f"ret_re_{h}",
                                name=f"ret_re_{h}")
                vec.tensor_sub(rre[:], r2[:], r1[:])
                r3 = wide.tile([128, W], F16, tag="theta", name="r3")
                vec.tensor_mul(r3[:], mim[:], qre[:])
                r4 = wide.tile([128, W], F16, tag="ab", name="r4")
                vec.tensor_mul(r4[:], mre[:], qim[:])
                rim = retp.tile([128, W], F16, tag=f"ret_im_{h}",
                                name=f"ret_im_{h}")
                vec.tensor_add(rim[:], r3[:], r4[:])
                ret_w[(h, "re")] = rre
                ret_w[(h, "im")] = rim

            if prev is not None:
                emit_tail_mm(prev)
            prev = {"ret": ret_w, "tok": tok}

        emit_tail_pre(prev)
        emit_tail_mm(prev)

    return nc


# --------------------------------------------------------------------------
# Host-side sharding / unsharding
# --------------------------------------------------------------------------
def shard_inputs(cfg, x, W_in, W_out, ln_gamma, ln_beta, pos_phases):
    c = cfg
    HD = N_CORES * c.NCH
    xT = np.ascontiguousarray(x.reshape(c.NTOK, c.DIM).T)          # [DIM, NTOK]
    # [p, k, tok] partition-major so one DMA covers all k-tiles of a chunk
    xt_h = np.ascontiguousarray(
        xT.reshape(c.KT1, 128, c.NTOK).transpose(1, 0, 2)
    ).astype(ml_dtypes.bfloat16)

    pos64 = pos_phases.astype(np.float64)
    cos_p = (0.5 * np.cos(pos64)).astype(np.float16)               # [T, HD]
    sin_p = (0.5 * np.sin(pos64)).astype(np.float16)

    Wg = (W_out * ln_gamma[None, :]).astype(np.float32)            # [DIM, 2HD]

    in_maps = []
    for cid in range(N_CORES):
        h0 = cid * c.NCH
        hs = slice(h0, h0 + c.NCH)
        w_ph = W_in[0 * HD + h0:0 * HD + h0 + c.NCH]               # [NCH, DIM]
        w_mg = W_in[1 * HD + h0:1 * HD + h0 + c.NCH]
        w_qr = W_in[2 * HD + h0:2 * HD + h0 + c.NCH]
        w_qi = W_in[3 * HD + h0:3 * HD + h0 + c.NCH]
        w_all = np.concatenate([w_ph, w_mg, w_qr, w_qi], axis=0)   # [4NCH, DIM]
        w1_h = np.ascontiguousarray(
            w_all.T.reshape(c.KT1, 128, 4 * c.NCH).transpose(1, 0, 2)
        ).astype(ml_dtypes.bfloat16)

        wg_re = Wg[:, 2 * h0:2 * (h0 + c.NCH):2]                   # [DIM, NCH]
        wg_im = Wg[:, 2 * h0 + 1:2 * (h0 + c.NCH):2]
        w2T = np.concatenate([wg_re.T, wg_im.T], axis=0)           # [2NCH, DIM]
        w2_h = np.ascontiguousarray(
            w2T.reshape(c.KT2, 128, c.DIM).transpose(1, 0, 2)
        ).astype(ml_dtypes.bfloat16)

        cp_h = np.ascontiguousarray(
            cos_p[:, hs].T.reshape(c.CT, 128, c.T).transpose(1, 0, 2))
        sp_h = np.ascontiguousarray(
            sin_p[:, hs].T.reshape(c.CT, 128, c.T).transpose(1, 0, 2))

        in_maps.append({
            "w1": w1_h, "w2": w2_h, "xt": xt_h,
            "cp": cp_h, "sp": sp_h,
        })
    return in_maps


def combine_outputs(cfg, results, W_out, ln_gamma, ln_beta, x_dtype):
    c = cfg
    NF = 2 * N_CORES * c.NCH
    P = np.zeros((c.DIM, c.NTOK), np.float64)
    S1 = np.zeros(c.NTOK, np.float64)
    S2 = np.zeros(c.NTOK, np.float64)
    for r in results:
        # outp is [128, DT, NTOK] partition-major of out^T -> [DIM, NTOK]
        op = r["outp"].transpose(1, 0, 2).reshape(c.DIM, c.NTOK)
        P += op.astype(np.float64)
        S1 += r["stats"][0].astype(np.float64)
        S2 += r["stats"][1].astype(np.float64)
    mu = S1 / NF
    var = S2 / NF - mu * mu
    istd = 1.0 / np.sqrt(var + LN_EPS)
    wg_sum = (W_out.astype(np.float64) @ ln_gamma.astype(np.float64))  # [DIM]
    b_out = (W_out.astype(np.float64) @ ln_beta.astype(np.float64))    # [DIM]
    out = istd[:, None] * (P.T - mu[:, None] * wg_sum[None, :]) + b_out[None, :]
    return out.reshape(c.B, c.T, c.DIM).astype(x_dtype)


def shard_inputs_v2(cfg, x, W_in, W_out, ln_gamma, ln_beta, pos_phases):
    c = cfg
    HD = N_CORES * c.NCH
    xT = np.ascontiguousarray(x.reshape(c.NTOK, c.DIM).T)          # [DIM, NTOK]
    xt_h = np.ascontiguousarray(
        xT.reshape(c.KT1, 128, c.NTOK).transpose(1, 0, 2)
    ).astype(np.float16)
    x8_h = np.ascontiguousarray(
        (xT * 16.0).reshape(c.KT1, 128, c.NTOK).transpose(1, 0, 2)
    ).astype(ml_dtypes.float8_e4m3)

    # pos/2, wrapped to [-pi/2, pi/2): theta_half = pi/2*tanh(ph) + pos/2
    pos64 = pos_phases.astype(np.float64)
    pos_half = (0.5 * (np.mod(pos64 + np.pi, 2 * np.pi) - np.pi)
                ).astype(np.float16)                               # [T, HD]

    Wg = (W_out * ln_gamma[None, :]).astype(np.float32)            # [DIM, 2HD]

    in_maps = []
    for cid in range(N_CORES):
        h0 = cid * c.NCH
        hs = slice(h0, h0 + c.NCH)
        w_ph = W_in[0 * HD + h0:0 * HD + h0 + c.NCH]
        w_mg = W_in[1 * HD + h0:1 * HD + h0 + c.NCH]
        w_qr = W_in[2 * HD + h0:2 * HD + h0 + c.NCH]
        w_qi = W_in[3 * HD + h0:3 * HD + h0 + c.NCH]
        w_all = np.concatenate([w_ph, w_mg, w_qr, w_qi], axis=0)   # [4NCH, DIM]
        w1_h = np.ascontiguousarray(
            w_all.T.reshape(c.KT1, 128, 4 * c.NCH).transpose(1, 0, 2)
        ).astype(np.float16)

        wg_re = Wg[:, 2 * h0:2 * (h0 + c.NCH):2]                   # [DIM, NCH]
        wg_im = Wg[:, 2 * h0 + 1:2 * (h0 + c.NCH):2]
        w2T = np.concatenate([wg_re.T, wg_im.T], axis=0)           # [2NCH, DIM]
        w2_h = np.ascontiguousarray(
            w2T.reshape(c.KT2, 128, c.DIM).transpose(1, 0, 2)
        ).astype(np.float16)

        pos_h = np.ascontiguousarray(
            pos_half[:, hs].T.reshape(c.CT, 128, c.T).transpose(1, 0, 2))

        # fp8(e4m3) copies for the magnitude channel (scales folded into
        # the on-chip tanh input scale: 0.5/(16*64)).
        w13_full = np.concatenate([w_ph, w_qr, w_qi], axis=0)
        w13_h = np.ascontiguousarray(
            w13_full.T.reshape(c.KT1, 128, 3 * c.NCH).transpose(1, 0, 2)
        ).astype(np.float16)
        wm8 = (w_mg.astype(np.float32) * 64.0).astype(
            ml_dtypes.float8_e4m3).astype(ml_dtypes.float8_e4m3)
        # layout [128, KT1//2, 2, NCH]: plane i of pair kk is k-tile 2kk+i
        wm8_h = np.ascontiguousarray(
            wm8.T.reshape(c.KT1 // 2, 2, 128, c.NCH).transpose(2, 0, 1, 3))
        in_maps.append({"w1": w1_h, "w2": w2_h, "xt": xt_h, "pos": pos_h,
                        "w13": w13_h, "w1m8": wm8_h, "x8": x8_h})
    return in_maps


def combine_outputs_v2(cfg, results, W_out, ln_gamma, ln_beta, x_dtype):
    c = cfg
    NF = 2 * N_CORES * c.NCH
    P = np.zeros((c.DIM, c.NTOK), np.float64)
    S1 = np.zeros(c.NTOK, np.float64)
    S2 = np.zeros(c.NTOK, np.float64)
    for r in results:
        op = r["outp"].transpose(1, 0, 2).reshape(c.DIM, c.NTOK)
        P += op.astype(np.float64)
        S1 += r["stats"][0].astype(np.float64)
        S2 += r["stats"][1].astype(np.float64)
    mu = S1 / NF
    var = S2 / NF - mu * mu
    istd = 1.0 / np.sqrt(var + LN_EPS)
    wg_sum = (W_out.astype(np.float64) @ ln_gamma.astype(np.float64))
    b_out = (W_out.astype(np.float64) @ ln_beta.astype(np.float64))
    out = istd[:, None] * (P.T - mu[:, None] * wg_sum[None, :]) + b_out[None, :]
    return out.reshape(c.B, c.T, c.DIM).astype(x_dtype)


import os

# Production configuration: fp8(e4m3) DoubleRow matmuls for the sigmoid
# magnitude channel (validated rel err 0.011 < 2e-2 on hardware).
DEFAULT_PROBE = "fp8mag"


def _active_build(cfg, reps=1, hw_reps=1, probe=None):
    env = os.environ.get("KERNEL_PROBE")
    base = DEFAULT_PROBE if env is None else env
    merged = ",".join(x for x in [base, probe or ""] if x) or None
    return build_program_v2(cfg, reps=reps, hw_reps=hw_reps, probe=merged)


# Active implementation selector (test.py/bench use these too)
BUILD = _active_build
SHARD = shard_inputs_v2
COMBINE = combine_outputs_v2
CN_ACTIVE = 256

_cached = {}


def kernel(x, W_in, W_out, ln_gamma, ln_beta, pos_phases):
    cfg = Cfg(B=x.shape[0], T=x.shape[1], DIM=x.shape[2],
              NCH=pos_phases.shape[1] // N_CORES, CN=CN_ACTIVE)
    key = (cfg.B, cfg.T, cfg.DIM, cfg.NCH)
    if key not in _cached:
        nc = BUILD(cfg)
        split_multiwait(nc)  # walrus workaround; CoreSim path must skip this
        _cached[key] = nc
    nc = _cached[key]
    in_maps = SHARD(cfg, np.asarray(x), np.asarray(W_in),
                    np.asarray(W_out), np.asarray(ln_gamma),
                    np.asarray(ln_beta), np.asarray(pos_phases))
    # the native run path rejects in_map keys the program doesn't declare
    declared = {a.memorylocations[0].name
                for a in nc.m.functions[0].allocations
                if isinstance(a, mybir.MemoryLocationSet)
                and a.kind == "ExternalInput"}
    in_maps = [{k: v for k, v in m.items() if k in declared} for m in in_maps]
    res = run_bass_kernel_spmd(nc, in_maps, list(range(N_CORES)))
    return COMBINE(cfg, res.results, np.asarray(W_out),
                   np.asarray(ln_gamma), np.asarray(ln_beta),
                   np.asarray(x).dtype)



# revision 5
# speedup vs baseline: 1.3724x; 1.3724x over previous
"""Trainium2 Bass kernel for nn_LongAttention (holographic long-attention block).

Computation (see reference):
  raw = x @ W_in.T -> split [c_phase | c_mag | q_re | q_im] per hd channel
  key = sigmoid(c_mag) * exp(i*(pi*tanh(c_phase) + pos_phase))
  state = cumsum_t(key);  ret = state * conj(q)
  ret_real = interleave(Re, Im) -> LayerNorm(2*hd) -> @ W_out.T

Distribution: hd (8192) split across 8 NeuronCores (1024 ch each); every core
handles both batches and all tokens; cores are fully independent. gamma is
folded into W_out on the host and the LayerNorm is algebraically deferred:
each core returns P = ret @ (W_out*gamma).T partials plus per-token
S1 = sum_f ret, S2 = sum_f ret^2; the host combines
out = istd * (sum_c P_c - mu * (W_out @ gamma)) + W_out @ beta.

Active implementation (build_program_v2, CN=256-token chunks):
 - f16 datapath end to end (matmul inputs, elementwise, scan output) --
   same speed as bf16 everywhere but ~8x finer mantissa, plus 2x DVE
   perf-modes on the 16-bit elementwise ops.
 - The magnitude-channel GEMM runs in fp8(e4m3) with perf_mode=DoubleRow
   (2 k-planes per instruction); the quantization scales (x*16, W*64) are
   folded into the on-chip tanh input scale. Sigmoid's 1/4 slope damps the
   fp8 noise; measured end-to-end rel err 0.011 < 2e-2.
 - sin/cos via the half-angle identity: th = (pi/2)*tanh(ph) + pos/2 with
   pos pre-wrapped to [-pi, pi) on the host, so |th| <= pi stays inside
   the ACT Sin LUT range; cos(2th) = 1-2*sin^2(th) gives the real part
   without a second LUT pass over an out-of-range argument.
 - The cumsum runs channel-major on the DVE as a prefix scan along the free
   (time) axis (fp32 internal state), carried across token chunks.
 - Per-token LN stats are folded on DVE/ACT (tree adds + squares) into one
   [128, 2*CN] tile and reduced across partitions by a single pair of
   ones-matmuls -- instead of 32 PE matmuls per chunk.
 - stats + proj_out for chunk n-1 are emitted during chunk n (software
   pipelining) so the in-order PE queue never waits on the chunk's serial
   ACT<->DVE elementwise chain; all hot pools are double-buffered.
"""

import sys
import numpy as np
import ml_dtypes

for _p in ("/opt/trn_rl_repo", "/root/.axon_site/_ro/trn_rl_repo"):
    if _p not in sys.path:
        sys.path.append(_p)

import bass_rust
import concourse.bass as bass
import concourse.tile as tile
import concourse.mybir as mybir
from concourse.bass_utils import run_bass_kernel_spmd

F32 = mybir.dt.float32
F8 = mybir.dt.float8e4
F16 = mybir.dt.float16
BF16 = mybir.dt.bfloat16
AF = mybir.ActivationFunctionType
ALU = mybir.AluOpType
PI = float(np.pi)

N_CORES = 8
LN_EPS = 1e-5


# --------------------------------------------------------------------------
# Workaround: this container's walrus rejects >1 semaphore wait per
# instruction ("Too many sync wait commands"). Split the extras onto
# same-engine NoOps inserted just before (engine FIFO keeps semantics).
# --------------------------------------------------------------------------
_nop_counter = [0]


def split_multiwait(nc):
    n_split = 0
    for f in nc.m.functions:
        for bb in f.blocks:
            il = bb.instructions
            i = 0
            while i < len(il):
                ins = il[i]
                si = ins.sync_info
                waits = list(si.on_wait) if si is not None and si.on_wait else []
                if len(waits) > 1:
                    for w in waits[:-1]:
                        _nop_counter[0] += 1
                        nop = bass_rust.InstNoOp(
                            name=f"mw_nop_{_nop_counter[0]}",
                            engine=ins.engine,
                            ins=[],
                            outs=[],
                        )
                        nop.sync_info = mybir.SyncInfo(on_wait=[w], on_update=[])
                        il.insert(i, nop)
                        i += 1
                    si.on_wait = [waits[-1]]
                    n_split += 1
                i += 1
    return n_split


# --------------------------------------------------------------------------
# Device program (SPMD: identical on all cores; per-core data differs)
# --------------------------------------------------------------------------
class Cfg:
    def __init__(self, B=2, T=2048, DIM=1024, NCH=1024, CN=256):
        self.B, self.T, self.DIM, self.NCH, self.CN = B, T, DIM, NCH, CN
        self.NTOK = B * T
        self.CT = NCH // 128          # channel tiles per core
        self.KT1 = DIM // 128         # contraction tiles for proj_in
        self.KT2 = 2 * self.CT        # contraction tiles for proj_out (re+im)
        self.DT = DIM // 128          # output dim tiles
        self.NCHUNK = self.NTOK // CN
        self.CPB = T // CN            # chunks per batch


def build_program(cfg: Cfg, reps: int = 1, hw_reps: int = 1,
                  probe: str | None = None):
    c = cfg
    assert c.CT % 4 == 0 or c.CT == 2
    SEGS = 4 if c.CT % 4 == 0 else 2   # channel tiles per wide tile
    NH = c.CT // SEGS                  # wide halves per chunk
    W = SEGS * c.CN                    # wide tile width
    nc = bass.Bass()

    class _Dup:
        def __init__(self, eng, on):
            self._eng, self._on = eng, on

        def __getattr__(self, n):
            f = getattr(self._eng, n)
            if not self._on:
                return f

            def g(*a, **k):
                r = f(*a, **k)
                f(*a, **k)
                return r
            return g

    pset = set(probe.split(",")) if probe else set()
    vec = _Dup(nc.vector, "dve2" in pset)
    sca = _Dup(nc.scalar, "act2" in pset)
    kt1_lim = c.KT1 // 2 if "pein_half" in pset else c.KT1
    kt2_lim = c.KT2 // 2 if "peout_half" in pset else c.KT2
    stats_on = "stats_off" not in pset

    w1 = nc.dram_tensor("w1", [128, c.KT1, 4 * c.NCH], BF16, kind="ExternalInput")
    w2 = nc.dram_tensor("w2", [128, c.KT2, c.DIM], BF16, kind="ExternalInput")
    xt = nc.dram_tensor("xt", [128, c.KT1, c.NTOK], BF16, kind="ExternalInput")
    cp = nc.dram_tensor("cp", [128, c.CT, c.T], F16, kind="ExternalInput")
    sp = nc.dram_tensor("sp", [128, c.CT, c.T], F16, kind="ExternalInput")
    outp = nc.dram_tensor("outp", [128, c.DT, c.NTOK], F32, kind="ExternalOutput")
    stats = nc.dram_tensor("stats", [2, c.NTOK], F32, kind="ExternalOutput")

    from contextlib import ExitStack
    with tile.TileContext(nc) as tc, ExitStack() as es:
        consts = es.enter_context(tc.tile_pool(name="consts", bufs=1))
        stream = es.enter_context(tc.tile_pool(name="stream", bufs=2))
        wide = es.enter_context(tc.tile_pool(name="wide", bufs=1))
        retp = es.enter_context(tc.tile_pool(name="retp", bufs=2))
        obp = es.enter_context(tc.tile_pool(name="obp", bufs=1))
        stc = es.enter_context(tc.tile_pool(name="stc", bufs=2))
        praw = es.enter_context(tc.tile_pool(name="praw", bufs=4, space="PSUM"))
        pstat = es.enter_context(tc.tile_pool(name="pstat", bufs=1, space="PSUM"))
        pstat2 = es.enter_context(tc.tile_pool(name="pstat2", bufs=1, space="PSUM"))
        pout = es.enter_context(tc.tile_pool(name="pout", bufs=2, space="PSUM"))

        w1_sb = consts.tile([128, c.KT1, 4 * c.NCH], BF16, tag="w1_sb")
        nc.sync.dma_start(out=w1_sb[:], in_=w1[:])
        w2_sb = consts.tile([128, c.KT2, c.DIM], BF16, tag="w2_sb")
        nc.sync.dma_start(out=w2_sb[:], in_=w2[:])
        w1_t = [w1_sb[:, k, :] for k in range(c.KT1)]
        w2_t = [w2_sb[:, k, :] for k in range(c.KT2)]

        ones_bf = consts.tile([128, 1], BF16, tag="ones")
        vec.memset(ones_bf[:], 1.0)
        half_pi = consts.tile([128, 1], F32, tag="half_pi")
        vec.memset(half_pi[:], PI / 2)
        car = {}
        for h in range(NH):
            for pl in ("re", "im"):
                car[(h, pl)] = consts.tile([128, SEGS], F32, tag=f"car_{h}_{pl}",
                                           name=f"car_{h}_{pl}")

        if hw_reps > 1:
            es.enter_context(tc.For_i(0, hw_reps))

        for n in [nn_ for _ in range(reps) for nn_ in range(c.NCHUNK)]:
            t0 = (n % c.CPB) * c.CN
            first_in_batch = t0 == 0
            tok = slice(n * c.CN, (n + 1) * c.CN)

            xcb = stream.tile([128, c.KT1, c.CN], BF16, tag="xcb")
            nc.sync.dma_start(out=xcb[:], in_=xt[:, :, tok])
            xc = [xcb[:, k, :] for k in range(c.KT1)]
            cpb = stream.tile([128, c.CT, c.CN], F16, tag="cpb")
            nc.sync.dma_start(out=cpb[:], in_=cp[:, :, t0:t0 + c.CN])
            spb = stream.tile([128, c.CT, c.CN], F16, tag="spb")
            nc.sync.dma_start(out=spb[:], in_=sp[:, :, t0:t0 + c.CN])

            ret_w = {}
            for h in range(NH):
                i0 = h * SEGS
                # ---- proj_in: 4 groups x SEGS channel tiles -> psum pairs ----
                # psum tile [128, 2*CN] holds channel tiles (j, j+1) of a group
                th_ph = wide.tile([128, W], F32, tag="th_ph", name="th_ph")
                th_mg = wide.tile([128, W], F32, tag="th_mg", name="th_mg")
                qre = wide.tile([128, W], F32, tag="qre", name="qre")
                qim = wide.tile([128, W], F32, tag="qim", name="qim")
                dest = {"ph": th_ph, "mg": th_mg, "qr": qre, "qi": qim}
                for j in range(0, SEGS, 2):
                    for gi, g in enumerate(("ph", "mg", "qr", "qi")):
                        p = praw.tile([128, 2 * c.CN], F32, tag="praw")
                        for half in range(2):
                            m = gi * c.CT + i0 + j + half
                            cols = slice(half * c.CN, (half + 1) * c.CN)
                            for k in range(kt1_lim):
                                nc.tensor.matmul(
                                    p[:, cols],
                                    w1_t[k][:, m * 128:(m + 1) * 128], xc[k],
                                    start=(k == 0), stop=(k == kt1_lim - 1))
                        wcols = slice(j * c.CN, (j + 2) * c.CN)
                        if g == "ph" or g == "mg":
                            sc = 1.0 if g == "ph" else 0.5
                            sca.activation(dest[g][:, wcols], p[:],
                                                 AF.Tanh, scale=sc)
                        elif "qdve" in pset:
                            vec.tensor_copy(dest[g][:, wcols], p[:])
                        else:
                            sca.copy(dest[g][:, wcols], p[:])

                # ---- content phasor (wide) ----
                sinp = wide.tile([128, W], F32, tag="sinp", name="sinp")
                sca.activation(sinp[:], th_ph[:], AF.Sin, scale=PI)
                tabs = wide.tile([128, W], F32, tag="tabs", name="tabs")
                sca.activation(tabs[:], th_ph[:], AF.Abs)
                cosp = wide.tile([128, W], F32, tag="th_ph", name="cosp")
                sca.activation(cosp[:], tabs[:], AF.Sin,
                                     bias=half_pi[:], scale=-PI)
                # 2*sigma = th_mg + 1 ; the 0.5 is folded into cp/sp on host
                ssin = wide.tile([128, W], F32, tag="tabs", name="ssin")
                vec.scalar_tensor_tensor(ssin[:], th_mg[:], 1.0, sinp[:],
                                               ALU.add, ALU.mult)
                scos = wide.tile([128, W], F32, tag="sinp", name="scos")
                vec.scalar_tensor_tensor(scos[:], th_mg[:], 1.0, cosp[:],
                                               ALU.add, ALU.mult)

                # ---- key = content * pos phasor (wide, cp/sp pre-halved) ----
                cps = cpb[:, i0:i0 + SEGS, :]
                sps = spb[:, i0:i0 + SEGS, :]
                ta = wide.tile([128, W], F32, tag="tmp1", name="ta")
                vec.tensor_mul(ta[:], scos[:], cps)
                tb = wide.tile([128, W], F32, tag="tmp2", name="tb")
                vec.tensor_mul(tb[:], ssin[:], sps)
                kre = wide.tile([128, W], F32, tag="kre", name="kre")
                vec.tensor_sub(kre[:], ta[:], tb[:])
                tc_ = wide.tile([128, W], F32, tag="tmp1", name="tc_")
                vec.tensor_mul(tc_[:], ssin[:], cps)
                td = wide.tile([128, W], F32, tag="tmp2", name="td")
                vec.tensor_mul(td[:], scos[:], sps)
                kim = wide.tile([128, W], F32, tag="kim", name="kim")
                vec.tensor_add(kim[:], tc_[:], td[:])

                # ---- prefix scan per channel tile segment ----
                mre = wide.tile([128, W], F32, tag="mre", name="mre")
                mim = wide.tile([128, W], F32, tag="mim", name="mim")
                for s in range(SEGS):
                    seg = slice(s * c.CN, (s + 1) * c.CN)
                    init_re = 0.0 if first_in_batch else car[(h, "re")][:, s:s + 1]
                    vec.tensor_tensor_scan(mre[:, seg], kre[:, seg],
                                                 kre[:, seg], init_re,
                                                 ALU.add, ALU.bypass)
                    init_im = 0.0 if first_in_batch else car[(h, "im")][:, s:s + 1]
                    vec.tensor_tensor_scan(mim[:, seg], kim[:, seg],
                                                 kim[:, seg], init_im,
                                                 ALU.add, ALU.bypass)
                if (n % c.CPB) != c.CPB - 1:
                    cre = mre.rearrange("p (s t) -> p s t", s=SEGS)[:, :, c.CN - 1]
                    vec.tensor_copy(car[(h, "re")][:], cre)
                    cim = mim.rearrange("p (s t) -> p s t", s=SEGS)[:, :, c.CN - 1]
                    vec.tensor_copy(car[(h, "im")][:], cim)

                # ---- retrieval = state * conj(q) (wide) ----
                r1 = wide.tile([128, W], F32, tag="tmp1", name="r1")
                vec.tensor_mul(r1[:], mre[:], qre[:])
                r2 = wide.tile([128, W], F32, tag="tmp2", name="r2")
                vec.tensor_mul(r2[:], mim[:], qim[:])
                rre = retp.tile([128, W], BF16, tag=f"ret_re_{h}",
                                name=f"ret_re_{h}")
                vec.tensor_add(rre[:], r1[:], r2[:])
                r3 = wide.tile([128, W], F32, tag="tmp1", name="r3")
                vec.tensor_mul(r3[:], mim[:], qre[:])
                r4 = wide.tile([128, W], F32, tag="tmp2", name="r4")
                vec.tensor_mul(r4[:], mre[:], qim[:])
                rim = retp.tile([128, W], BF16, tag=f"ret_im_{h}",
                                name=f"ret_im_{h}")
                vec.tensor_sub(rim[:], r3[:], r4[:])
                ret_w[(h, "re")] = rre
                ret_w[(h, "im")] = rim

            # ---- per-token stats via ones-matmuls ----
            ps1 = pstat.tile([1, c.CN], F32, tag="ps1")
            ps2 = pstat2.tile([1, c.CN], F32, tag="ps2")
            n_st = 2 * c.CT
            idx = 0
            for h in range(NH):
                for pl in ("re", "im"):
                    rw = ret_w[(h, pl)]
                    sq = wide.tile([128, W], BF16, tag="sq", name="sq")
                    vec.tensor_mul(sq[:], rw[:], rw[:])
                    if not stats_on:
                        continue
                    for s in range(SEGS):
                        seg = slice(s * c.CN, (s + 1) * c.CN)
                        nc.tensor.matmul(ps1[:], ones_bf[:], rw[:, seg],
                                         start=(idx == 0), stop=(idx == n_st - 1))
                        nc.tensor.matmul(ps2[:], ones_bf[:], sq[:, seg],
                                         start=(idx == 0), stop=(idx == n_st - 1))
                        idx += 1
            if not stats_on:
                nc.tensor.matmul(ps1[:], ones_bf[:], ret_w[(0, "re")][:, 0:c.CN],
                                 start=True, stop=True)
                nc.tensor.matmul(ps2[:], ones_bf[:], ret_w[(0, "im")][:, 0:c.CN],
                                 start=True, stop=True)
            s1c = stc.tile([1, c.CN], F32, tag="s1c", name="s1c")
            sca.copy(s1c[:], ps1[:])
            nc.sync.dma_start(out=stats[0:1, tok], in_=s1c[:])
            s2c = stc.tile([1, c.CN], F32, tag="s2c", name="s2c")
            sca.copy(s2c[:], ps2[:])
            nc.sync.dma_start(out=stats[1:2, tok], in_=s2c[:])

            # ---- proj_out partial (accumulate over all chpl tiles) ----
            ob = obp.tile([128, c.DT, c.CN], F32, tag="ob", name="ob")
            for d in range(c.DT):
                po = pout.tile([128, c.CN], F32, tag="pout")
                for k in range(kt2_lim):
                    if k < c.CT:
                        h, s, pl = k // SEGS, k % SEGS, "re"
                    else:
                        h, s, pl = (k - c.CT) // SEGS, (k - c.CT) % SEGS, "im"
                    rt = ret_w[(h, pl)][:, s * c.CN:(s + 1) * c.CN]
                    nc.tensor.matmul(po[:], w2_t[k][:, d * 128:(d + 1) * 128],
                                     rt, start=(k == 0), stop=(k == kt2_lim - 1))
                sca.copy(ob[:, d, :], po[:])
            nc.sync.dma_start(out=outp[:, :, tok], in_=ob[:])

    return nc


def build_program_v2(cfg: Cfg, reps: int = 1, hw_reps: int = 1,
                     probe: str | None = None):
    """v2: f16 datapath, CN=512, double-angle sin/cos (one pos tensor),
    stats folded on DVE (single ones-matmul per chunk), engine-balanced.

    key = sigma(mg)*exp(i*theta), theta = pi*tanh(ph) + pos.
    With th = theta/2 = (pi/2)*tanh(ph) + pos/2 (|th| <= pi, LUT-valid):
      sh = sin(th), ch = cos(th) = sin(pi/2 - |th|)
      kreN = (sh^2 - 0.5)*mg2 = -sigma*cos(theta)   (mg2 = tanh(mg/2)+1 = 2*sigma)
      kim  = sh*ch*mg2        =  sigma*sin(theta)
    The negated real part flows through the scan (SreN = -Sre); retrieval
    compensates: rre = mim*qim - mreN*qre ; rim = mim*qre + mreN*qim.
    """
    c = cfg
    assert c.CT % 4 == 0
    SEGS = 4
    NH = c.CT // SEGS
    W = SEGS * c.CN
    nc = bass.Bass()

    class _Dup:
        def __init__(self, eng, on):
            self._eng, self._on = eng, on

        def __getattr__(self, n):
            f = getattr(self._eng, n)
            if not self._on:
                return f

            def g(*a, **k):
                r = f(*a, **k)
                f(*a, **k)
                return r
            return g

    pset = set(probe.split(",")) if probe else set()
    vec = _Dup(nc.vector, "dve2" in pset)
    sca = _Dup(nc.scalar, "act2" in pset)
    kt1_lim = c.KT1 // 2 if "pein_half" in pset else c.KT1
    kt2_lim = c.KT2 // 2 if "peout_half" in pset else c.KT2

    fp8mag = "fp8mag" in pset
    pin2 = "pin512" in pset
    if fp8mag:
        w13 = nc.dram_tensor("w13", [128, c.KT1, 3 * c.NCH], F16,
                             kind="ExternalInput")
        w1m8 = nc.dram_tensor("w1m8", [128, c.KT1 // 2, 2, c.NCH], F8,
                              kind="ExternalInput")
        x8 = nc.dram_tensor("x8", [128, c.KT1, c.NTOK], F8,
                            kind="ExternalInput")
    else:
        w1 = nc.dram_tensor("w1", [128, c.KT1, 4 * c.NCH], F16,
                            kind="ExternalInput")
    w2 = nc.dram_tensor("w2", [128, c.KT2, c.DIM], F16, kind="ExternalInput")
    xt = nc.dram_tensor("xt", [128, c.KT1, c.NTOK], F16, kind="ExternalInput")
    pos = nc.dram_tensor("pos", [128, c.CT, c.T], F16, kind="ExternalInput")
    outp = nc.dram_tensor("outp", [128, c.DT, c.NTOK], BF16,
                          kind="ExternalOutput")
    stats = nc.dram_tensor("stats", [2, c.NTOK], F32, kind="ExternalOutput")

    from contextlib import ExitStack
    with tile.TileContext(nc) as tc, ExitStack() as es:
        small = c.CN <= 256
        praw_bufs = 4 if (small and not pin2) else 2
        pout_bufs = 2
        if "praw3" in pset:
            praw_bufs, pout_bufs = (6, 2) if small else (3, 1)
        consts = es.enter_context(tc.tile_pool(name="consts", bufs=1))
        stream = es.enter_context(tc.tile_pool(name="stream", bufs=2))
        wide_bufs = 2 if small else 1
        for p_ in pset:
            if p_.startswith("wb"):
                wide_bufs = int(p_[2:])
        wide = es.enter_context(tc.tile_pool(name="wide", bufs=wide_bufs))
        retp = es.enter_context(tc.tile_pool(name="retp", bufs=2 if small else 1))
        obp = es.enter_context(tc.tile_pool(name="obp",
                                            bufs=1 if (pin2 or not small) else 2))
        stb = es.enter_context(tc.tile_pool(name="stb", bufs=1 if pin2 else 2))
        stc = es.enter_context(tc.tile_pool(name="stc", bufs=1 if pin2 else 2))
        praw = es.enter_context(tc.tile_pool(name="praw", bufs=praw_bufs,
                                             space="PSUM"))
        pstat = es.enter_context(tc.tile_pool(name="pstat", bufs=1, space="PSUM"))
        pout = es.enter_context(tc.tile_pool(name="pout", bufs=pout_bufs,
                                             space="PSUM"))

        if fp8mag:
            w1_sb = consts.tile([128, c.KT1, 3 * c.NCH], F16, tag="w1_sb")
            nc.sync.dma_start(out=w1_sb[:], in_=w13[:])
            w1m8_sb = consts.tile([128, c.KT1 // 2, 2, c.NCH], F8,
                                  tag="w1m8_sb")
            nc.sync.dma_start(out=w1m8_sb[:], in_=w1m8[:])
        else:
            w1_sb = consts.tile([128, c.KT1, 4 * c.NCH], F16, tag="w1_sb")
            nc.sync.dma_start(out=w1_sb[:], in_=w1[:])
        w2_sb = consts.tile([128, c.KT2, c.DIM], F16, tag="w2_sb")
        nc.sync.dma_start(out=w2_sb[:], in_=w2[:])
        w1_t = [w1_sb[:, k, :] for k in range(c.KT1)]
        w2_t = [w2_sb[:, k, :] for k in range(c.KT2)]

        ones_bf = consts.tile([128, 1], BF16, tag="ones")
        vec.memset(ones_bf[:], 1.0)
        one_f = consts.tile([128, 1], F32, tag="one_f")
        vec.memset(one_f[:], 1.0)
        half_pi = consts.tile([128, 1], F32, tag="half_pi")
        vec.memset(half_pi[:], PI / 2)
        car = {}
        for h in range(NH):
            for pl in ("re", "im"):
                car[(h, pl)] = consts.tile([128, SEGS], F16, tag=f"car_{h}_{pl}",
                                           name=f"car_{h}_{pl}")

        if hw_reps > 1:
            es.enter_context(tc.For_i(0, hw_reps))

        h2 = W // 2

        def emit_tail_pre(st):
            """DVE folds + ACT squares for the previous chunk's stats."""
            rw = st["ret"]
            a = wide.tile([128, W], F16, tag="stA", bufs=1, name="a")
            vec.tensor_add(a[:], rw[(0, "re")][:], rw[(0, "im")][:])
            b = wide.tile([128, W], F16, tag="stB", bufs=1, name="b")
            vec.tensor_add(b[:], rw[(1, "re")][:], rw[(1, "im")][:])
            rs = wide.tile([128, W], F16, tag="stC", bufs=1, name="rs")
            vec.tensor_add(rs[:], a[:], b[:])
            f = wide.tile([128, h2], F16, tag="stD", bufs=1, name="f")
            vec.tensor_add(f[:], rs[:, 0:h2], rs[:, h2:W])
            stt = stb.tile([128, 2 * c.CN], BF16, tag="stt", name="stt")
            vec.tensor_add(stt[:, 0:c.CN], f[:, 0:c.CN], f[:, c.CN:h2])
            def _sq(dst, src):
                if "sqdve" in pset:
                    vec.tensor_mul(dst, src, src)
                else:
                    sca.activation(dst, src, AF.Square)
            s0 = wide.tile([128, W], BF16, tag="stA", bufs=1, name="s0")
            _sq(s0[:], rw[(0, "re")][:])
            s1 = wide.tile([128, W], BF16, tag="stB", bufs=1, name="s1")
            _sq(s1[:], rw[(0, "im")][:])
            a2 = wide.tile([128, W], BF16, tag="stC", bufs=1, name="a2")
            vec.tensor_add(a2[:], s0[:], s1[:])
            s2 = wide.tile([128, W], BF16, tag="stA", bufs=1, name="s2")
            _sq(s2[:], rw[(1, "re")][:])
            s3 = wide.tile([128, W], BF16, tag="stB", bufs=1, name="s3")
            _sq(s3[:], rw[(1, "im")][:])
            b2 = wide.tile([128, W], BF16, tag="stD", bufs=1, name="b2")
            vec.tensor_add(b2[:], s2[:], s3[:])
            ss = wide.tile([128, W], BF16, tag="stA", bufs=1, name="ss")
            vec.tensor_add(ss[:], a2[:], b2[:])
            f2 = wide.tile([128, h2], BF16, tag="stB", bufs=1, name="f2")
            vec.tensor_add(f2[:], ss[:, 0:h2], ss[:, h2:W])
            vec.tensor_add(stt[:, c.CN:2 * c.CN], f2[:, 0:c.CN],
                           f2[:, c.CN:h2])
            st["stt"] = stt

        def emit_tail_mm(st):
            """Stats matmul + proj_out for the previous chunk."""
            stt, tok_p, rw = st["stt"], st["tok"], st["ret"]
            ps = pstat.tile([1, 2 * c.CN], F32, tag="ps")
            nc.tensor.matmul(ps[:, 0:c.CN], ones_bf[:], stt[:, 0:c.CN],
                             start=True, stop=True)
            nc.tensor.matmul(ps[:, c.CN:2 * c.CN], ones_bf[:],
                             stt[:, c.CN:2 * c.CN], start=True, stop=True)
            sc = stc.tile([1, 2 * c.CN], F32, tag="sc", name="sc")
            sca.copy(sc[:], ps[:])
            nc.sync.dma_start(out=stats[0:1, tok_p], in_=sc[:, 0:c.CN])
            nc.sync.dma_start(out=stats[1:2, tok_p], in_=sc[:, c.CN:2 * c.CN])
            ob = obp.tile([128, c.DT, c.CN], BF16, tag="ob", name="ob")
            for d in range(c.DT):
                po = pout.tile([128, c.CN], F32, tag="pout")
                for k in range(kt2_lim):
                    if k < c.CT:
                        h, s, pl = k // SEGS, k % SEGS, "re"
                    else:
                        h, s, pl = (k - c.CT) // SEGS, (k - c.CT) % SEGS, "im"
                    rt = rw[(h, pl)][:, s * c.CN:(s + 1) * c.CN]
                    nc.tensor.matmul(po[:], w2_t[k][:, d * 128:(d + 1) * 128],
                                     rt, start=(k == 0), stop=(k == kt2_lim - 1))
                if "obdve" in pset:
                    vec.tensor_copy(ob[:, d, :], po[:])
                else:
                    sca.copy(ob[:, d, :], po[:])
            nc.sync.dma_start(out=outp[:, :, tok_p], in_=ob[:])

        prev = None
        pend_in = {}
        for n in [nn_ for _ in range(reps) for nn_ in range(c.NCHUNK)]:
            t0 = (n % c.CPB) * c.CN
            first_in_batch = t0 == 0
            tok = slice(n * c.CN, (n + 1) * c.CN)

            if prev is not None:
                emit_tail_pre(prev)

            posb = stream.tile([128, c.CT, c.CN], F16, tag="posb")
            nc.sync.dma_start(out=posb[:], in_=pos[:, :, t0:t0 + c.CN])

            gnames = {"ph": "th_ph", "mg": "th_mg", "qr": "qre", "qi": "qim"}
            if not pin2 or n % 2 == 0:
                ntin = 2 * c.CN if pin2 else c.CN
                itok = slice(n * c.CN, n * c.CN + ntin)
                sb = 1 if pin2 else None
                xcb = stream.tile([128, c.KT1, ntin], F16, tag="xcb", bufs=sb)
                nc.sync.dma_start(out=xcb[:], in_=xt[:, :, itok])
                if fp8mag:
                    x8cb = stream.tile([128, c.KT1, ntin], F8, tag="x8cb",
                                       bufs=sb)
                    nc.sync.dma_start(out=x8cb[:], in_=x8[:, :, itok])
                ib = 4 if pin2 else None
                cur_in, nxt_in = {}, {}
                for h in range(NH):
                    for nm in ("th_ph", "th_mg", "qre", "qim"):
                        cur_in[(h, nm)] = wide.tile([128, W], F16, tag=nm,
                                                    name=nm, bufs=ib)
                        if pin2:
                            nxt_in[(h, nm)] = wide.tile([128, W], F16, tag=nm,
                                                        name=nm + "b", bufs=ib)
                dmaps = [(cur_in, 0)] + ([(nxt_in, 1)] if pin2 else [])
                for h in range(NH):
                    i0 = h * SEGS
                    for j in range(0, SEGS, 2):
                        for g in ("ph", "mg", "qr", "qi"):
                            p = praw.tile([128, 2 * ntin], F32, tag="praw")
                            pv = p.rearrange("p (c t) -> p c t", c=2)
                            nm = gnames[g]
                            wcols = slice(j * c.CN, (j + 2) * c.CN)
                            if g == "mg" and fp8mag:
                                nk = c.KT1 // 2
                                for half in range(2):
                                    mch = i0 + j + half
                                    for kk in range(nk):
                                        nc.tensor.matmul(
                                            pv[:, half, :],
                                            w1m8_sb[:, kk, :,
                                                    mch * 128:(mch + 1) * 128],
                                            x8cb[:, 2 * kk:2 * kk + 2, :],
                                            start=(kk == 0),
                                            stop=(kk == nk - 1),
                                            perf_mode=mybir.MatmulPerfMode.DoubleRow)
                                for dmap, cc in dmaps:
                                    src = pv[:, :, cc * c.CN:(cc + 1) * c.CN]
                                    sca.activation(dmap[(h, nm)][:, wcols],
                                                   src, AF.Tanh,
                                                   scale=0.5 / 1024.0)
                                continue
                            if fp8mag:
                                gi = {"ph": 0, "qr": 1, "qi": 2}[g]
                            else:
                                gi = {"ph": 0, "mg": 1, "qr": 2, "qi": 3}[g]
                            for half in range(2):
                                m = gi * c.CT + i0 + j + half
                                for k in range(kt1_lim):
                                    nc.tensor.matmul(
                                        pv[:, half, :],
                                        w1_t[k][:, m * 128:(m + 1) * 128],
                                        xcb[:, k, :],
                                        start=(k == 0),
                                        stop=(k == kt1_lim - 1))
                            for dmap, cc in dmaps:
                                src = pv[:, :, cc * c.CN:(cc + 1) * c.CN]
                                dst = dmap[(h, nm)][:, wcols]
                                if g == "ph":
                                    sca.activation(dst, src, AF.Tanh)
                                elif g == "mg":
                                    sca.activation(dst, src, AF.Tanh, scale=0.5)
                                elif "qdve" in pset:
                                    vec.tensor_copy(dst, src)
                                else:
                                    sca.copy(dst, src)
                if pin2:
                    pend_in.clear()
                    pend_in.update(nxt_in)
            else:
                cur_in = dict(pend_in)

            ret_w = {}
            for h in range(NH):
                i0 = h * SEGS
                th_ph = cur_in[(h, "th_ph")]
                th_mg = cur_in[(h, "th_mg")]
                qre = cur_in[(h, "qre")]
                qim = cur_in[(h, "qim")]

                # th = (pi/2)*tanh(ph) + pos/2 ; th_ph already scaled by pi/2
                pos_h = posb[:, i0:i0 + SEGS, :]
                theta = wide.tile([128, W], F16, tag="theta", name="theta")
                vec.scalar_tensor_tensor(theta[:], th_ph[:], PI / 2, pos_h,
                                         ALU.mult, ALU.add)
                sh = wide.tile([128, W], F16, tag="sh", name="sh")
                sca.activation(sh[:], theta[:], AF.Sin)
                ab = wide.tile([128, W], F16, tag="ab", name="ab")
                if "abdve" in pset:
                    vec.tensor_scalar(ab[:], theta[:], 0.0, None, ALU.abs_max)
                else:
                    sca.activation(ab[:], theta[:], AF.Abs)
                ch = wide.tile([128, W], F16, tag="theta", name="ch")
                sca.activation(ch[:], ab[:], AF.Sin, bias=half_pi[:], scale=-1.0)
                sqh = wide.tile([128, W], F16, tag="ab", name="sqh")
                if "sqhdve" in pset:
                    vec.tensor_mul(sqh[:], sh[:], sh[:])
                else:
                    sca.activation(sqh[:], sh[:], AF.Square)
                mg2 = wide.tile([128, W], F16, tag="mg2", name="mg2")
                if "mg2dve" in pset:
                    vec.tensor_scalar(mg2[:], th_mg[:], 1.0, None, ALU.add)
                else:
                    sca.activation(mg2[:], th_mg[:], AF.Identity, bias=one_f[:])

                kreN = wide.tile([128, W], F16,
                                 tag="kreN" if pin2 else "th_ph", name="kreN")
                vec.scalar_tensor_tensor(kreN[:], sqh[:], 0.5, mg2[:],
                                         ALU.subtract, ALU.mult)
                tt = wide.tile([128, W], F16,
                               tag="tt" if pin2 else "th_mg", name="tt")
                vec.tensor_mul(tt[:], sh[:], ch[:])
                kim = wide.tile([128, W], F16, tag="sh", name="kim")
                vec.tensor_mul(kim[:], tt[:], mg2[:])

                mre = wide.tile([128, W], F16, tag="mre", name="mre")
                mim = wide.tile([128, W], F16, tag="mim", name="mim")
                for s in range(SEGS):
                    seg = slice(s * c.CN, (s + 1) * c.CN)
                    init_re = 0.0 if first_in_batch else car[(h, "re")][:, s:s + 1]
                    vec.tensor_tensor_scan(mre[:, seg], kreN[:, seg],
                                           kreN[:, seg], init_re,
                                           ALU.add, ALU.bypass)
                    init_im = 0.0 if first_in_batch else car[(h, "im")][:, s:s + 1]
                    vec.tensor_tensor_scan(mim[:, seg], kim[:, seg],
                                           kim[:, seg], init_im,
                                           ALU.add, ALU.bypass)
                if (n % c.CPB) != c.CPB - 1:
                    cre = mre.rearrange("p (s t) -> p s t", s=SEGS)[:, :, c.CN - 1]
                    vec.tensor_copy(car[(h, "re")][:], cre)
                    cim = mim.rearrange("p (s t) -> p s t", s=SEGS)[:, :, c.CN - 1]
                    vec.tensor_copy(car[(h, "im")][:], cim)

                # retrieval (mreN = -Sre):
                #   rre = mim*qim - mreN*qre ; rim = mim*qre + mreN*qim
                r1 = wide.tile([128, W], F16, tag="theta", name="r1")
                vec.tensor_mul(r1[:], mre[:], qre[:])
                r2 = wide.tile([128, W], F16, tag="ab", name="r2")
                vec.tensor_mul(r2[:], mim[:], qim[:])
                rre = retp.tile([128, W], F16, tag=f"ret_re_{h}",
                                name=f"ret_re_{h}")
                vec.tensor_sub(rre[:], r2[:], r1[:])
                r3 = wide.tile([128, W], F16, tag="theta", name="r3")
                vec.tensor_mul(r3[:], mim[:], qre[:])
                r4 = wide.tile([128, W], F16, tag="ab", name="r4")
                vec.tensor_mul(r4[:], mre[:], qim[:])
                rim = retp.tile([128, W], F16, tag=f"ret_im_{h}",
                                name=f"ret_im_{h}")
                vec.tensor_add(rim[:], r3[:], r4[:])
                ret_w[(h, "re")] = rre
                ret_w[(h, "im")] = rim

            if prev is not None:
                emit_tail_mm(prev)
            prev = {"ret": ret_w, "tok": tok}

        emit_tail_pre(prev)
        emit_tail_mm(prev)

    return nc


def build_program_v3(cfg: Cfg, reps: int = 1, hw_reps: int = 1,
                     probe: str | None = None):
    """v3: all-f16 matmuls (fp8 dropped -- measured no win on HW), ACT chain
    cut to 3 LUT ops/half, Pool (GPSIMD) engine recruited for the squares
    and sin-products, per-chunk emission ordered so every engine queue is
    dependency-ready (ph/mg GEMMs before q GEMMs, chain interleaved).

    Engine budget per 256-token chunk (target: PE-bound):
      PE   proj_in 256 MM + stats 2 + proj_out 128 MM        ~28.5us
      DVE  theta/mg2/kreN/kim, scans, retrieval, folds, ob   ~23us
      ACT  16 psum drains + sh/ab/ch + sc                    ~16.5us
      Pool sq/tt2 + stats squares                            ~17us
    """
    c = cfg
    assert c.CT % 4 == 0
    SEGS = 4
    NH = c.CT // SEGS
    W = SEGS * c.CN
    nc = bass.Bass()

    class _Dup:
        def __init__(self, eng, on):
            self._eng, self._on = eng, on

        def __getattr__(self, n):
            f = getattr(self._eng, n)
            if not self._on:
                return f

            def g(*a, **k):
                r = f(*a, **k)
                f(*a, **k)
                return r
            return g

    pset = set(probe.split(",")) if probe else set()
    vec = _Dup(nc.vector, "dve2" in pset)
    sca = _Dup(nc.scalar, "act2" in pset)
    pool = _Dup(nc.gpsimd, "pool2" in pset)
    kt1_lim = c.KT1 // 2 if "pein_half" in pset else c.KT1
    kt2_lim = c.KT2 // 2 if "peout_half" in pset else c.KT2
    sq_eng = sca if "sqact" in pset else pool       # stats squares
    ch_eng = vec if "poolchain_off" in pset else pool  # sq/tt2 in chain

    w1 = nc.dram_tensor("w1", [128, c.KT1, 4 * c.NCH], F16,
                        kind="ExternalInput")
    w2 = nc.dram_tensor("w2", [128, c.KT2, c.DIM], F16, kind="ExternalInput")
    xt = nc.dram_tensor("xt", [128, c.KT1, c.NTOK], F16, kind="ExternalInput")
    pos = nc.dram_tensor("pos", [128, c.CT, c.T], F16, kind="ExternalInput")
    DD = c.DT // 2
    outp = nc.dram_tensor("outp", [128, DD, 2, c.NTOK], BF16,
                          kind="ExternalOutput")
    stats = nc.dram_tensor("stats", [2, c.NTOK], F32, kind="ExternalOutput")

    from contextlib import ExitStack
    with tile.TileContext(nc) as tc, ExitStack() as es:
        praw_bufs = 4 if "praw4" in pset else 6
        consts = es.enter_context(tc.tile_pool(name="consts", bufs=1))
        stream = es.enter_context(tc.tile_pool(name="stream", bufs=2))
        wide = es.enter_context(tc.tile_pool(name="wide", bufs=2))
        retp = es.enter_context(tc.tile_pool(name="retp", bufs=2))
        obp = es.enter_context(tc.tile_pool(name="obp", bufs=2))
        stb = es.enter_context(tc.tile_pool(name="stb", bufs=2))
        stc = es.enter_context(tc.tile_pool(name="stc", bufs=2))
        praw = es.enter_context(tc.tile_pool(name="praw", bufs=praw_bufs,
                                             space="PSUM"))
        pstat = es.enter_context(tc.tile_pool(name="pstat", bufs=1,
                                              space="PSUM"))
        pout = es.enter_context(tc.tile_pool(name="pout", bufs=1,
                                             space="PSUM"))

        w1_sb = consts.tile([128, c.KT1, 4 * c.NCH], F16, tag="w1_sb")
        nc.sync.dma_start(out=w1_sb[:], in_=w1[:])
        w2_sb = consts.tile([128, c.KT2, c.DIM], F16, tag="w2_sb")
        nc.sync.dma_start(out=w2_sb[:], in_=w2[:])
        w1_t = [w1_sb[:, k, :] for k in range(c.KT1)]
        w2_t = [w2_sb[:, k, :] for k in range(c.KT2)]

        ones_bf = consts.tile([128, 1], BF16, tag="ones")
        vec.memset(ones_bf[:], 1.0)
        half_pi = consts.tile([128, 1], F32, tag="half_pi")
        vec.memset(half_pi[:], PI / 2)
        car = {}
        for h in range(NH):
            for pl in ("re", "im"):
                car[(h, pl)] = consts.tile([128, SEGS], F16,
                                           tag=f"car_{h}_{pl}",
                                           name=f"car_{h}_{pl}")

        if hw_reps > 1:
            es.enter_context(tc.For_i(0, hw_reps))

        h2 = W // 2

        def emit_tail_pre(st):
            """Stats for chunk n-1: Pool squares + DVE fold tree."""
            rw = st["ret"]
            sqs = {}
            for h in range(NH):
                for pl in ("re", "im"):
                    s = wide.tile([128, W], BF16, tag=f"sq_{h}_{pl}", bufs=1,
                                  name=f"s_{h}_{pl}")
                    sq_eng.tensor_mul(s[:], rw[(h, pl)][:], rw[(h, pl)][:])
                    sqs[(h, pl)] = s
            a = wide.tile([128, W], F16, tag="stA", bufs=1, name="a")
            vec.tensor_add(a[:], rw[(0, "re")][:], rw[(0, "im")][:])
            b = wide.tile([128, W], F16, tag="stB", bufs=1, name="b")
            vec.tensor_add(b[:], rw[(1, "re")][:], rw[(1, "im")][:])
            rs = wide.tile([128, W], F16, tag="stC", bufs=1, name="rs")
            vec.tensor_add(rs[:], a[:], b[:])
            f = wide.tile([128, h2], F16, tag="stD", bufs=1, name="f")
            vec.tensor_add(f[:], rs[:, 0:h2], rs[:, h2:W])
            stt = stb.tile([128, 2 * c.CN], BF16, tag="stt", name="stt")
            vec.tensor_add(stt[:, 0:c.CN], f[:, 0:c.CN], f[:, c.CN:h2])
            a2 = wide.tile([128, W], BF16, tag="stA", bufs=1, name="a2")
            vec.tensor_add(a2[:], sqs[(0, "re")][:], sqs[(0, "im")][:])
            b2 = wide.tile([128, W], BF16, tag="stB", bufs=1, name="b2")
            vec.tensor_add(b2[:], sqs[(1, "re")][:], sqs[(1, "im")][:])
            ss = wide.tile([128, W], BF16, tag="stC", bufs=1, name="ss")
            vec.tensor_add(ss[:], a2[:], b2[:])
            f2 = wide.tile([128, h2], BF16, tag="stD", bufs=1, name="f2")
            vec.tensor_add(f2[:], ss[:, 0:h2], ss[:, h2:W])
            vec.tensor_add(stt[:, c.CN:2 * c.CN], f2[:, 0:c.CN],
                           f2[:, c.CN:h2])
            st["stt"] = stt

        def emit_tail_mm(st):
            """Stats matmuls + proj_out for chunk n-1."""
            stt, tok_p, rw = st["stt"], st["tok"], st["ret"]
            ps = pstat.tile([1, 2 * c.CN], F32, tag="ps")
            nc.tensor.matmul(ps[:, 0:c.CN], ones_bf[:], stt[:, 0:c.CN],
                             start=True, stop=True)
            nc.tensor.matmul(ps[:, c.CN:2 * c.CN], ones_bf[:],
                             stt[:, c.CN:2 * c.CN], start=True, stop=True)
            sc = stc.tile([1, 2 * c.CN], F32, tag="sc", name="sc")
            sca.copy(sc[:], ps[:])
            nc.sync.dma_start(out=stats[0:1, tok_p], in_=sc[:, 0:c.CN])
            nc.sync.dma_start(out=stats[1:2, tok_p], in_=sc[:, c.CN:2 * c.CN])
            for dd in range(DD):
                po = pout.tile([128, 2, c.CN], F32, tag="pout")
                for di in range(2):
                    d = dd * 2 + di
                    for k in range(kt2_lim):
                        if k < c.CT:
                            h, s, pl = k // SEGS, k % SEGS, "re"
                        else:
                            h, s, pl = ((k - c.CT) // SEGS,
                                        (k - c.CT) % SEGS, "im")
                        rt = rw[(h, pl)][:, s * c.CN:(s + 1) * c.CN]
                        nc.tensor.matmul(po[:, di, :],
                                         w2_t[k][:, d * 128:(d + 1) * 128],
                                         rt, start=(k == 0),
                                         stop=(k == kt2_lim - 1))
                ob = obp.tile([128, 2, c.CN], BF16, tag="ob", name="ob")
                if "obact" in pset:
                    sca.copy(ob[:], po[:])
                else:
                    vec.tensor_copy(ob[:], po[:])
                nc.sync.dma_start(out=outp[:, dd, :, tok_p], in_=ob[:])

        def drain_group(p, g, dst, wcols):
            if g == "ph":
                sca.activation(dst[:, wcols], p[:], AF.Tanh)
            elif g == "mg":
                sca.activation(dst[:, wcols], p[:], AF.Tanh, scale=0.5)
            else:
                sca.copy(dst[:, wcols], p[:])

        prev = None
        for n in [nn_ for _ in range(reps) for nn_ in range(c.NCHUNK)]:
            t0 = (n % c.CPB) * c.CN
            first_in_batch = t0 == 0
            tok = slice(n * c.CN, (n + 1) * c.CN)

            if prev is not None:
                emit_tail_pre(prev)

            posb = stream.tile([128, c.CT, c.CN], F16, tag="posb")
            nc.sync.dma_start(out=posb[:], in_=pos[:, :, t0:t0 + c.CN])
            xcb = stream.tile([128, c.KT1, c.CN], F16, tag="xcb")
            nc.sync.dma_start(out=xcb[:], in_=xt[:, :, tok])

            gidx = {"ph": 0, "mg": 1, "qr": 2, "qi": 3}
            ret_w = {}
            for h in range(NH):
                i0 = h * SEGS
                th_ph = wide.tile([128, W], F16, tag="th_ph", name="th_ph")
                th_mg = wide.tile([128, W], F16, tag="th_mg", name="th_mg")
                qre = wide.tile([128, W], F16, tag="qre", name="qre")
                qim = wide.tile([128, W], F16, tag="qim", name="qim")
                dest = {"ph": th_ph, "mg": th_mg, "qr": qre, "qi": qim}

                def gemm_pass(groups):
                    for j in (0, 2):
                        for g in groups:
                            p = praw.tile([128, 2 * c.CN], F32, tag="praw")
                            for half in range(2):
                                m = gidx[g] * c.CT + i0 + j + half
                                cols = slice(half * c.CN, (half + 1) * c.CN)
                                for k in range(kt1_lim):
                                    nc.tensor.matmul(
                                        p[:, cols],
                                        w1_t[k][:, m * 128:(m + 1) * 128],
                                        xcb[:, k, :],
                                        start=(k == 0),
                                        stop=(k == kt1_lim - 1))
                            drain_group(p, g, dest[g],
                                        slice(j * c.CN, (j + 2) * c.CN))

                # phase/magnitude GEMMs first: the chain head depends on them
                gemm_pass(("ph", "mg"))
                theta = wide.tile([128, W], F16, tag="theta", name="theta")
                vec.scalar_tensor_tensor(theta[:], th_ph[:], PI / 2,
                                         posb[:, i0:i0 + SEGS, :],
                                         ALU.mult, ALU.add)
                mg2 = wide.tile([128, W], F16, tag="mg2", name="mg2")
                vec.tensor_scalar(mg2[:], th_mg[:], 1.0, None, ALU.add)
                sh = wide.tile([128, W], F16, tag="sh", name="sh")
                sca.activation(sh[:], theta[:], AF.Sin)
                ab = wide.tile([128, W], F16, tag="ab", bufs=1, name="ab")
                sca.activation(ab[:], theta[:], AF.Abs)
                ch = wide.tile([128, W], F16, tag="ch", bufs=1, name="ch")
                sca.activation(ch[:], ab[:], AF.Sin, bias=half_pi[:],
                               scale=-1.0)
                sq = wide.tile([128, W], F16, tag="sq", bufs=1, name="sq")
                ch_eng.tensor_mul(sq[:], sh[:], sh[:])
                tt2 = wide.tile([128, W], F16, tag="tt2", bufs=1, name="tt2")
                ch_eng.tensor_mul(tt2[:], sh[:], ch[:])

                # query GEMMs while the chain runs on ACT/DVE/Pool
                gemm_pass(("qr", "qi"))

                kreN = wide.tile([128, W], F16, tag="kreN", bufs=1,
                                 name="kreN")
                vec.scalar_tensor_tensor(kreN[:], sq[:], 0.5, mg2[:],
                                         ALU.subtract, ALU.mult)
                kim = wide.tile([128, W], F16, tag="kim", bufs=1, name="kim")
                vec.tensor_mul(kim[:], tt2[:], mg2[:])

                mre = wide.tile([128, W], F16, tag="mre", name="mre")
                mim = wide.tile([128, W], F16, tag="mim", name="mim")
                for s in range(SEGS):
                    seg = slice(s * c.CN, (s + 1) * c.CN)
                    init_re = (0.0 if first_in_batch
                               else car[(h, "re")][:, s:s + 1])
                    vec.tensor_tensor_scan(mre[:, seg], kreN[:, seg],
                                           kreN[:, seg], init_re,
                                           ALU.add, ALU.bypass)
                    init_im = (0.0 if first_in_batch
                               else car[(h, "im")][:, s:s + 1])
                    vec.tensor_tensor_scan(mim[:, seg], kim[:, seg],
                                           kim[:, seg], init_im,
                                           ALU.add, ALU.bypass)
                if (n % c.CPB) != c.CPB - 1:
                    cre = mre.rearrange("p (s t) -> p s t", s=SEGS)[:, :,
                                                                    c.CN - 1]
                    vec.tensor_copy(car[(h, "re")][:], cre)
                    cim = mim.rearrange("p (s t) -> p s t", s=SEGS)[:, :,
                                                                    c.CN - 1]
                    vec.tensor_copy(car[(h, "im")][:], cim)

                # retrieval (mreN = -Sre):
                #   rre = mim*qim - mreN*qre ; rim = mim*qre + mreN*qim
                r1 = wide.tile([128, W], F16, tag="r1", bufs=1, name="r1")
                vec.tensor_mul(r1[:], mre[:], qre[:])
                r2 = wide.tile([128, W], F16, tag="r2", bufs=1, name="r2")
                vec.tensor_mul(r2[:], mim[:], qim[:])
                rre = retp.tile([128, W], F16, tag=f"ret_re_{h}",
                                name=f"ret_re_{h}")
                vec.tensor_sub(rre[:], r2[:], r1[:])
                r3 = wide.tile([128, W], F16, tag="r1", bufs=1, name="r3")
                vec.tensor_mul(r3[:], mim[:], qre[:])
                r4 = wide.tile([128, W], F16, tag="r2", bufs=1, name="r4")
                vec.tensor_mul(r4[:], mre[:], qim[:])
                rim = retp.tile([128, W], F16, tag=f"ret_im_{h}",
                                name=f"ret_im_{h}")
                vec.tensor_add(rim[:], r3[:], r4[:])
                ret_w[(h, "re")] = rre
                ret_w[(h, "im")] = rim

            if prev is not None:
                emit_tail_mm(prev)
            prev = {"ret": ret_w, "tok": tok}

        emit_tail_pre(prev)
        emit_tail_mm(prev)

    return nc


# --------------------------------------------------------------------------
# Host-side sharding / unsharding
# --------------------------------------------------------------------------
def shard_inputs(cfg, x, W_in, W_out, ln_gamma, ln_beta, pos_phases):
    c = cfg
    HD = N_CORES * c.NCH
    xT = np.ascontiguousarray(x.reshape(c.NTOK, c.DIM).T)          # [DIM, NTOK]
    # [p, k, tok] partition-major so one DMA covers all k-tiles of a chunk
    xt_h = np.ascontiguousarray(
        xT.reshape(c.KT1, 128, c.NTOK).transpose(1, 0, 2)
    ).astype(ml_dtypes.bfloat16)

    pos64 = pos_phases.astype(np.float64)
    cos_p = (0.5 * np.cos(pos64)).astype(np.float16)               # [T, HD]
    sin_p = (0.5 * np.sin(pos64)).astype(np.float16)

    Wg = (W_out * ln_gamma[None, :]).astype(np.float32)            # [DIM, 2HD]

    in_maps = []
    for cid in range(N_CORES):
        h0 = cid * c.NCH
        hs = slice(h0, h0 + c.NCH)
        w_ph = W_in[0 * HD + h0:0 * HD + h0 + c.NCH]               # [NCH, DIM]
        w_mg = W_in[1 * HD + h0:1 * HD + h0 + c.NCH]
        w_qr = W_in[2 * HD + h0:2 * HD + h0 + c.NCH]
        w_qi = W_in[3 * HD + h0:3 * HD + h0 + c.NCH]
        w_all = np.concatenate([w_ph, w_mg, w_qr, w_qi], axis=0)   # [4NCH, DIM]
        w1_h = np.ascontiguousarray(
            w_all.T.reshape(c.KT1, 128, 4 * c.NCH).transpose(1, 0, 2)
        ).astype(ml_dtypes.bfloat16)

        wg_re = Wg[:, 2 * h0:2 * (h0 + c.NCH):2]                   # [DIM, NCH]
        wg_im = Wg[:, 2 * h0 + 1:2 * (h0 + c.NCH):2]
        w2T = np.concatenate([wg_re.T, wg_im.T], axis=0)           # [2NCH, DIM]
        w2_h = np.ascontiguousarray(
            w2T.reshape(c.KT2, 128, c.DIM).transpose(1, 0, 2)
        ).astype(ml_dtypes.bfloat16)

        cp_h = np.ascontiguousarray(
            cos_p[:, hs].T.reshape(c.CT, 128, c.T).transpose(1, 0, 2))
        sp_h = np.ascontiguousarray(
            sin_p[:, hs].T.reshape(c.CT, 128, c.T).transpose(1, 0, 2))

        in_maps.append({
            "w1": w1_h, "w2": w2_h, "xt": xt_h,
            "cp": cp_h, "sp": sp_h,
        })
    return in_maps


def combine_outputs(cfg, results, W_out, ln_gamma, ln_beta, x_dtype):
    c = cfg
    NF = 2 * N_CORES * c.NCH
    P = np.zeros((c.DIM, c.NTOK), np.float64)
    S1 = np.zeros(c.NTOK, np.float64)
    S2 = np.zeros(c.NTOK, np.float64)
    for r in results:
        # outp is [128, DT, NTOK] partition-major of out^T -> [DIM, NTOK]
        op = r["outp"].transpose(1, 0, 2).reshape(c.DIM, c.NTOK)
        P += op.astype(np.float64)
        S1 += r["stats"][0].astype(np.float64)
        S2 += r["stats"][1].astype(np.float64)
    mu = S1 / NF
    var = S2 / NF - mu * mu
    istd = 1.0 / np.sqrt(var + LN_EPS)
    wg_sum = (W_out.astype(np.float64) @ ln_gamma.astype(np.float64))  # [DIM]
    b_out = (W_out.astype(np.float64) @ ln_beta.astype(np.float64))    # [DIM]
    out = istd[:, None] * (P.T - mu[:, None] * wg_sum[None, :]) + b_out[None, :]
    return out.reshape(c.B, c.T, c.DIM).astype(x_dtype)


def shard_inputs_v2(cfg, x, W_in, W_out, ln_gamma, ln_beta, pos_phases):
    c = cfg
    HD = N_CORES * c.NCH
    xT = np.ascontiguousarray(x.reshape(c.NTOK, c.DIM).T)          # [DIM, NTOK]
    xt_h = np.ascontiguousarray(
        xT.reshape(c.KT1, 128, c.NTOK).transpose(1, 0, 2)
    ).astype(np.float16)
    x8_h = np.ascontiguousarray(
        (xT * 16.0).reshape(c.KT1, 128, c.NTOK).transpose(1, 0, 2)
    ).astype(ml_dtypes.float8_e4m3)

    # pos/2, wrapped to [-pi/2, pi/2): theta_half = pi/2*tanh(ph) + pos/2
    pos64 = pos_phases.astype(np.float64)
    pos_half = (0.5 * (np.mod(pos64 + np.pi, 2 * np.pi) - np.pi)
                ).astype(np.float16)                               # [T, HD]

    Wg = (W_out * ln_gamma[None, :]).astype(np.float32)            # [DIM, 2HD]

    in_maps = []
    for cid in range(N_CORES):
        h0 = cid * c.NCH
        hs = slice(h0, h0 + c.NCH)
        w_ph = W_in[0 * HD + h0:0 * HD + h0 + c.NCH]
        w_mg = W_in[1 * HD + h0:1 * HD + h0 + c.NCH]
        w_qr = W_in[2 * HD + h0:2 * HD + h0 + c.NCH]
        w_qi = W_in[3 * HD + h0:3 * HD + h0 + c.NCH]
        w_all = np.concatenate([w_ph, w_mg, w_qr, w_qi], axis=0)   # [4NCH, DIM]
        w1_h = np.ascontiguousarray(
            w_all.T.reshape(c.KT1, 128, 4 * c.NCH).transpose(1, 0, 2)
        ).astype(np.float16)

        wg_re = Wg[:, 2 * h0:2 * (h0 + c.NCH):2]                   # [DIM, NCH]
        wg_im = Wg[:, 2 * h0 + 1:2 * (h0 + c.NCH):2]
        w2T = np.concatenate([wg_re.T, wg_im.T], axis=0)           # [2NCH, DIM]
        w2_h = np.ascontiguousarray(
            w2T.reshape(c.KT2, 128, c.DIM).transpose(1, 0, 2)
        ).astype(np.float16)

        pos_h = np.ascontiguousarray(
            pos_half[:, hs].T.reshape(c.CT, 128, c.T).transpose(1, 0, 2))

        # fp8(e4m3) copies for the magnitude channel (scales folded into
        # the on-chip tanh input scale: 0.5/(16*64)).
        w13_full = np.concatenate([w_ph, w_qr, w_qi], axis=0)
        w13_h = np.ascontiguousarray(
            w13_full.T.reshape(c.KT1, 128, 3 * c.NCH).transpose(1, 0, 2)
        ).astype(np.float16)
        wm8 = (w_mg.astype(np.float32) * 64.0).astype(
            ml_dtypes.float8_e4m3).astype(ml_dtypes.float8_e4m3)
        # layout [128, KT1//2, 2, NCH]: plane i of pair kk is k-tile 2kk+i
        wm8_h = np.ascontiguousarray(
            wm8.T.reshape(c.KT1 // 2, 2, 128, c.NCH).transpose(2, 0, 1, 3))
        in_maps.append({"w1": w1_h, "w2": w2_h, "xt": xt_h, "pos": pos_h,
                        "w13": w13_h, "w1m8": wm8_h, "x8": x8_h})
    return in_maps


def combine_outputs_v2(cfg, results, W_out, ln_gamma, ln_beta, x_dtype):
    c = cfg
    NF = 2 * N_CORES * c.NCH
    P = np.zeros((c.DIM, c.NTOK), np.float64)
    S1 = np.zeros(c.NTOK, np.float64)
    S2 = np.zeros(c.NTOK, np.float64)
    for r in results:
        op = r["outp"].transpose(1, 0, 2).reshape(c.DIM, c.NTOK)
        P += op.astype(np.float64)
        S1 += r["stats"][0].astype(np.float64)
        S2 += r["stats"][1].astype(np.float64)
    mu = S1 / NF
    var = S2 / NF - mu * mu
    istd = 1.0 / np.sqrt(var + LN_EPS)
    wg_sum = (W_out.astype(np.float64) @ ln_gamma.astype(np.float64))
    b_out = (W_out.astype(np.float64) @ ln_beta.astype(np.float64))
    out = istd[:, None] * (P.T - mu[:, None] * wg_sum[None, :]) + b_out[None, :]
    return out.reshape(c.B, c.T, c.DIM).astype(x_dtype)


def shard_inputs_v3(cfg, x, W_in, W_out, ln_gamma, ln_beta, pos_phases):
    c = cfg
    HD = N_CORES * c.NCH
    xT = np.ascontiguousarray(x.reshape(c.NTOK, c.DIM).T)          # [DIM, NTOK]
    xt_h = np.ascontiguousarray(
        xT.reshape(c.KT1, 128, c.NTOK).transpose(1, 0, 2)
    ).astype(np.float16)

    # pos/2, wrapped to [-pi/2, pi/2): theta_half = pi/2*tanh(ph) + pos/2
    pos64 = pos_phases.astype(np.float64)
    pos_half = (0.5 * (np.mod(pos64 + np.pi, 2 * np.pi) - np.pi)
                ).astype(np.float16)                               # [T, HD]

    Wg = (W_out * ln_gamma[None, :]).astype(np.float32)            # [DIM, 2HD]

    in_maps = []
    for cid in range(N_CORES):
        h0 = cid * c.NCH
        hs = slice(h0, h0 + c.NCH)
        w_ph = W_in[0 * HD + h0:0 * HD + h0 + c.NCH]
        w_mg = W_in[1 * HD + h0:1 * HD + h0 + c.NCH]
        w_qr = W_in[2 * HD + h0:2 * HD + h0 + c.NCH]
        w_qi = W_in[3 * HD + h0:3 * HD + h0 + c.NCH]
        w_all = np.concatenate([w_ph, w_mg, w_qr, w_qi], axis=0)   # [4NCH, DIM]
        w1_h = np.ascontiguousarray(
            w_all.T.reshape(c.KT1, 128, 4 * c.NCH).transpose(1, 0, 2)
        ).astype(np.float16)

        wg_re = Wg[:, 2 * h0:2 * (h0 + c.NCH):2]                   # [DIM, NCH]
        wg_im = Wg[:, 2 * h0 + 1:2 * (h0 + c.NCH):2]
        w2T = np.concatenate([wg_re.T, wg_im.T], axis=0)           # [2NCH, DIM]
        w2_h = np.ascontiguousarray(
            w2T.reshape(c.KT2, 128, c.DIM).transpose(1, 0, 2)
        ).astype(np.float16)

        pos_h = np.ascontiguousarray(
            pos_half[:, hs].T.reshape(c.CT, 128, c.T).transpose(1, 0, 2))
        in_maps.append({"w1": w1_h, "w2": w2_h, "xt": xt_h, "pos": pos_h})
    return in_maps


def combine_outputs_v3(cfg, results, W_out, ln_gamma, ln_beta, x_dtype):
    c = cfg
    NF = 2 * N_CORES * c.NCH
    P = np.zeros((c.DIM, c.NTOK), np.float64)
    S1 = np.zeros(c.NTOK, np.float64)
    S2 = np.zeros(c.NTOK, np.float64)
    for r in results:
        # outp [128, DD, 2, NTOK]: out[(dd*2+di)*128 + p, t]
        op = r["outp"].transpose(1, 2, 0, 3).reshape(c.DIM, c.NTOK)
        P += op.astype(np.float64)
        S1 += r["stats"][0].astype(np.float64)
        S2 += r["stats"][1].astype(np.float64)
    mu = S1 / NF
    var = S2 / NF - mu * mu
    istd = 1.0 / np.sqrt(var + LN_EPS)
    wg_sum = (W_out.astype(np.float64) @ ln_gamma.astype(np.float64))
    b_out = (W_out.astype(np.float64) @ ln_beta.astype(np.float64))
    out = istd[:, None] * (P.T - mu[:, None] * wg_sum[None, :]) + b_out[None, :]
    return out.reshape(c.B, c.T, c.DIM).astype(x_dtype)


import os

# v3: all-f16 datapath, Pool engine offload, no fp8.
DEFAULT_PROBE = ""


def _active_build(cfg, reps=1, hw_reps=1, probe=None):
    env = os.environ.get("KERNEL_PROBE")
    base = DEFAULT_PROBE if env is None else env
    merged = ",".join(x for x in [base, probe or ""] if x) or None
    return build_program_v3(cfg, reps=reps, hw_reps=hw_reps, probe=merged)


# Active implementation selector (test.py/bench use these too)
BUILD = _active_build
SHARD = shard_inputs_v3
COMBINE = combine_outputs_v3
CN_ACTIVE = 256

_cached = {}


def kernel(x, W_in, W_out, ln_gamma, ln_beta, pos_phases):
    cfg = Cfg(B=x.shape[0], T=x.shape[1], DIM=x.shape[2],
              NCH=pos_phases.shape[1] // N_CORES, CN=CN_ACTIVE)
    key = (cfg.B, cfg.T, cfg.DIM, cfg.NCH)
    if key not in _cached:
        nc = BUILD(cfg)
        split_multiwait(nc)  # walrus workaround; CoreSim path must skip this
        _cached[key] = nc
    nc = _cached[key]
    in_maps = SHARD(cfg, np.asarray(x), np.asarray(W_in),
                    np.asarray(W_out), np.asarray(ln_gamma),
                    np.asarray(ln_beta), np.asarray(pos_phases))
    # the native run path rejects in_map keys the program doesn't declare
    declared = {a.memorylocations[0].name
                for a in nc.m.functions[0].allocations
                if isinstance(a, mybir.MemoryLocationSet)
                and a.kind == "ExternalInput"}
    in_maps = [{k: v for k, v in m.items() if k in declared} for m in in_maps]
    res = run_bass_kernel_spmd(nc, in_maps, list(range(N_CORES)))
    return COMBINE(cfg, res.results, np.asarray(W_out),
                   np.asarray(ln_gamma), np.asarray(ln_beta),
                   np.asarray(x).dtype)



# revision 7
# speedup vs baseline: 1.5074x; 1.0984x over previous
"""Trainium2 Bass kernel for nn_LongAttention (holographic long-attention block).

Computation (see reference):
  raw = x @ W_in.T -> split [c_phase | c_mag | q_re | q_im] per hd channel
  key = sigmoid(c_mag) * exp(i*(pi*tanh(c_phase) + pos_phase))
  state = cumsum_t(key);  ret = state * conj(q)
  ret_real = interleave(Re, Im) -> LayerNorm(2*hd) -> @ W_out.T

Distribution: hd (8192) split across 8 NeuronCores (1024 ch each); every core
handles both batches and all tokens; cores are fully independent. gamma is
folded into W_out on the host and the LayerNorm is algebraically deferred:
each core returns P = ret @ (W_out*gamma).T partials plus per-token
S1 = sum_f ret, S2 = sum_f ret^2; the host combines
out = istd * (sum_c P_c - mu * (W_out @ gamma)) + W_out @ beta.

Active implementation (build_program_v2, CN=256-token chunks):
 - f16 datapath end to end (matmul inputs, elementwise, scan output) --
   same speed as bf16 everywhere but ~8x finer mantissa, plus 2x DVE
   perf-modes on the 16-bit elementwise ops.
 - The magnitude-channel GEMM runs in fp8(e4m3) with perf_mode=DoubleRow
   (2 k-planes per instruction); the quantization scales (x*16, W*64) are
   folded into the on-chip tanh input scale. Sigmoid's 1/4 slope damps the
   fp8 noise; measured end-to-end rel err 0.011 < 2e-2.
 - sin/cos via the half-angle identity: th = (pi/2)*tanh(ph) + pos/2 with
   pos pre-wrapped to [-pi, pi) on the host, so |th| <= pi stays inside
   the ACT Sin LUT range; cos(2th) = 1-2*sin^2(th) gives the real part
   without a second LUT pass over an out-of-range argument.
 - The cumsum runs channel-major on the DVE as a prefix scan along the free
   (time) axis (fp32 internal state), carried across token chunks.
 - Per-token LN stats are folded on DVE/ACT (tree adds + squares) into one
   [128, 2*CN] tile and reduced across partitions by a single pair of
   ones-matmuls -- instead of 32 PE matmuls per chunk.
 - stats + proj_out for chunk n-1 are emitted during chunk n (software
   pipelining) so the in-order PE queue never waits on the chunk's serial
   ACT<->DVE elementwise chain; all hot pools are double-buffered.
"""

import sys
import numpy as np
import ml_dtypes

for _p in ("/opt/trn_rl_repo", "/root/.axon_site/_ro/trn_rl_repo"):
    if _p not in sys.path:
        sys.path.append(_p)

import bass_rust
import concourse.bass as bass
import concourse.tile as tile
import concourse.mybir as mybir
from concourse.bass_utils import run_bass_kernel_spmd

F32 = mybir.dt.float32
F8 = mybir.dt.float8e4
F16 = mybir.dt.float16
BF16 = mybir.dt.bfloat16
AF = mybir.ActivationFunctionType
ALU = mybir.AluOpType
PI = float(np.pi)

N_CORES = 8
LN_EPS = 1e-5


# --------------------------------------------------------------------------
# Workaround: this container's walrus rejects >1 semaphore wait per
# instruction ("Too many sync wait commands"). Split the extras onto
# same-engine NoOps inserted just before (engine FIFO keeps semantics).
# --------------------------------------------------------------------------
_nop_counter = [0]


def split_multiwait(nc):
    n_split = 0
    for f in nc.m.functions:
        for bb in f.blocks:
            il = bb.instructions
            i = 0
            while i < len(il):
                ins = il[i]
                si = ins.sync_info
                waits = list(si.on_wait) if si is not None and si.on_wait else []
                if len(waits) > 1:
                    for w in waits[:-1]:
                        _nop_counter[0] += 1
                        nop = bass_rust.InstNoOp(
                            name=f"mw_nop_{_nop_counter[0]}",
                            engine=ins.engine,
                            ins=[],
                            outs=[],
                        )
                        nop.sync_info = mybir.SyncInfo(on_wait=[w], on_update=[])
                        il.insert(i, nop)
                        i += 1
                    si.on_wait = [waits[-1]]
                    n_split += 1
                i += 1
    return n_split


# --------------------------------------------------------------------------
# Device program (SPMD: identical on all cores; per-core data differs)
# --------------------------------------------------------------------------
class Cfg:
    def __init__(self, B=2, T=2048, DIM=1024, NCH=1024, CN=256):
        self.B, self.T, self.DIM, self.NCH, self.CN = B, T, DIM, NCH, CN
        self.NTOK = B * T
        self.CT = NCH // 128          # channel tiles per core
        self.KT1 = DIM // 128         # contraction tiles for proj_in
        self.KT2 = 2 * self.CT        # contraction tiles for proj_out (re+im)
        self.DT = DIM // 128          # output dim tiles
        self.NCHUNK = self.NTOK // CN
        self.CPB = T // CN            # chunks per batch


def build_program(cfg: Cfg, reps: int = 1, hw_reps: int = 1,
                  probe: str | None = None):
    c = cfg
    assert c.CT % 4 == 0 or c.CT == 2
    SEGS = 4 if c.CT % 4 == 0 else 2   # channel tiles per wide tile
    NH = c.CT // SEGS                  # wide halves per chunk
    W = SEGS * c.CN                    # wide tile width
    nc = bass.Bass()

    class _Dup:
        def __init__(self, eng, on):
            self._eng, self._on = eng, on

        def __getattr__(self, n):
            f = getattr(self._eng, n)
            if not self._on:
                return f

            def g(*a, **k):
                r = f(*a, **k)
                f(*a, **k)
                return r
            return g

    pset = set(probe.split(",")) if probe else set()
    vec = _Dup(nc.vector, "dve2" in pset)
    sca = _Dup(nc.scalar, "act2" in pset)
    kt1_lim = c.KT1 // 2 if "pein_half" in pset else c.KT1
    kt2_lim = c.KT2 // 2 if "peout_half" in pset else c.KT2
    stats_on = "stats_off" not in pset

    w1 = nc.dram_tensor("w1", [128, c.KT1, 4 * c.NCH], BF16, kind="ExternalInput")
    w2 = nc.dram_tensor("w2", [128, c.KT2, c.DIM], BF16, kind="ExternalInput")
    xt = nc.dram_tensor("xt", [128, c.KT1, c.NTOK], BF16, kind="ExternalInput")
    cp = nc.dram_tensor("cp", [128, c.CT, c.T], F16, kind="ExternalInput")
    sp = nc.dram_tensor("sp", [128, c.CT, c.T], F16, kind="ExternalInput")
    outp = nc.dram_tensor("outp", [128, c.DT, c.NTOK], F32, kind="ExternalOutput")
    stats = nc.dram_tensor("stats", [2, c.NTOK], F32, kind="ExternalOutput")

    from contextlib import ExitStack
    with tile.TileContext(nc) as tc, ExitStack() as es:
        consts = es.enter_context(tc.tile_pool(name="consts", bufs=1))
        stream = es.enter_context(tc.tile_pool(name="stream", bufs=2))
        wide = es.enter_context(tc.tile_pool(name="wide", bufs=1))
        retp = es.enter_context(tc.tile_pool(name="retp", bufs=2))
        obp = es.enter_context(tc.tile_pool(name="obp", bufs=1))
        stc = es.enter_context(tc.tile_pool(name="stc", bufs=2))
        praw = es.enter_context(tc.tile_pool(name="praw", bufs=4, space="PSUM"))
        pstat = es.enter_context(tc.tile_pool(name="pstat", bufs=1, space="PSUM"))
        pstat2 = es.enter_context(tc.tile_pool(name="pstat2", bufs=1, space="PSUM"))
        pout = es.enter_context(tc.tile_pool(name="pout", bufs=2, space="PSUM"))

        w1_sb = consts.tile([128, c.KT1, 4 * c.NCH], BF16, tag="w1_sb")
        nc.sync.dma_start(out=w1_sb[:], in_=w1[:])
        w2_sb = consts.tile([128, c.KT2, c.DIM], BF16, tag="w2_sb")
        nc.sync.dma_start(out=w2_sb[:], in_=w2[:])
        w1_t = [w1_sb[:, k, :] for k in range(c.KT1)]
        w2_t = [w2_sb[:, k, :] for k in range(c.KT2)]

        ones_bf = consts.tile([128, 1], BF16, tag="ones")
        vec.memset(ones_bf[:], 1.0)
        half_pi = consts.tile([128, 1], F32, tag="half_pi")
        vec.memset(half_pi[:], PI / 2)
        car = {}
        for h in range(NH):
            for pl in ("re", "im"):
                car[(h, pl)] = consts.tile([128, SEGS], F32, tag=f"car_{h}_{pl}",
                                           name=f"car_{h}_{pl}")

        if hw_reps > 1:
            es.enter_context(tc.For_i(0, hw_reps))

        for n in [nn_ for _ in range(reps) for nn_ in range(c.NCHUNK)]:
            t0 = (n % c.CPB) * c.CN
            first_in_batch = t0 == 0
            tok = slice(n * c.CN, (n + 1) * c.CN)

            xcb = stream.tile([128, c.KT1, c.CN], BF16, tag="xcb")
            nc.sync.dma_start(out=xcb[:], in_=xt[:, :, tok])
            xc = [xcb[:, k, :] for k in range(c.KT1)]
            cpb = stream.tile([128, c.CT, c.CN], F16, tag="cpb")
            nc.sync.dma_start(out=cpb[:], in_=cp[:, :, t0:t0 + c.CN])
            spb = stream.tile([128, c.CT, c.CN], F16, tag="spb")
            nc.sync.dma_start(out=spb[:], in_=sp[:, :, t0:t0 + c.CN])

            ret_w = {}
            for h in range(NH):
                i0 = h * SEGS
                # ---- proj_in: 4 groups x SEGS channel tiles -> psum pairs ----
                # psum tile [128, 2*CN] holds channel tiles (j, j+1) of a group
                th_ph = wide.tile([128, W], F32, tag="th_ph", name="th_ph")
                th_mg = wide.tile([128, W], F32, tag="th_mg", name="th_mg")
                qre = wide.tile([128, W], F32, tag="qre", name="qre")
                qim = wide.tile([128, W], F32, tag="qim", name="qim")
                dest = {"ph": th_ph, "mg": th_mg, "qr": qre, "qi": qim}
                for j in range(0, SEGS, 2):
                    for gi, g in enumerate(("ph", "mg", "qr", "qi")):
                        p = praw.tile([128, 2 * c.CN], F32, tag="praw")
                        for half in range(2):
                            m = gi * c.CT + i0 + j + half
                            cols = slice(half * c.CN, (half + 1) * c.CN)
                            for k in range(kt1_lim):
                                nc.tensor.matmul(
                                    p[:, cols],
                                    w1_t[k][:, m * 128:(m + 1) * 128], xc[k],
                                    start=(k == 0), stop=(k == kt1_lim - 1))
                        wcols = slice(j * c.CN, (j + 2) * c.CN)
                        if g == "ph" or g == "mg":
                            sc = 1.0 if g == "ph" else 0.5
                            sca.activation(dest[g][:, wcols], p[:],
                                                 AF.Tanh, scale=sc)
                        elif "qdve" in pset:
                            vec.tensor_copy(dest[g][:, wcols], p[:])
                        else:
                            sca.copy(dest[g][:, wcols], p[:])

                # ---- content phasor (wide) ----
                sinp = wide.tile([128, W], F32, tag="sinp", name="sinp")
                sca.activation(sinp[:], th_ph[:], AF.Sin, scale=PI)
                tabs = wide.tile([128, W], F32, tag="tabs", name="tabs")
                sca.activation(tabs[:], th_ph[:], AF.Abs)
                cosp = wide.tile([128, W], F32, tag="th_ph", name="cosp")
                sca.activation(cosp[:], tabs[:], AF.Sin,
                                     bias=half_pi[:], scale=-PI)
                # 2*sigma = th_mg + 1 ; the 0.5 is folded into cp/sp on host
                ssin = wide.tile([128, W], F32, tag="tabs", name="ssin")
                vec.scalar_tensor_tensor(ssin[:], th_mg[:], 1.0, sinp[:],
                                               ALU.add, ALU.mult)
                scos = wide.tile([128, W], F32, tag="sinp", name="scos")
                vec.scalar_tensor_tensor(scos[:], th_mg[:], 1.0, cosp[:],
                                               ALU.add, ALU.mult)

                # ---- key = content * pos phasor (wide, cp/sp pre-halved) ----
                cps = cpb[:, i0:i0 + SEGS, :]
                sps = spb[:, i0:i0 + SEGS, :]
                ta = wide.tile([128, W], F32, tag="tmp1", name="ta")
                vec.tensor_mul(ta[:], scos[:], cps)
                tb = wide.tile([128, W], F32, tag="tmp2", name="tb")
                vec.tensor_mul(tb[:], ssin[:], sps)
                kre = wide.tile([128, W], F32, tag="kre", name="kre")
                vec.tensor_sub(kre[:], ta[:], tb[:])
                tc_ = wide.tile([128, W], F32, tag="tmp1", name="tc_")
                vec.tensor_mul(tc_[:], ssin[:], cps)
                td = wide.tile([128, W], F32, tag="tmp2", name="td")
                vec.tensor_mul(td[:], scos[:], sps)
                kim = wide.tile([128, W], F32, tag="kim", name="kim")
                vec.tensor_add(kim[:], tc_[:], td[:])

                # ---- prefix scan per channel tile segment ----
                mre = wide.tile([128, W], F32, tag="mre", name="mre")
                mim = wide.tile([128, W], F32, tag="mim", name="mim")
                for s in range(SEGS):
                    seg = slice(s * c.CN, (s + 1) * c.CN)
                    init_re = 0.0 if first_in_batch else car[(h, "re")][:, s:s + 1]
                    vec.tensor_tensor_scan(mre[:, seg], kre[:, seg],
                                                 kre[:, seg], init_re,
                                                 ALU.add, ALU.bypass)
                    init_im = 0.0 if first_in_batch else car[(h, "im")][:, s:s + 1]
                    vec.tensor_tensor_scan(mim[:, seg], kim[:, seg],
                                                 kim[:, seg], init_im,
                                                 ALU.add, ALU.bypass)
                if (n % c.CPB) != c.CPB - 1:
                    cre = mre.rearrange("p (s t) -> p s t", s=SEGS)[:, :, c.CN - 1]
                    vec.tensor_copy(car[(h, "re")][:], cre)
                    cim = mim.rearrange("p (s t) -> p s t", s=SEGS)[:, :, c.CN - 1]
                    vec.tensor_copy(car[(h, "im")][:], cim)

                # ---- retrieval = state * conj(q) (wide) ----
                r1 = wide.tile([128, W], F32, tag="tmp1", name="r1")
                vec.tensor_mul(r1[:], mre[:], qre[:])
                r2 = wide.tile([128, W], F32, tag="tmp2", name="r2")
                vec.tensor_mul(r2[:], mim[:], qim[:])
                rre = retp.tile([128, W], BF16, tag=f"ret_re_{h}",
                                name=f"ret_re_{h}")
                vec.tensor_add(rre[:], r1[:], r2[:])
                r3 = wide.tile([128, W], F32, tag="tmp1", name="r3")
                vec.tensor_mul(r3[:], mim[:], qre[:])
                r4 = wide.tile([128, W], F32, tag="tmp2", name="r4")
                vec.tensor_mul(r4[:], mre[:], qim[:])
                rim = retp.tile([128, W], BF16, tag=f"ret_im_{h}",
                                name=f"ret_im_{h}")
                vec.tensor_sub(rim[:], r3[:], r4[:])
                ret_w[(h, "re")] = rre
                ret_w[(h, "im")] = rim

            # ---- per-token stats via ones-matmuls ----
            ps1 = pstat.tile([1, c.CN], F32, tag="ps1")
            ps2 = pstat2.tile([1, c.CN], F32, tag="ps2")
            n_st = 2 * c.CT
            idx = 0
            for h in range(NH):
                for pl in ("re", "im"):
                    rw = ret_w[(h, pl)]
                    sq = wide.tile([128, W], BF16, tag="sq", name="sq")
                    vec.tensor_mul(sq[:], rw[:], rw[:])
                    if not stats_on:
                        continue
                    for s in range(SEGS):
                        seg = slice(s * c.CN, (s + 1) * c.CN)
                        nc.tensor.matmul(ps1[:], ones_bf[:], rw[:, seg],
                                         start=(idx == 0), stop=(idx == n_st - 1))
                        nc.tensor.matmul(ps2[:], ones_bf[:], sq[:, seg],
                                         start=(idx == 0), stop=(idx == n_st - 1))
                        idx += 1
            if not stats_on:
                nc.tensor.matmul(ps1[:], ones_bf[:], ret_w[(0, "re")][:, 0:c.CN],
                                 start=True, stop=True)
                nc.tensor.matmul(ps2[:], ones_bf[:], ret_w[(0, "im")][:, 0:c.CN],
                                 start=True, stop=True)
            s1c = stc.tile([1, c.CN], F32, tag="s1c", name="s1c")
            sca.copy(s1c[:], ps1[:])
            nc.sync.dma_start(out=stats[0:1, tok], in_=s1c[:])
            s2c = stc.tile([1, c.CN], F32, tag="s2c", name="s2c")
            sca.copy(s2c[:], ps2[:])
            nc.sync.dma_start(out=stats[1:2, tok], in_=s2c[:])

            # ---- proj_out partial (accumulate over all chpl tiles) ----
            ob = obp.tile([128, c.DT, c.CN], F32, tag="ob", name="ob")
            for d in range(c.DT):
                po = pout.tile([128, c.CN], F32, tag="pout")
                for k in range(kt2_lim):
                    if k < c.CT:
                        h, s, pl = k // SEGS, k % SEGS, "re"
                    else:
                        h, s, pl = (k - c.CT) // SEGS, (k - c.CT) % SEGS, "im"
                    rt = ret_w[(h, pl)][:, s * c.CN:(s + 1) * c.CN]
                    nc.tensor.matmul(po[:], w2_t[k][:, d * 128:(d + 1) * 128],
                                     rt, start=(k == 0), stop=(k == kt2_lim - 1))
                sca.copy(ob[:, d, :], po[:])
            nc.sync.dma_start(out=outp[:, :, tok], in_=ob[:])

    return nc


def build_program_v2(cfg: Cfg, reps: int = 1, hw_reps: int = 1,
                     probe: str | None = None):
    """v2: f16 datapath, CN=512, double-angle sin/cos (one pos tensor),
    stats folded on DVE (single ones-matmul per chunk), engine-balanced.

    key = sigma(mg)*exp(i*theta), theta = pi*tanh(ph) + pos.
    With th = theta/2 = (pi/2)*tanh(ph) + pos/2 (|th| <= pi, LUT-valid):
      sh = sin(th), ch = cos(th) = sin(pi/2 - |th|)
      kreN = (sh^2 - 0.5)*mg2 = -sigma*cos(theta)   (mg2 = tanh(mg/2)+1 = 2*sigma)
      kim  = sh*ch*mg2        =  sigma*sin(theta)
    The negated real part flows through the scan (SreN = -Sre); retrieval
    compensates: rre = mim*qim - mreN*qre ; rim = mim*qre + mreN*qim.
    """
    c = cfg
    assert c.CT % 4 == 0
    SEGS = 4
    NH = c.CT // SEGS
    W = SEGS * c.CN
    nc = bass.Bass()

    class _Dup:
        def __init__(self, eng, on):
            self._eng, self._on = eng, on

        def __getattr__(self, n):
            f = getattr(self._eng, n)
            if not self._on:
                return f

            def g(*a, **k):
                r = f(*a, **k)
                f(*a, **k)
                return r
            return g

    pset = set(probe.split(",")) if probe else set()
    vec = _Dup(nc.vector, "dve2" in pset)
    sca = _Dup(nc.scalar, "act2" in pset)
    kt1_lim = c.KT1 // 2 if "pein_half" in pset else c.KT1
    kt2_lim = c.KT2 // 2 if "peout_half" in pset else c.KT2

    fp8mag = "fp8mag" in pset
    pin2 = "pin512" in pset
    if fp8mag:
        w13 = nc.dram_tensor("w13", [128, c.KT1, 3 * c.NCH], F16,
                             kind="ExternalInput")
        w1m8 = nc.dram_tensor("w1m8", [128, c.KT1 // 2, 2, c.NCH], F8,
                              kind="ExternalInput")
        x8 = nc.dram_tensor("x8", [128, c.KT1, c.NTOK], F8,
                            kind="ExternalInput")
    else:
        w1 = nc.dram_tensor("w1", [128, c.KT1, 4 * c.NCH], F16,
                            kind="ExternalInput")
    w2 = nc.dram_tensor("w2", [128, c.KT2, c.DIM], F16, kind="ExternalInput")
    xt = nc.dram_tensor("xt", [128, c.KT1, c.NTOK], F16, kind="ExternalInput")
    pos = nc.dram_tensor("pos", [128, c.CT, c.T], F16, kind="ExternalInput")
    outp = nc.dram_tensor("outp", [128, c.DT, c.NTOK], BF16,
                          kind="ExternalOutput")
    stats = nc.dram_tensor("stats", [2, c.NTOK], F32, kind="ExternalOutput")

    from contextlib import ExitStack
    with tile.TileContext(nc) as tc, ExitStack() as es:
        small = c.CN <= 256
        praw_bufs = 4 if (small and not pin2) else 2
        pout_bufs = 2
        if "praw3" in pset:
            praw_bufs, pout_bufs = (6, 2) if small else (3, 1)
        consts = es.enter_context(tc.tile_pool(name="consts", bufs=1))
        stream = es.enter_context(tc.tile_pool(name="stream", bufs=2))
        wide_bufs = 2 if small else 1
        for p_ in pset:
            if p_.startswith("wb"):
                wide_bufs = int(p_[2:])
        wide = es.enter_context(tc.tile_pool(name="wide", bufs=wide_bufs))
        retp = es.enter_context(tc.tile_pool(name="retp", bufs=2 if small else 1))
        obp = es.enter_context(tc.tile_pool(name="obp",
                                            bufs=1 if (pin2 or not small) else 2))
        stb = es.enter_context(tc.tile_pool(name="stb", bufs=1 if pin2 else 2))
        stc = es.enter_context(tc.tile_pool(name="stc", bufs=1 if pin2 else 2))
        praw = es.enter_context(tc.tile_pool(name="praw", bufs=praw_bufs,
                                             space="PSUM"))
        pstat = es.enter_context(tc.tile_pool(name="pstat", bufs=1, space="PSUM"))
        pout = es.enter_context(tc.tile_pool(name="pout", bufs=pout_bufs,
                                             space="PSUM"))

        if fp8mag:
            w1_sb = consts.tile([128, c.KT1, 3 * c.NCH], F16, tag="w1_sb")
            nc.sync.dma_start(out=w1_sb[:], in_=w13[:])
            w1m8_sb = consts.tile([128, c.KT1 // 2, 2, c.NCH], F8,
                                  tag="w1m8_sb")
            nc.sync.dma_start(out=w1m8_sb[:], in_=w1m8[:])
        else:
            w1_sb = consts.tile([128, c.KT1, 4 * c.NCH], F16, tag="w1_sb")
            nc.sync.dma_start(out=w1_sb[:], in_=w1[:])
        w2_sb = consts.tile([128, c.KT2, c.DIM], F16, tag="w2_sb")
        nc.sync.dma_start(out=w2_sb[:], in_=w2[:])
        w1_t = [w1_sb[:, k, :] for k in range(c.KT1)]
        w2_t = [w2_sb[:, k, :] for k in range(c.KT2)]

        ones_bf = consts.tile([128, 1], BF16, tag="ones")
        vec.memset(ones_bf[:], 1.0)
        one_f = consts.tile([128, 1], F32, tag="one_f")
        vec.memset(one_f[:], 1.0)
        half_pi = consts.tile([128, 1], F32, tag="half_pi")
        vec.memset(half_pi[:], PI / 2)
        car = {}
        for h in range(NH):
            for pl in ("re", "im"):
                car[(h, pl)] = consts.tile([128, SEGS], F16, tag=f"car_{h}_{pl}",
                                           name=f"car_{h}_{pl}")

        if hw_reps > 1:
            es.enter_context(tc.For_i(0, hw_reps))

        h2 = W // 2

        def emit_tail_pre(st):
            """DVE folds + ACT squares for the previous chunk's stats."""
            rw = st["ret"]
            a = wide.tile([128, W], F16, tag="stA", bufs=1, name="a")
            vec.tensor_add(a[:], rw[(0, "re")][:], rw[(0, "im")][:])
            b = wide.tile([128, W], F16, tag="stB", bufs=1, name="b")
            vec.tensor_add(b[:], rw[(1, "re")][:], rw[(1, "im")][:])
            rs = wide.tile([128, W], F16, tag="stC", bufs=1, name="rs")
            vec.tensor_add(rs[:], a[:], b[:])
            f = wide.tile([128, h2], F16, tag="stD", bufs=1, name="f")
            vec.tensor_add(f[:], rs[:, 0:h2], rs[:, h2:W])
            stt = stb.tile([128, 2 * c.CN], BF16, tag="stt", name="stt")
            vec.tensor_add(stt[:, 0:c.CN], f[:, 0:c.CN], f[:, c.CN:h2])
            def _sq(dst, src):
                if "sqdve" in pset:
                    vec.tensor_mul(dst, src, src)
                else:
                    sca.activation(dst, src, AF.Square)
            s0 = wide.tile([128, W], BF16, tag="stA", bufs=1, name="s0")
            _sq(s0[:], rw[(0, "re")][:])
            s1 = wide.tile([128, W], BF16, tag="stB", bufs=1, name="s1")
            _sq(s1[:], rw[(0, "im")][:])
            a2 = wide.tile([128, W], BF16, tag="stC", bufs=1, name="a2")
            vec.tensor_add(a2[:], s0[:], s1[:])
            s2 = wide.tile([128, W], BF16, tag="stA", bufs=1, name="s2")
            _sq(s2[:], rw[(1, "re")][:])
            s3 = wide.tile([128, W], BF16, tag="stB", bufs=1, name="s3")
            _sq(s3[:], rw[(1, "im")][:])
            b2 = wide.tile([128, W], BF16, tag="stD", bufs=1, name="b2")
            vec.tensor_add(b2[:], s2[:], s3[:])
            ss = wide.tile([128, W], BF16, tag="stA", bufs=1, name="ss")
            vec.tensor_add(ss[:], a2[:], b2[:])
            f2 = wide.tile([128, h2], BF16, tag="stB", bufs=1, name="f2")
            vec.tensor_add(f2[:], ss[:, 0:h2], ss[:, h2:W])
            vec.tensor_add(stt[:, c.CN:2 * c.CN], f2[:, 0:c.CN],
                           f2[:, c.CN:h2])
            st["stt"] = stt

        def emit_tail_mm(st):
            """Stats matmul + proj_out for the previous chunk."""
            stt, tok_p, rw = st["stt"], st["tok"], st["ret"]
            ps = pstat.tile([1, 2 * c.CN], F32, tag="ps")
            nc.tensor.matmul(ps[:, 0:c.CN], ones_bf[:], stt[:, 0:c.CN],
                             start=True, stop=True)
            nc.tensor.matmul(ps[:, c.CN:2 * c.CN], ones_bf[:],
                             stt[:, c.CN:2 * c.CN], start=True, stop=True)
            sc = stc.tile([1, 2 * c.CN], F32, tag="sc", name="sc")
            sca.copy(sc[:], ps[:])
            nc.sync.dma_start(out=stats[0:1, tok_p], in_=sc[:, 0:c.CN])
            nc.sync.dma_start(out=stats[1:2, tok_p], in_=sc[:, c.CN:2 * c.CN])
            ob = obp.tile([128, c.DT, c.CN], BF16, tag="ob", name="ob")
            for d in range(c.DT):
                po = pout.tile([128, c.CN], F32, tag="pout")
                for k in range(kt2_lim):
                    if k < c.CT:
                        h, s, pl = k // SEGS, k % SEGS, "re"
                    else:
                        h, s, pl = (k - c.CT) // SEGS, (k - c.CT) % SEGS, "im"
                    rt = rw[(h, pl)][:, s * c.CN:(s + 1) * c.CN]
                    nc.tensor.matmul(po[:], w2_t[k][:, d * 128:(d + 1) * 128],
                                     rt, start=(k == 0), stop=(k == kt2_lim - 1))
                if "obdve" in pset:
                    vec.tensor_copy(ob[:, d, :], po[:])
                else:
                    sca.copy(ob[:, d, :], po[:])
            nc.sync.dma_start(out=outp[:, :, tok_p], in_=ob[:])

        prev = None
        pend_in = {}
        for n in [nn_ for _ in range(reps) for nn_ in range(c.NCHUNK)]:
            t0 = (n % c.CPB) * c.CN
            first_in_batch = t0 == 0
            tok = slice(n * c.CN, (n + 1) * c.CN)

            if prev is not None:
                emit_tail_pre(prev)

            posb = stream.tile([128, c.CT, c.CN], F16, tag="posb")
            nc.sync.dma_start(out=posb[:], in_=pos[:, :, t0:t0 + c.CN])

            gnames = {"ph": "th_ph", "mg": "th_mg", "qr": "qre", "qi": "qim"}
            if not pin2 or n % 2 == 0:
                ntin = 2 * c.CN if pin2 else c.CN
                itok = slice(n * c.CN, n * c.CN + ntin)
                sb = 1 if pin2 else None
                xcb = stream.tile([128, c.KT1, ntin], F16, tag="xcb", bufs=sb)
                nc.sync.dma_start(out=xcb[:], in_=xt[:, :, itok])
                if fp8mag:
                    x8cb = stream.tile([128, c.KT1, ntin], F8, tag="x8cb",
                                       bufs=sb)
                    nc.sync.dma_start(out=x8cb[:], in_=x8[:, :, itok])
                ib = 4 if pin2 else None
                cur_in, nxt_in = {}, {}
                for h in range(NH):
                    for nm in ("th_ph", "th_mg", "qre", "qim"):
                        cur_in[(h, nm)] = wide.tile([128, W], F16, tag=nm,
                                                    name=nm, bufs=ib)
                        if pin2:
                            nxt_in[(h, nm)] = wide.tile([128, W], F16, tag=nm,
                                                        name=nm + "b", bufs=ib)
                dmaps = [(cur_in, 0)] + ([(nxt_in, 1)] if pin2 else [])
                for h in range(NH):
                    i0 = h * SEGS
                    for j in range(0, SEGS, 2):
                        for g in ("ph", "mg", "qr", "qi"):
                            p = praw.tile([128, 2 * ntin], F32, tag="praw")
                            pv = p.rearrange("p (c t) -> p c t", c=2)
                            nm = gnames[g]
                            wcols = slice(j * c.CN, (j + 2) * c.CN)
                            if g == "mg" and fp8mag:
                                nk = c.KT1 // 2
                                for half in range(2):
                                    mch = i0 + j + half
                                    for kk in range(nk):
                                        nc.tensor.matmul(
                                            pv[:, half, :],
                                            w1m8_sb[:, kk, :,
                                                    mch * 128:(mch + 1) * 128],
                                            x8cb[:, 2 * kk:2 * kk + 2, :],
                                            start=(kk == 0),
                                            stop=(kk == nk - 1),
                                            perf_mode=mybir.MatmulPerfMode.DoubleRow)
                                for dmap, cc in dmaps:
                                    src = pv[:, :, cc * c.CN:(cc + 1) * c.CN]
                                    sca.activation(dmap[(h, nm)][:, wcols],
                                                   src, AF.Tanh,
                                                   scale=0.5 / 1024.0)
                                continue
                            if fp8mag:
                                gi = {"ph": 0, "qr": 1, "qi": 2}[g]
                            else:
                                gi = {"ph": 0, "mg": 1, "qr": 2, "qi": 3}[g]
                            for half in range(2):
                                m = gi * c.CT + i0 + j + half
                                for k in range(kt1_lim):
                                    nc.tensor.matmul(
                                        pv[:, half, :],
                                        w1_t[k][:, m * 128:(m + 1) * 128],
                                        xcb[:, k, :],
                                        start=(k == 0),
                                        stop=(k == kt1_lim - 1))
                            for dmap, cc in dmaps:
                                src = pv[:, :, cc * c.CN:(cc + 1) * c.CN]
                                dst = dmap[(h, nm)][:, wcols]
                                if g == "ph":
                                    sca.activation(dst, src, AF.Tanh)
                                elif g == "mg":
                                    sca.activation(dst, src, AF.Tanh, scale=0.5)
                                elif "qdve" in pset:
                                    vec.tensor_copy(dst, src)
                                else:
                                    sca.copy(dst, src)
                if pin2:
                    pend_in.clear()
                    pend_in.update(nxt_in)
            else:
                cur_in = dict(pend_in)

            ret_w = {}
            for h in range(NH):
                i0 = h * SEGS
                th_ph = cur_in[(h, "th_ph")]
                th_mg = cur_in[(h, "th_mg")]
                qre = cur_in[(h, "qre")]
                qim = cur_in[(h, "qim")]

                # th = (pi/2)*tanh(ph) + pos/2 ; th_ph already scaled by pi/2
                pos_h = posb[:, i0:i0 + SEGS, :]
                theta = wide.tile([128, W], F16, tag="theta", name="theta")
                vec.scalar_tensor_tensor(theta[:], th_ph[:], PI / 2, pos_h,
                                         ALU.mult, ALU.add)
                sh = wide.tile([128, W], F16, tag="sh", name="sh")
                sca.activation(sh[:], theta[:], AF.Sin)
                ab = wide.tile([128, W], F16, tag="ab", name="ab")
                if "abdve" in pset:
                    vec.tensor_scalar(ab[:], theta[:], 0.0, None, ALU.abs_max)
                else:
                    sca.activation(ab[:], theta[:], AF.Abs)
                ch = wide.tile([128, W], F16, tag="theta", name="ch")
                sca.activation(ch[:], ab[:], AF.Sin, bias=half_pi[:], scale=-1.0)
                sqh = wide.tile([128, W], F16, tag="ab", name="sqh")
                if "sqhdve" in pset:
                    vec.tensor_mul(sqh[:], sh[:], sh[:])
                else:
                    sca.activation(sqh[:], sh[:], AF.Square)
                mg2 = wide.tile([128, W], F16, tag="mg2", name="mg2")
                if "mg2dve" in pset:
                    vec.tensor_scalar(mg2[:], th_mg[:], 1.0, None, ALU.add)
                else:
                    sca.activation(mg2[:], th_mg[:], AF.Identity, bias=one_f[:])

                kreN = wide.tile([128, W], F16,
                                 tag="kreN" if pin2 else "th_ph", name="kreN")
                vec.scalar_tensor_tensor(kreN[:], sqh[:], 0.5, mg2[:],
                                         ALU.subtract, ALU.mult)
                tt = wide.tile([128, W], F16,
                               tag="tt" if pin2 else "th_mg", name="tt")
                vec.tensor_mul(tt[:], sh[:], ch[:])
                kim = wide.tile([128, W], F16, tag="sh", name="kim")
                vec.tensor_mul(kim[:], tt[:], mg2[:])

                mre = wide.tile([128, W], F16, tag="mre", name="mre")
                mim = wide.tile([128, W], F16, tag="mim", name="mim")
                for s in range(SEGS):
                    seg = slice(s * c.CN, (s + 1) * c.CN)
                    init_re = 0.0 if first_in_batch else car[(h, "re")][:, s:s + 1]
                    vec.tensor_tensor_scan(mre[:, seg], kreN[:, seg],
                                           kreN[:, seg], init_re,
                                           ALU.add, ALU.bypass)
                    init_im = 0.0 if first_in_batch else car[(h, "im")][:, s:s + 1]
                    vec.tensor_tensor_scan(mim[:, seg], kim[:, seg],
                                           kim[:, seg], init_im,
                                           ALU.add, ALU.bypass)
                if (n % c.CPB) != c.CPB - 1:
                    cre = mre.rearrange("p (s t) -> p s t", s=SEGS)[:, :, c.CN - 1]
                    vec.tensor_copy(car[(h, "re")][:], cre)
                    cim = mim.rearrange("p (s t) -> p s t", s=SEGS)[:, :, c.CN - 1]
                    vec.tensor_copy(car[(h, "im")][:], cim)

                # retrieval (mreN = -Sre):
                #   rre = mim*qim - mreN*qre ; rim = mim*qre + mreN*qim
                r1 = wide.tile([128, W], F16, tag="theta", name="r1")
                vec.tensor_mul(r1[:], mre[:], qre[:])
                r2 = wide.tile([128, W], F16, tag="ab", name="r2")
                vec.tensor_mul(r2[:], mim[:], qim[:])
                rre = retp.tile([128, W], F16, tag=f"ret_re_{h}",
                                name=f"ret_re_{h}")
                vec.tensor_sub(rre[:], r2[:], r1[:])
                r3 = wide.tile([128, W], F16, tag="theta", name="r3")
                vec.tensor_mul(r3[:], mim[:], qre[:])
                r4 = wide.tile([128, W], F16, tag="ab", name="r4")
                vec.tensor_mul(r4[:], mre[:], qim[:])
                rim = retp.tile([128, W], F16, tag=f"ret_im_{h}",
                                name=f"ret_im_{h}")
                vec.tensor_add(rim[:], r3[:], r4[:])
                ret_w[(h, "re")] = rre
                ret_w[(h, "im")] = rim

            if prev is not None:
                emit_tail_mm(prev)
            prev = {"ret": ret_w, "tok": tok}

        emit_tail_pre(prev)
        emit_tail_mm(prev)

    return nc


def build_program_v3(cfg: Cfg, reps: int = 1, hw_reps: int = 1,
                     probe: str | None = None):
    """v3: all-f16 matmuls (fp8 dropped -- measured no win on HW), ACT chain
    cut to 3 LUT ops/half, Pool (GPSIMD) engine recruited for the squares
    and sin-products, per-chunk emission ordered so every engine queue is
    dependency-ready (ph/mg GEMMs before q GEMMs, chain interleaved).

    Engine budget per 256-token chunk (target: PE-bound):
      PE   proj_in 256 MM + stats 2 + proj_out 128 MM        ~28.5us
      DVE  theta/mg2/kreN/kim, scans, retrieval, folds, ob   ~23us
      ACT  16 psum drains + sh/ab/ch + sc                    ~16.5us
      Pool sq/tt2 + stats squares                            ~17us
    """
    c = cfg
    assert c.CT % 4 == 0
    SEGS = 4
    NH = c.CT // SEGS
    W = SEGS * c.CN
    nc = bass.Bass()

    class _Dup:
        def __init__(self, eng, on):
            self._eng, self._on = eng, on

        def __getattr__(self, n):
            f = getattr(self._eng, n)
            if not self._on:
                return f

            def g(*a, **k):
                r = f(*a, **k)
                f(*a, **k)
                return r
            return g

    pset = set(probe.split(",")) if probe else set()
    vec = _Dup(nc.vector, "dve2" in pset)
    sca = _Dup(nc.scalar, "act2" in pset)
    pool = _Dup(nc.gpsimd, "pool2" in pset)
    kt1_lim = c.KT1 // 2 if "pein_half" in pset else c.KT1
    kt2_lim = c.KT2 // 2 if "peout_half" in pset else c.KT2
    if "sqact" in pset:                              # stats squares
        def emit_sq(dst, src):
            sca.activation(dst, src, AF.Square)
    elif "sqdve" in pset:
        def emit_sq(dst, src):
            vec.tensor_mul(dst, src, src)
    else:
        def emit_sq(dst, src):
            pool.tensor_mul(dst, src, src)
    ch_eng = vec if "poolchain_off" in pset else pool  # sq/tt2 in chain

    w1 = nc.dram_tensor("w1", [128, c.KT1, 4 * c.NCH], F16,
                        kind="ExternalInput")
    w2 = nc.dram_tensor("w2", [128, c.KT2, c.DIM], F16, kind="ExternalInput")
    xt = nc.dram_tensor("xt", [128, c.KT1, c.NTOK], F16, kind="ExternalInput")
    pos = nc.dram_tensor("pos", [128, c.CT, c.T], F16, kind="ExternalInput")
    DD = c.DT // 2
    outp = nc.dram_tensor("outp", [128, DD, 2, c.NTOK], BF16,
                          kind="ExternalOutput")
    stats = nc.dram_tensor("stats", [2, c.NTOK], F32, kind="ExternalOutput")

    from contextlib import ExitStack
    with tile.TileContext(nc) as tc, ExitStack() as es:
        praw_bufs = 4 if "praw4" in pset else 6
        consts = es.enter_context(tc.tile_pool(name="consts", bufs=1))
        stream = es.enter_context(tc.tile_pool(name="stream", bufs=2))
        wide = es.enter_context(tc.tile_pool(name="wide", bufs=2))
        retp = es.enter_context(tc.tile_pool(name="retp", bufs=2))
        obp = es.enter_context(tc.tile_pool(name="obp", bufs=2))
        stb = es.enter_context(tc.tile_pool(name="stb", bufs=2))
        stc = es.enter_context(tc.tile_pool(name="stc", bufs=2))
        praw = es.enter_context(tc.tile_pool(name="praw", bufs=praw_bufs,
                                             space="PSUM"))
        pstat = es.enter_context(tc.tile_pool(name="pstat", bufs=1,
                                              space="PSUM"))
        pout = es.enter_context(tc.tile_pool(name="pout", bufs=1,
                                             space="PSUM"))

        w1_sb = consts.tile([128, c.KT1, 4 * c.NCH], F16, tag="w1_sb")
        nc.sync.dma_start(out=w1_sb[:], in_=w1[:])
        w2_sb = consts.tile([128, c.KT2, c.DIM], F16, tag="w2_sb")
        nc.sync.dma_start(out=w2_sb[:], in_=w2[:])
        w1_t = [w1_sb[:, k, :] for k in range(c.KT1)]
        w2_t = [w2_sb[:, k, :] for k in range(c.KT2)]

        ones_bf = consts.tile([128, 1], BF16, tag="ones")
        vec.memset(ones_bf[:], 1.0)
        half_pi = consts.tile([128, 1], F32, tag="half_pi")
        vec.memset(half_pi[:], PI / 2)
        car = {}
        for h in range(NH):
            for pl in ("re", "im"):
                car[(h, pl)] = consts.tile([128, SEGS], F16,
                                           tag=f"car_{h}_{pl}",
                                           name=f"car_{h}_{pl}")

        if hw_reps > 1:
            es.enter_context(tc.For_i(0, hw_reps))

        h2 = W // 2

        def emit_tail_pre(st):
            """Stats for chunk n-1: Pool squares + DVE fold tree."""
            rw = st["ret"]
            sqs = {}
            for h in range(NH):
                for pl in ("re", "im"):
                    s = wide.tile([128, W], BF16, tag=f"sq_{h}_{pl}", bufs=1,
                                  name=f"s_{h}_{pl}")
                    emit_sq(s[:], rw[(h, pl)][:])
                    sqs[(h, pl)] = s
            a = wide.tile([128, W], F16, tag="stA", bufs=1, name="a")
            vec.tensor_add(a[:], rw[(0, "re")][:], rw[(0, "im")][:])
            b = wide.tile([128, W], F16, tag="stB", bufs=1, name="b")
            vec.tensor_add(b[:], rw[(1, "re")][:], rw[(1, "im")][:])
            rs = wide.tile([128, W], F16, tag="stC", bufs=1, name="rs")
            vec.tensor_add(rs[:], a[:], b[:])
            f = wide.tile([128, h2], F16, tag="stD", bufs=1, name="f")
            vec.tensor_add(f[:], rs[:, 0:h2], rs[:, h2:W])
            stt = stb.tile([128, 2 * c.CN], BF16, tag="stt", name="stt")
            vec.tensor_add(stt[:, 0:c.CN], f[:, 0:c.CN], f[:, c.CN:h2])
            a2 = wide.tile([128, W], BF16, tag="stA", bufs=1, name="a2")
            vec.tensor_add(a2[:], sqs[(0, "re")][:], sqs[(0, "im")][:])
            b2 = wide.tile([128, W], BF16, tag="stB", bufs=1, name="b2")
            vec.tensor_add(b2[:], sqs[(1, "re")][:], sqs[(1, "im")][:])
            ss = wide.tile([128, W], BF16, tag="stC", bufs=1, name="ss")
            vec.tensor_add(ss[:], a2[:], b2[:])
            f2 = wide.tile([128, h2], BF16, tag="stD", bufs=1, name="f2")
            vec.tensor_add(f2[:], ss[:, 0:h2], ss[:, h2:W])
            vec.tensor_add(stt[:, c.CN:2 * c.CN], f2[:, 0:c.CN],
                           f2[:, c.CN:h2])
            st["stt"] = stt

        def emit_tail_mm(st):
            """Stats matmuls + proj_out for chunk n-1."""
            stt, tok_p, rw = st["stt"], st["tok"], st["ret"]
            ps = pstat.tile([1, 2 * c.CN], F32, tag="ps")
            nc.tensor.matmul(ps[:, 0:c.CN], ones_bf[:], stt[:, 0:c.CN],
                             start=True, stop=True)
            nc.tensor.matmul(ps[:, c.CN:2 * c.CN], ones_bf[:],
                             stt[:, c.CN:2 * c.CN], start=True, stop=True)
            sc = stc.tile([1, 2 * c.CN], F32, tag="sc", name="sc")
            sca.copy(sc[:], ps[:])
            nc.sync.dma_start(out=stats[0:1, tok_p], in_=sc[:, 0:c.CN])
            nc.sync.dma_start(out=stats[1:2, tok_p], in_=sc[:, c.CN:2 * c.CN])
            for dd in range(DD):
                po = pout.tile([128, 2, c.CN], F32, tag="pout")
                for di in range(2):
                    d = dd * 2 + di
                    for k in range(kt2_lim):
                        if k < c.CT:
                            h, s, pl = k // SEGS, k % SEGS, "re"
                        else:
                            h, s, pl = ((k - c.CT) // SEGS,
                                        (k - c.CT) % SEGS, "im")
                        rt = rw[(h, pl)][:, s * c.CN:(s + 1) * c.CN]
                        nc.tensor.matmul(po[:, di, :],
                                         w2_t[k][:, d * 128:(d + 1) * 128],
                                         rt, start=(k == 0),
                                         stop=(k == kt2_lim - 1))
                ob = obp.tile([128, 2, c.CN], BF16, tag="ob", name="ob")
                if "obact" in pset:
                    sca.copy(ob[:], po[:])
                else:
                    vec.tensor_copy(ob[:], po[:])
                nc.sync.dma_start(out=outp[:, dd, :, tok_p], in_=ob[:])

        def drain_group(p, g, dst, wcols):
            if g == "ph":
                sca.activation(dst[:, wcols], p[:], AF.Tanh)
            elif g == "mg":
                sca.activation(dst[:, wcols], p[:], AF.Tanh, scale=0.5)
            else:
                sca.copy(dst[:, wcols], p[:])

        prev = None
        for n in [nn_ for _ in range(reps) for nn_ in range(c.NCHUNK)]:
            t0 = (n % c.CPB) * c.CN
            first_in_batch = t0 == 0
            tok = slice(n * c.CN, (n + 1) * c.CN)

            if prev is not None:
                emit_tail_pre(prev)

            posb = stream.tile([128, c.CT, c.CN], F16, tag="posb")
            nc.sync.dma_start(out=posb[:], in_=pos[:, :, t0:t0 + c.CN])
            xcb = stream.tile([128, c.KT1, c.CN], F16, tag="xcb")
            nc.sync.dma_start(out=xcb[:], in_=xt[:, :, tok])

            gidx = {"ph": 0, "mg": 1, "qr": 2, "qi": 3}
            ret_w = {}
            for h in range(NH):
                i0 = h * SEGS
                th_ph = wide.tile([128, W], F16, tag="th_ph", name="th_ph")
                th_mg = wide.tile([128, W], F16, tag="th_mg", name="th_mg")
                qre = wide.tile([128, W], F16, tag="qre", name="qre")
                qim = wide.tile([128, W], F16, tag="qim", name="qim")
                dest = {"ph": th_ph, "mg": th_mg, "qr": qre, "qi": qim}

                def gemm_pass(groups):
                    for j in (0, 2):
                        for g in groups:
                            p = praw.tile([128, 2 * c.CN], F32, tag="praw")
                            for half in range(2):
                                m = gidx[g] * c.CT + i0 + j + half
                                cols = slice(half * c.CN, (half + 1) * c.CN)
                                for k in range(kt1_lim):
                                    nc.tensor.matmul(
                                        p[:, cols],
                                        w1_t[k][:, m * 128:(m + 1) * 128],
                                        xcb[:, k, :],
                                        start=(k == 0),
                                        stop=(k == kt1_lim - 1))
                            drain_group(p, g, dest[g],
                                        slice(j * c.CN, (j + 2) * c.CN))

                # phase/magnitude GEMMs first: the chain head depends on them
                gemm_pass(("ph", "mg"))
                theta = wide.tile([128, W], F16, tag="theta", name="theta")
                vec.scalar_tensor_tensor(theta[:], th_ph[:], PI / 2,
                                         posb[:, i0:i0 + SEGS, :],
                                         ALU.mult, ALU.add)
                mg2 = wide.tile([128, W], F16, tag="mg2", name="mg2")
                vec.tensor_scalar(mg2[:], th_mg[:], 1.0, None, ALU.add)
                sh = wide.tile([128, W], F16, tag="sh", name="sh")
                sca.activation(sh[:], theta[:], AF.Sin)
                ab = wide.tile([128, W], F16, tag="ab", bufs=1, name="ab")
                sca.activation(ab[:], theta[:], AF.Abs)
                ch = wide.tile([128, W], F16, tag="ch", bufs=1, name="ch")
                sca.activation(ch[:], ab[:], AF.Sin, bias=half_pi[:],
                               scale=-1.0)
                sq = wide.tile([128, W], F16, tag="sq", bufs=1, name="sq")
                ch_eng.tensor_mul(sq[:], sh[:], sh[:])
                tt2 = wide.tile([128, W], F16, tag="tt2", bufs=1, name="tt2")
                ch_eng.tensor_mul(tt2[:], sh[:], ch[:])

                # query GEMMs while the chain runs on ACT/DVE/Pool
                gemm_pass(("qr", "qi"))

                kreN = wide.tile([128, W], F16, tag="kreN", bufs=1,
                                 name="kreN")
                vec.scalar_tensor_tensor(kreN[:], sq[:], 0.5, mg2[:],
                                         ALU.subtract, ALU.mult)
                kim = wide.tile([128, W], F16, tag="kim", bufs=1, name="kim")
                vec.tensor_mul(kim[:], tt2[:], mg2[:])

                mre = wide.tile([128, W], F16, tag="mre", name="mre")
                mim = wide.tile([128, W], F16, tag="mim", name="mim")
                for s in range(SEGS):
                    seg = slice(s * c.CN, (s + 1) * c.CN)
                    init_re = (0.0 if first_in_batch
                               else car[(h, "re")][:, s:s + 1])
                    vec.tensor_tensor_scan(mre[:, seg], kreN[:, seg],
                                           kreN[:, seg], init_re,
                                           ALU.add, ALU.bypass)
                    init_im = (0.0 if first_in_batch
                               else car[(h, "im")][:, s:s + 1])
                    vec.tensor_tensor_scan(mim[:, seg], kim[:, seg],
                                           kim[:, seg], init_im,
                                           ALU.add, ALU.bypass)
                if (n % c.CPB) != c.CPB - 1:
                    cre = mre.rearrange("p (s t) -> p s t", s=SEGS)[:, :,
                                                                    c.CN - 1]
                    vec.tensor_copy(car[(h, "re")][:], cre)
                    cim = mim.rearrange("p (s t) -> p s t", s=SEGS)[:, :,
                                                                    c.CN - 1]
                    vec.tensor_copy(car[(h, "im")][:], cim)

                # retrieval (mreN = -Sre):
                #   rre = mim*qim - mreN*qre ; rim = mim*qre + mreN*qim
                r1 = wide.tile([128, W], F16, tag="r1", bufs=1, name="r1")
                vec.tensor_mul(r1[:], mre[:], qre[:])
                r2 = wide.tile([128, W], F16, tag="r2", bufs=1, name="r2")
                vec.tensor_mul(r2[:], mim[:], qim[:])
                rre = retp.tile([128, W], F16, tag=f"ret_re_{h}",
                                name=f"ret_re_{h}")
                vec.tensor_sub(rre[:], r2[:], r1[:])
                r3 = wide.tile([128, W], F16, tag="r1", bufs=1, name="r3")
                vec.tensor_mul(r3[:], mim[:], qre[:])
                r4 = wide.tile([128, W], F16, tag="r2", bufs=1, name="r4")
                vec.tensor_mul(r4[:], mre[:], qim[:])
                rim = retp.tile([128, W], F16, tag=f"ret_im_{h}",
                                name=f"ret_im_{h}")
                vec.tensor_add(rim[:], r3[:], r4[:])
                ret_w[(h, "re")] = rre
                ret_w[(h, "im")] = rim

            if prev is not None:
                emit_tail_mm(prev)
            prev = {"ret": ret_w, "tok": tok}

        emit_tail_pre(prev)
        emit_tail_mm(prev)

    return nc


# --------------------------------------------------------------------------
# Host-side sharding / unsharding
# --------------------------------------------------------------------------
def shard_inputs(cfg, x, W_in, W_out, ln_gamma, ln_beta, pos_phases):
    c = cfg
    HD = N_CORES * c.NCH
    xT = np.ascontiguousarray(x.reshape(c.NTOK, c.DIM).T)          # [DIM, NTOK]
    # [p, k, tok] partition-major so one DMA covers all k-tiles of a chunk
    xt_h = np.ascontiguousarray(
        xT.reshape(c.KT1, 128, c.NTOK).transpose(1, 0, 2)
    ).astype(ml_dtypes.bfloat16)

    pos64 = pos_phases.astype(np.float64)
    cos_p = (0.5 * np.cos(pos64)).astype(np.float16)               # [T, HD]
    sin_p = (0.5 * np.sin(pos64)).astype(np.float16)

    Wg = (W_out * ln_gamma[None, :]).astype(np.float32)            # [DIM, 2HD]

    in_maps = []
    for cid in range(N_CORES):
        h0 = cid * c.NCH
        hs = slice(h0, h0 + c.NCH)
        w_ph = W_in[0 * HD + h0:0 * HD + h0 + c.NCH]               # [NCH, DIM]
        w_mg = W_in[1 * HD + h0:1 * HD + h0 + c.NCH]
        w_qr = W_in[2 * HD + h0:2 * HD + h0 + c.NCH]
        w_qi = W_in[3 * HD + h0:3 * HD + h0 + c.NCH]
        w_all = np.concatenate([w_ph, w_mg, w_qr, w_qi], axis=0)   # [4NCH, DIM]
        w1_h = np.ascontiguousarray(
            w_all.T.reshape(c.KT1, 128, 4 * c.NCH).transpose(1, 0, 2)
        ).astype(ml_dtypes.bfloat16)

        wg_re = Wg[:, 2 * h0:2 * (h0 + c.NCH):2]                   # [DIM, NCH]
        wg_im = Wg[:, 2 * h0 + 1:2 * (h0 + c.NCH):2]
        w2T = np.concatenate([wg_re.T, wg_im.T], axis=0)           # [2NCH, DIM]
        w2_h = np.ascontiguousarray(
            w2T.reshape(c.KT2, 128, c.DIM).transpose(1, 0, 2)
        ).astype(ml_dtypes.bfloat16)

        cp_h = np.ascontiguousarray(
            cos_p[:, hs].T.reshape(c.CT, 128, c.T).transpose(1, 0, 2))
        sp_h = np.ascontiguousarray(
            sin_p[:, hs].T.reshape(c.CT, 128, c.T).transpose(1, 0, 2))

        in_maps.append({
            "w1": w1_h, "w2": w2_h, "xt": xt_h,
            "cp": cp_h, "sp": sp_h,
        })
    return in_maps


def combine_outputs(cfg, results, W_out, ln_gamma, ln_beta, x_dtype):
    c = cfg
    NF = 2 * N_CORES * c.NCH
    P = np.zeros((c.DIM, c.NTOK), np.float64)
    S1 = np.zeros(c.NTOK, np.float64)
    S2 = np.zeros(c.NTOK, np.float64)
    for r in results:
        # outp is [128, DT, NTOK] partition-major of out^T -> [DIM, NTOK]
        op = r["outp"].transpose(1, 0, 2).reshape(c.DIM, c.NTOK)
        P += op.astype(np.float64)
        S1 += r["stats"][0].astype(np.float64)
        S2 += r["stats"][1].astype(np.float64)
    mu = S1 / NF
    var = S2 / NF - mu * mu
    istd = 1.0 / np.sqrt(var + LN_EPS)
    wg_sum = (W_out.astype(np.float64) @ ln_gamma.astype(np.float64))  # [DIM]
    b_out = (W_out.astype(np.float64) @ ln_beta.astype(np.float64))    # [DIM]
    out = istd[:, None] * (P.T - mu[:, None] * wg_sum[None, :]) + b_out[None, :]
    return out.reshape(c.B, c.T, c.DIM).astype(x_dtype)


def shard_inputs_v2(cfg, x, W_in, W_out, ln_gamma, ln_beta, pos_phases):
    c = cfg
    HD = N_CORES * c.NCH
    xT = np.ascontiguousarray(x.reshape(c.NTOK, c.DIM).T)          # [DIM, NTOK]
    xt_h = np.ascontiguousarray(
        xT.reshape(c.KT1, 128, c.NTOK).transpose(1, 0, 2)
    ).astype(np.float16)
    x8_h = np.ascontiguousarray(
        (xT * 16.0).reshape(c.KT1, 128, c.NTOK).transpose(1, 0, 2)
    ).astype(ml_dtypes.float8_e4m3)

    # pos/2, wrapped to [-pi/2, pi/2): theta_half = pi/2*tanh(ph) + pos/2
    pos64 = pos_phases.astype(np.float64)
    pos_half = (0.5 * (np.mod(pos64 + np.pi, 2 * np.pi) - np.pi)
                ).astype(np.float16)                               # [T, HD]

    Wg = (W_out * ln_gamma[None, :]).astype(np.float32)            # [DIM, 2HD]

    in_maps = []
    for cid in range(N_CORES):
        h0 = cid * c.NCH
        hs = slice(h0, h0 + c.NCH)
        w_ph = W_in[0 * HD + h0:0 * HD + h0 + c.NCH]
        w_mg = W_in[1 * HD + h0:1 * HD + h0 + c.NCH]
        w_qr = W_in[2 * HD + h0:2 * HD + h0 + c.NCH]
        w_qi = W_in[3 * HD + h0:3 * HD + h0 + c.NCH]
        w_all = np.concatenate([w_ph, w_mg, w_qr, w_qi], axis=0)   # [4NCH, DIM]
        w1_h = np.ascontiguousarray(
            w_all.T.reshape(c.KT1, 128, 4 * c.NCH).transpose(1, 0, 2)
        ).astype(np.float16)

        wg_re = Wg[:, 2 * h0:2 * (h0 + c.NCH):2]                   # [DIM, NCH]
        wg_im = Wg[:, 2 * h0 + 1:2 * (h0 + c.NCH):2]
        w2T = np.concatenate([wg_re.T, wg_im.T], axis=0)           # [2NCH, DIM]
        w2_h = np.ascontiguousarray(
            w2T.reshape(c.KT2, 128, c.DIM).transpose(1, 0, 2)
        ).astype(np.float16)

        pos_h = np.ascontiguousarray(
            pos_half[:, hs].T.reshape(c.CT, 128, c.T).transpose(1, 0, 2))

        # fp8(e4m3) copies for the magnitude channel (scales folded into
        # the on-chip tanh input scale: 0.5/(16*64)).
        w13_full = np.concatenate([w_ph, w_qr, w_qi], axis=0)
        w13_h = np.ascontiguousarray(
            w13_full.T.reshape(c.KT1, 128, 3 * c.NCH).transpose(1, 0, 2)
        ).astype(np.float16)
        wm8 = (w_mg.astype(np.float32) * 64.0).astype(
            ml_dtypes.float8_e4m3).astype(ml_dtypes.float8_e4m3)
        # layout [128, KT1//2, 2, NCH]: plane i of pair kk is k-tile 2kk+i
        wm8_h = np.ascontiguousarray(
            wm8.T.reshape(c.KT1 // 2, 2, 128, c.NCH).transpose(2, 0, 1, 3))
        in_maps.append({"w1": w1_h, "w2": w2_h, "xt": xt_h, "pos": pos_h,
                        "w13": w13_h, "w1m8": wm8_h, "x8": x8_h})
    return in_maps


def combine_outputs_v2(cfg, results, W_out, ln_gamma, ln_beta, x_dtype):
    c = cfg
    NF = 2 * N_CORES * c.NCH
    P = np.zeros((c.DIM, c.NTOK), np.float64)
    S1 = np.zeros(c.NTOK, np.float64)
    S2 = np.zeros(c.NTOK, np.float64)
    for r in results:
        op = r["outp"].transpose(1, 0, 2).reshape(c.DIM, c.NTOK)
        P += op.astype(np.float64)
        S1 += r["stats"][0].astype(np.float64)
        S2 += r["stats"][1].astype(np.float64)
    mu = S1 / NF
    var = S2 / NF - mu * mu
    istd = 1.0 / np.sqrt(var + LN_EPS)
    wg_sum = (W_out.astype(np.float64) @ ln_gamma.astype(np.float64))
    b_out = (W_out.astype(np.float64) @ ln_beta.astype(np.float64))
    out = istd[:, None] * (P.T - mu[:, None] * wg_sum[None, :]) + b_out[None, :]
    return out.reshape(c.B, c.T, c.DIM).astype(x_dtype)


def shard_inputs_v3(cfg, x, W_in, W_out, ln_gamma, ln_beta, pos_phases):
    c = cfg
    HD = N_CORES * c.NCH
    xT = np.ascontiguousarray(x.reshape(c.NTOK, c.DIM).T)          # [DIM, NTOK]
    xt_h = np.ascontiguousarray(
        xT.reshape(c.KT1, 128, c.NTOK).transpose(1, 0, 2)
    ).astype(np.float16)

    # pos/2, wrapped to [-pi/2, pi/2): theta_half = pi/2*tanh(ph) + pos/2
    pos64 = pos_phases.astype(np.float64)
    pos_half = (0.5 * (np.mod(pos64 + np.pi, 2 * np.pi) - np.pi)
                ).astype(np.float16)                               # [T, HD]

    Wg = (W_out * ln_gamma[None, :]).astype(np.float32)            # [DIM, 2HD]

    in_maps = []
    for cid in range(N_CORES):
        h0 = cid * c.NCH
        hs = slice(h0, h0 + c.NCH)
        w_ph = W_in[0 * HD + h0:0 * HD + h0 + c.NCH]
        w_mg = W_in[1 * HD + h0:1 * HD + h0 + c.NCH]
        w_qr = W_in[2 * HD + h0:2 * HD + h0 + c.NCH]
        w_qi = W_in[3 * HD + h0:3 * HD + h0 + c.NCH]
        w_all = np.concatenate([w_ph, w_mg, w_qr, w_qi], axis=0)   # [4NCH, DIM]
        w1_h = np.ascontiguousarray(
            w_all.T.reshape(c.KT1, 128, 4 * c.NCH).transpose(1, 0, 2)
        ).astype(np.float16)

        wg_re = Wg[:, 2 * h0:2 * (h0 + c.NCH):2]                   # [DIM, NCH]
        wg_im = Wg[:, 2 * h0 + 1:2 * (h0 + c.NCH):2]
        w2T = np.concatenate([wg_re.T, wg_im.T], axis=0)           # [2NCH, DIM]
        w2_h = np.ascontiguousarray(
            w2T.reshape(c.KT2, 128, c.DIM).transpose(1, 0, 2)
        ).astype(np.float16)

        pos_h = np.ascontiguousarray(
            pos_half[:, hs].T.reshape(c.CT, 128, c.T).transpose(1, 0, 2))
        in_maps.append({"w1": w1_h, "w2": w2_h, "xt": xt_h, "pos": pos_h})
    return in_maps


def combine_outputs_v3(cfg, results, W_out, ln_gamma, ln_beta, x_dtype):
    c = cfg
    NF = 2 * N_CORES * c.NCH
    P = np.zeros((c.DIM, c.NTOK), np.float64)
    S1 = np.zeros(c.NTOK, np.float64)
    S2 = np.zeros(c.NTOK, np.float64)
    for r in results:
        # outp [128, DD, 2, NTOK]: out[(dd*2+di)*128 + p, t]
        op = r["outp"].transpose(1, 2, 0, 3).reshape(c.DIM, c.NTOK)
        P += op.astype(np.float64)
        S1 += r["stats"][0].astype(np.float64)
        S2 += r["stats"][1].astype(np.float64)
    mu = S1 / NF
    var = S2 / NF - mu * mu
    istd = 1.0 / np.sqrt(var + LN_EPS)
    wg_sum = (W_out.astype(np.float64) @ ln_gamma.astype(np.float64))
    b_out = (W_out.astype(np.float64) @ ln_beta.astype(np.float64))
    out = istd[:, None] * (P.T - mu[:, None] * wg_sum[None, :]) + b_out[None, :]
    return out.reshape(c.B, c.T, c.DIM).astype(x_dtype)


import os

# v3: all-f16 datapath, Pool engine offload, no fp8.
DEFAULT_PROBE = ""


def _active_build(cfg, reps=1, hw_reps=1, probe=None):
    env = os.environ.get("KERNEL_PROBE")
    base = DEFAULT_PROBE if env is None else env
    merged = ",".join(x for x in [base, probe or ""] if x) or None
    return build_program_v3(cfg, reps=reps, hw_reps=hw_reps, probe=merged)


# Active implementation selector (test.py/bench use these too)
BUILD = _active_build
SHARD = shard_inputs_v3
COMBINE = combine_outputs_v3
CN_ACTIVE = 256

_cached = {}


def kernel(x, W_in, W_out, ln_gamma, ln_beta, pos_phases):
    cfg = Cfg(B=x.shape[0], T=x.shape[1], DIM=x.shape[2],
              NCH=pos_phases.shape[1] // N_CORES, CN=CN_ACTIVE)
    key = (cfg.B, cfg.T, cfg.DIM, cfg.NCH)
    if key not in _cached:
        nc = BUILD(cfg)
        split_multiwait(nc)  # walrus workaround; CoreSim path must skip this
        _cached[key] = nc
    nc = _cached[key]
    in_maps = SHARD(cfg, np.asarray(x), np.asarray(W_in),
                    np.asarray(W_out), np.asarray(ln_gamma),
                    np.asarray(ln_beta), np.asarray(pos_phases))
    # the native run path rejects in_map keys the program doesn't declare
    declared = {a.memorylocations[0].name
                for a in nc.m.functions[0].allocations
                if isinstance(a, mybir.MemoryLocationSet)
                and a.kind == "ExternalInput"}
    in_maps = [{k: v for k, v in m.items() if k in declared} for m in in_maps]
    res = run_bass_kernel_spmd(nc, in_maps, list(range(N_CORES)))
    return COMBINE(cfg, res.results, np.asarray(W_out),
                   np.asarray(ln_gamma), np.asarray(ln_beta),
                   np.asarray(x).dtype)



# revision 19
# speedup vs baseline: 1.7094x; 1.1340x over previous
"""Trainium2 Bass kernel for nn_LongAttention (holographic long-attention block).

Computation (see reference):
  raw = x @ W_in.T -> split [c_phase | c_mag | q_re | q_im] per hd channel
  key = sigmoid(c_mag) * exp(i*(pi*tanh(c_phase) + pos_phase))
  state = cumsum_t(key);  ret = state * conj(q)
  ret_real = interleave(Re, Im) -> LayerNorm(2*hd) -> @ W_out.T

Distribution: hd (8192) split across 8 NeuronCores (1024 ch each); every core
handles both batches and all tokens; cores are fully independent. gamma is
folded into W_out on the host and the LayerNorm is algebraically deferred:
each core returns P = ret @ (W_out*gamma).T partials plus per-token
S1 = sum_f ret, S2 = sum_f ret^2; the host combines
out = istd * (sum_c P_c - mu * (W_out @ gamma)) + W_out @ beta.

Active implementation (build_program_v2, CN=256-token chunks):
 - f16 datapath end to end (matmul inputs, elementwise, scan output) --
   same speed as bf16 everywhere but ~8x finer mantissa, plus 2x DVE
   perf-modes on the 16-bit elementwise ops.
 - The magnitude-channel GEMM runs in fp8(e4m3) with perf_mode=DoubleRow
   (2 k-planes per instruction); the quantization scales (x*16, W*64) are
   folded into the on-chip tanh input scale. Sigmoid's 1/4 slope damps the
   fp8 noise; measured end-to-end rel err 0.011 < 2e-2.
 - sin/cos via the half-angle identity: th = (pi/2)*tanh(ph) + pos/2 with
   pos pre-wrapped to [-pi, pi) on the host, so |th| <= pi stays inside
   the ACT Sin LUT range; cos(2th) = 1-2*sin^2(th) gives the real part
   without a second LUT pass over an out-of-range argument.
 - The cumsum runs channel-major on the DVE as a prefix scan along the free
   (time) axis (fp32 internal state), carried across token chunks.
 - Per-token LN stats are folded on DVE/ACT (tree adds + squares) into one
   [128, 2*CN] tile and reduced across partitions by a single pair of
   ones-matmuls -- instead of 32 PE matmuls per chunk.
 - stats + proj_out for chunk n-1 are emitted during chunk n (software
   pipelining) so the in-order PE queue never waits on the chunk's serial
   ACT<->DVE elementwise chain; all hot pools are double-buffered.
"""

import sys
import numpy as np
import ml_dtypes

for _p in ("/opt/trn_rl_repo", "/root/.axon_site/_ro/trn_rl_repo"):
    if _p not in sys.path:
        sys.path.append(_p)

import bass_rust
import concourse.bass as bass
import concourse.tile as tile
import concourse.mybir as mybir
from concourse.bass_utils import run_bass_kernel_spmd

F32 = mybir.dt.float32
F8 = mybir.dt.float8e4
F16 = mybir.dt.float16
BF16 = mybir.dt.bfloat16
AF = mybir.ActivationFunctionType
ALU = mybir.AluOpType
PI = float(np.pi)

N_CORES = 8
LN_EPS = 1e-5


# --------------------------------------------------------------------------
# Workaround: this container's walrus rejects >1 semaphore wait per
# instruction ("Too many sync wait commands"). Split the extras onto
# same-engine NoOps inserted just before (engine FIFO keeps semantics).
# --------------------------------------------------------------------------
_nop_counter = [0]


def split_multiwait(nc):
    n_split = 0
    for f in nc.m.functions:
        for bb in f.blocks:
            il = bb.instructions
            i = 0
            while i < len(il):
                ins = il[i]
                si = ins.sync_info
                waits = list(si.on_wait) if si is not None and si.on_wait else []
                if len(waits) > 1:
                    for w in waits[:-1]:
                        _nop_counter[0] += 1
                        nop = bass_rust.InstNoOp(
                            name=f"mw_nop_{_nop_counter[0]}",
                            engine=ins.engine,
                            ins=[],
                            outs=[],
                        )
                        nop.sync_info = mybir.SyncInfo(on_wait=[w], on_update=[])
                        il.insert(i, nop)
                        i += 1
                    si.on_wait = [waits[-1]]
                    n_split += 1
                i += 1
    return n_split


# --------------------------------------------------------------------------
# Device program (SPMD: identical on all cores; per-core data differs)
# --------------------------------------------------------------------------
class Cfg:
    def __init__(self, B=2, T=2048, DIM=1024, NCH=1024, CN=256):
        self.B, self.T, self.DIM, self.NCH, self.CN = B, T, DIM, NCH, CN
        self.NTOK = B * T
        self.CT = NCH // 128          # channel tiles per core
        self.KT1 = DIM // 128         # contraction tiles for proj_in
        self.KT2 = 2 * self.CT        # contraction tiles for proj_out (re+im)
        self.DT = DIM // 128          # output dim tiles
        self.NCHUNK = self.NTOK // CN
        self.CPB = T // CN            # chunks per batch


def build_program(cfg: Cfg, reps: int = 1, hw_reps: int = 1,
                  probe: str | None = None):
    c = cfg
    assert c.CT % 4 == 0 or c.CT == 2
    SEGS = 4 if c.CT % 4 == 0 else 2   # channel tiles per wide tile
    NH = c.CT // SEGS                  # wide halves per chunk
    W = SEGS * c.CN                    # wide tile width
    nc = bass.Bass()

    class _Dup:
        def __init__(self, eng, on):
            self._eng, self._on = eng, on

        def __getattr__(self, n):
            f = getattr(self._eng, n)
            if not self._on:
                return f

            def g(*a, **k):
                r = f(*a, **k)
                f(*a, **k)
                return r
            return g

    pset = set(probe.split(",")) if probe else set()
    vec = _Dup(nc.vector, "dve2" in pset)
    sca = _Dup(nc.scalar, "act2" in pset)
    kt1_lim = c.KT1 // 2 if "pein_half" in pset else c.KT1
    kt2_lim = c.KT2 // 2 if "peout_half" in pset else c.KT2
    stats_on = "stats_off" not in pset

    w1 = nc.dram_tensor("w1", [128, c.KT1, 4 * c.NCH], BF16, kind="ExternalInput")
    w2 = nc.dram_tensor("w2", [128, c.KT2, c.DIM], BF16, kind="ExternalInput")
    xt = nc.dram_tensor("xt", [128, c.KT1, c.NTOK], BF16, kind="ExternalInput")
    cp = nc.dram_tensor("cp", [128, c.CT, c.T], F16, kind="ExternalInput")
    sp = nc.dram_tensor("sp", [128, c.CT, c.T], F16, kind="ExternalInput")
    outp = nc.dram_tensor("outp", [128, c.DT, c.NTOK], F32, kind="ExternalOutput")
    stats = nc.dram_tensor("stats", [2, c.NTOK], F32, kind="ExternalOutput")

    from contextlib import ExitStack
    with tile.TileContext(nc) as tc, ExitStack() as es:
        consts = es.enter_context(tc.tile_pool(name="consts", bufs=1))
        stream = es.enter_context(tc.tile_pool(name="stream", bufs=2))
        wide = es.enter_context(tc.tile_pool(name="wide", bufs=1))
        retp = es.enter_context(tc.tile_pool(name="retp", bufs=2))
        obp = es.enter_context(tc.tile_pool(name="obp", bufs=1))
        stc = es.enter_context(tc.tile_pool(name="stc", bufs=2))
        praw = es.enter_context(tc.tile_pool(name="praw", bufs=4, space="PSUM"))
        pstat = es.enter_context(tc.tile_pool(name="pstat", bufs=1, space="PSUM"))
        pstat2 = es.enter_context(tc.tile_pool(name="pstat2", bufs=1, space="PSUM"))
        pout = es.enter_context(tc.tile_pool(name="pout", bufs=2, space="PSUM"))

        w1_sb = consts.tile([128, c.KT1, 4 * c.NCH], BF16, tag="w1_sb")
        nc.sync.dma_start(out=w1_sb[:], in_=w1[:])
        w2_sb = consts.tile([128, c.KT2, c.DIM], BF16, tag="w2_sb")
        nc.sync.dma_start(out=w2_sb[:], in_=w2[:])
        w1_t = [w1_sb[:, k, :] for k in range(c.KT1)]
        w2_t = [w2_sb[:, k, :] for k in range(c.KT2)]

        ones_bf = consts.tile([128, 1], BF16, tag="ones")
        vec.memset(ones_bf[:], 1.0)
        half_pi = consts.tile([128, 1], F32, tag="half_pi")
        vec.memset(half_pi[:], PI / 2)
        car = {}
        for h in range(NH):
            for pl in ("re", "im"):
                car[(h, pl)] = consts.tile([128, SEGS], F32, tag=f"car_{h}_{pl}",
                                           name=f"car_{h}_{pl}")

        if hw_reps > 1:
            es.enter_context(tc.For_i(0, hw_reps))

        for n in [nn_ for _ in range(reps) for nn_ in range(c.NCHUNK)]:
            t0 = (n % c.CPB) * c.CN
            first_in_batch = t0 == 0
            tok = slice(n * c.CN, (n + 1) * c.CN)

            xcb = stream.tile([128, c.KT1, c.CN], BF16, tag="xcb")
            nc.sync.dma_start(out=xcb[:], in_=xt[:, :, tok])
            xc = [xcb[:, k, :] for k in range(c.KT1)]
            cpb = stream.tile([128, c.CT, c.CN], F16, tag="cpb")
            nc.sync.dma_start(out=cpb[:], in_=cp[:, :, t0:t0 + c.CN])
            spb = stream.tile([128, c.CT, c.CN], F16, tag="spb")
            nc.sync.dma_start(out=spb[:], in_=sp[:, :, t0:t0 + c.CN])

            ret_w = {}
            for h in range(NH):
                i0 = h * SEGS
                # ---- proj_in: 4 groups x SEGS channel tiles -> psum pairs ----
                # psum tile [128, 2*CN] holds channel tiles (j, j+1) of a group
                th_ph = wide.tile([128, W], F32, tag="th_ph", name="th_ph")
                th_mg = wide.tile([128, W], F32, tag="th_mg", name="th_mg")
                qre = wide.tile([128, W], F32, tag="qre", name="qre")
                qim = wide.tile([128, W], F32, tag="qim", name="qim")
                dest = {"ph": th_ph, "mg": th_mg, "qr": qre, "qi": qim}
                for j in range(0, SEGS, 2):
                    for gi, g in enumerate(("ph", "mg", "qr", "qi")):
                        p = praw.tile([128, 2 * c.CN], F32, tag="praw")
                        for half in range(2):
                            m = gi * c.CT + i0 + j + half
                            cols = slice(half * c.CN, (half + 1) * c.CN)
                            for k in range(kt1_lim):
                                nc.tensor.matmul(
                                    p[:, cols],
                                    w1_t[k][:, m * 128:(m + 1) * 128], xc[k],
                                    start=(k == 0), stop=(k == kt1_lim - 1))
                        wcols = slice(j * c.CN, (j + 2) * c.CN)
                        if g == "ph" or g == "mg":
                            sc = 1.0 if g == "ph" else 0.5
                            sca.activation(dest[g][:, wcols], p[:],
                                                 AF.Tanh, scale=sc)
                        elif "qdve" in pset:
                            vec.tensor_copy(dest[g][:, wcols], p[:])
                        else:
                            sca.copy(dest[g][:, wcols], p[:])

                # ---- content phasor (wide) ----
                sinp = wide.tile([128, W], F32, tag="sinp", name="sinp")
                sca.activation(sinp[:], th_ph[:], AF.Sin, scale=PI)
                tabs = wide.tile([128, W], F32, tag="tabs", name="tabs")
                sca.activation(tabs[:], th_ph[:], AF.Abs)
                cosp = wide.tile([128, W], F32, tag="th_ph", name="cosp")
                sca.activation(cosp[:], tabs[:], AF.Sin,
                                     bias=half_pi[:], scale=-PI)
                # 2*sigma = th_mg + 1 ; the 0.5 is folded into cp/sp on host
                ssin = wide.tile([128, W], F32, tag="tabs", name="ssin")
                vec.scalar_tensor_tensor(ssin[:], th_mg[:], 1.0, sinp[:],
                                               ALU.add, ALU.mult)
                scos = wide.tile([128, W], F32, tag="sinp", name="scos")
                vec.scalar_tensor_tensor(scos[:], th_mg[:], 1.0, cosp[:],
                                               ALU.add, ALU.mult)

                # ---- key = content * pos phasor (wide, cp/sp pre-halved) ----
                cps = cpb[:, i0:i0 + SEGS, :]
                sps = spb[:, i0:i0 + SEGS, :]
                ta = wide.tile([128, W], F32, tag="tmp1", name="ta")
                vec.tensor_mul(ta[:], scos[:], cps)
                tb = wide.tile([128, W], F32, tag="tmp2", name="tb")
                vec.tensor_mul(tb[:], ssin[:], sps)
                kre = wide.tile([128, W], F32, tag="kre", name="kre")
                vec.tensor_sub(kre[:], ta[:], tb[:])
                tc_ = wide.tile([128, W], F32, tag="tmp1", name="tc_")
                vec.tensor_mul(tc_[:], ssin[:], cps)
                td = wide.tile([128, W], F32, tag="tmp2", name="td")
                vec.tensor_mul(td[:], scos[:], sps)
                kim = wide.tile([128, W], F32, tag="kim", name="kim")
                vec.tensor_add(kim[:], tc_[:], td[:])

                # ---- prefix scan per channel tile segment ----
                mre = wide.tile([128, W], F32, tag="mre", name="mre")
                mim = wide.tile([128, W], F32, tag="mim", name="mim")
                for s in range(SEGS):
                    seg = slice(s * c.CN, (s + 1) * c.CN)
                    init_re = 0.0 if first_in_batch else car[(h, "re")][:, s:s + 1]
                    vec.tensor_tensor_scan(mre[:, seg], kre[:, seg],
                                                 kre[:, seg], init_re,
                                                 ALU.add, ALU.bypass)
                    init_im = 0.0 if first_in_batch else car[(h, "im")][:, s:s + 1]
                    vec.tensor_tensor_scan(mim[:, seg], kim[:, seg],
                                                 kim[:, seg], init_im,
                                                 ALU.add, ALU.bypass)
                if (n % c.CPB) != c.CPB - 1:
                    cre = mre.rearrange("p (s t) -> p s t", s=SEGS)[:, :, c.CN - 1]
                    vec.tensor_copy(car[(h, "re")][:], cre)
                    cim = mim.rearrange("p (s t) -> p s t", s=SEGS)[:, :, c.CN - 1]
                    vec.tensor_copy(car[(h, "im")][:], cim)

                # ---- retrieval = state * conj(q) (wide) ----
                r1 = wide.tile([128, W], F32, tag="tmp1", name="r1")
                vec.tensor_mul(r1[:], mre[:], qre[:])
                r2 = wide.tile([128, W], F32, tag="tmp2", name="r2")
                vec.tensor_mul(r2[:], mim[:], qim[:])
                rre = retp.tile([128, W], BF16, tag=f"ret_re_{h}",
                                name=f"ret_re_{h}")
                vec.tensor_add(rre[:], r1[:], r2[:])
                r3 = wide.tile([128, W], F32, tag="tmp1", name="r3")
                vec.tensor_mul(r3[:], mim[:], qre[:])
                r4 = wide.tile([128, W], F32, tag="tmp2", name="r4")
                vec.tensor_mul(r4[:], mre[:], qim[:])
                rim = retp.tile([128, W], BF16, tag=f"ret_im_{h}",
                                name=f"ret_im_{h}")
                vec.tensor_sub(rim[:], r3[:], r4[:])
                ret_w[(h, "re")] = rre
                ret_w[(h, "im")] = rim

            # ---- per-token stats via ones-matmuls ----
            ps1 = pstat.tile([1, c.CN], F32, tag="ps1")
            ps2 = pstat2.tile([1, c.CN], F32, tag="ps2")
            n_st = 2 * c.CT
            idx = 0
            for h in range(NH):
                for pl in ("re", "im"):
                    rw = ret_w[(h, pl)]
                    sq = wide.tile([128, W], BF16, tag="sq", name="sq")
                    vec.tensor_mul(sq[:], rw[:], rw[:])
                    if not stats_on:
                        continue
                    for s in range(SEGS):
                        seg = slice(s * c.CN, (s + 1) * c.CN)
                        nc.tensor.matmul(ps1[:], ones_bf[:], rw[:, seg],
                                         start=(idx == 0), stop=(idx == n_st - 1))
                        nc.tensor.matmul(ps2[:], ones_bf[:], sq[:, seg],
                                         start=(idx == 0), stop=(idx == n_st - 1))
                        idx += 1
            if not stats_on:
                nc.tensor.matmul(ps1[:], ones_bf[:], ret_w[(0, "re")][:, 0:c.CN],
                                 start=True, stop=True)
                nc.tensor.matmul(ps2[:], ones_bf[:], ret_w[(0, "im")][:, 0:c.CN],
                                 start=True, stop=True)
            s1c = stc.tile([1, c.CN], F32, tag="s1c", name="s1c")
            sca.copy(s1c[:], ps1[:])
            nc.sync.dma_start(out=stats[0:1, tok], in_=s1c[:])
            s2c = stc.tile([1, c.CN], F32, tag="s2c", name="s2c")
            sca.copy(s2c[:], ps2[:])
            nc.sync.dma_start(out=stats[1:2, tok], in_=s2c[:])

            # ---- proj_out partial (accumulate over all chpl tiles) ----
            ob = obp.tile([128, c.DT, c.CN], F32, tag="ob", name="ob")
            for d in range(c.DT):
                po = pout.tile([128, c.CN], F32, tag="pout")
                for k in range(kt2_lim):
                    if k < c.CT:
                        h, s, pl = k // SEGS, k % SEGS, "re"
                    else:
                        h, s, pl = (k - c.CT) // SEGS, (k - c.CT) % SEGS, "im"
                    rt = ret_w[(h, pl)][:, s * c.CN:(s + 1) * c.CN]
                    nc.tensor.matmul(po[:], w2_t[k][:, d * 128:(d + 1) * 128],
                                     rt, start=(k == 0), stop=(k == kt2_lim - 1))
                sca.copy(ob[:, d, :], po[:])
            nc.sync.dma_start(out=outp[:, :, tok], in_=ob[:])

    return nc


def build_program_v2(cfg: Cfg, reps: int = 1, hw_reps: int = 1,
                     probe: str | None = None):
    """v2: f16 datapath, CN=512, double-angle sin/cos (one pos tensor),
    stats folded on DVE (single ones-matmul per chunk), engine-balanced.

    key = sigma(mg)*exp(i*theta), theta = pi*tanh(ph) + pos.
    With th = theta/2 = (pi/2)*tanh(ph) + pos/2 (|th| <= pi, LUT-valid):
      sh = sin(th), ch = cos(th) = sin(pi/2 - |th|)
      kreN = (sh^2 - 0.5)*mg2 = -sigma*cos(theta)   (mg2 = tanh(mg/2)+1 = 2*sigma)
      kim  = sh*ch*mg2        =  sigma*sin(theta)
    The negated real part flows through the scan (SreN = -Sre); retrieval
    compensates: rre = mim*qim - mreN*qre ; rim = mim*qre + mreN*qim.
    """
    c = cfg
    assert c.CT % 4 == 0
    SEGS = 4
    NH = c.CT // SEGS
    W = SEGS * c.CN
    nc = bass.Bass()

    class _Dup:
        def __init__(self, eng, on):
            self._eng, self._on = eng, on

        def __getattr__(self, n):
            f = getattr(self._eng, n)
            if not self._on:
                return f

            def g(*a, **k):
                r = f(*a, **k)
                f(*a, **k)
                return r
            return g

    pset = set(probe.split(",")) if probe else set()
    vec = _Dup(nc.vector, "dve2" in pset)
    sca = _Dup(nc.scalar, "act2" in pset)
    kt1_lim = c.KT1 // 2 if "pein_half" in pset else c.KT1
    kt2_lim = c.KT2 // 2 if "peout_half" in pset else c.KT2

    fp8mag = "fp8mag" in pset
    pin2 = "pin512" in pset
    if fp8mag:
        w13 = nc.dram_tensor("w13", [128, c.KT1, 3 * c.NCH], F16,
                             kind="ExternalInput")
        w1m8 = nc.dram_tensor("w1m8", [128, c.KT1 // 2, 2, c.NCH], F8,
                              kind="ExternalInput")
        x8 = nc.dram_tensor("x8", [128, c.KT1, c.NTOK], F8,
                            kind="ExternalInput")
    else:
        w1 = nc.dram_tensor("w1", [128, c.KT1, 4 * c.NCH], F16,
                            kind="ExternalInput")
    w2 = nc.dram_tensor("w2", [128, c.KT2, c.DIM], F16, kind="ExternalInput")
    xt = nc.dram_tensor("xt", [128, c.KT1, c.NTOK], F16, kind="ExternalInput")
    pos = nc.dram_tensor("pos", [128, c.CT, c.T], F16, kind="ExternalInput")
    outp = nc.dram_tensor("outp", [128, c.DT, c.NTOK], BF16,
                          kind="ExternalOutput")
    stats = nc.dram_tensor("stats", [2, c.NTOK], F32, kind="ExternalOutput")

    from contextlib import ExitStack
    with tile.TileContext(nc) as tc, ExitStack() as es:
        small = c.CN <= 256
        praw_bufs = 4 if ((small and not pin2) or "praw1b" in pset) else 2
        pout_bufs = 2
        if "praw3" in pset:
            praw_bufs, pout_bufs = (6, 2) if small else (3, 1)
        consts = es.enter_context(tc.tile_pool(name="consts", bufs=1))
        stream = es.enter_context(tc.tile_pool(name="stream", bufs=2))
        wide_bufs = 2 if small else 1
        for p_ in pset:
            if p_.startswith("wb"):
                wide_bufs = int(p_[2:])
        wide = es.enter_context(tc.tile_pool(name="wide", bufs=wide_bufs))
        retp = es.enter_context(tc.tile_pool(name="retp", bufs=2 if small else 1))
        obp = es.enter_context(tc.tile_pool(name="obp",
                                            bufs=1 if (pin2 or not small) else 2))
        small_stage = pin2 or "obsplit" in pset
        stb = es.enter_context(tc.tile_pool(name="stb",
                                            bufs=1 if small_stage else 2))
        stc = es.enter_context(tc.tile_pool(name="stc",
                                            bufs=1 if small_stage else 2))
        praw = es.enter_context(tc.tile_pool(name="praw", bufs=praw_bufs,
                                             space="PSUM"))
        pstat = es.enter_context(tc.tile_pool(name="pstat", bufs=1, space="PSUM"))
        pout = es.enter_context(tc.tile_pool(name="pout", bufs=pout_bufs,
                                             space="PSUM"))

        if fp8mag:
            w1_sb = consts.tile([128, c.KT1, 3 * c.NCH], F16, tag="w1_sb")
            nc.sync.dma_start(out=w1_sb[:], in_=w13[:])
            w1m8_sb = consts.tile([128, c.KT1 // 2, 2, c.NCH], F8,
                                  tag="w1m8_sb")
            nc.sync.dma_start(out=w1m8_sb[:], in_=w1m8[:])
        else:
            w1_sb = consts.tile([128, c.KT1, 4 * c.NCH], F16, tag="w1_sb")
            nc.sync.dma_start(out=w1_sb[:], in_=w1[:])
        w2_sb = consts.tile([128, c.KT2, c.DIM], F16, tag="w2_sb")
        nc.sync.dma_start(out=w2_sb[:], in_=w2[:])
        w1_t = [w1_sb[:, k, :] for k in range(c.KT1)]
        w2_t = [w2_sb[:, k, :] for k in range(c.KT2)]

        ones_bf = consts.tile([128, 1], BF16, tag="ones")
        vec.memset(ones_bf[:], 1.0)
        one_f = consts.tile([128, 1], F32, tag="one_f")
        vec.memset(one_f[:], 1.0)
        half_pi = consts.tile([128, 1], F32, tag="half_pi")
        vec.memset(half_pi[:], PI / 2)
        car = {}
        for h in range(NH):
            for pl in ("re", "im"):
                car[(h, pl)] = consts.tile([128, SEGS], F16, tag=f"car_{h}_{pl}",
                                           name=f"car_{h}_{pl}")

        if hw_reps > 1:
            es.enter_context(tc.For_i(0, hw_reps))

        h2 = W // 2

        def emit_tail_pre(st):
            """DVE folds + ACT squares for the previous chunk's stats."""
            rw = st["ret"]
            a = wide.tile([128, W], F16, tag="stA", bufs=1, name="a")
            vec.tensor_add(a[:], rw[(0, "re")][:], rw[(0, "im")][:])
            b = wide.tile([128, W], F16, tag="stB", bufs=1, name="b")
            vec.tensor_add(b[:], rw[(1, "re")][:], rw[(1, "im")][:])
            rs = wide.tile([128, W], F16, tag="stC", bufs=1, name="rs")
            vec.tensor_add(rs[:], a[:], b[:])
            f = wide.tile([128, h2], F16, tag="stD", bufs=1, name="f")
            vec.tensor_add(f[:], rs[:, 0:h2], rs[:, h2:W])
            stt = stb.tile([128, 2 * c.CN], BF16, tag="stt", name="stt")
            vec.tensor_add(stt[:, 0:c.CN], f[:, 0:c.CN], f[:, c.CN:h2])
            def _sq(dst, src):
                if "sqdve" in pset:
                    vec.tensor_mul(dst, src, src)
                else:
                    sca.activation(dst, src, AF.Square)
            s0 = wide.tile([128, W], BF16, tag="stA", bufs=1, name="s0")
            _sq(s0[:], rw[(0, "re")][:])
            s1 = wide.tile([128, W], BF16, tag="stB", bufs=1, name="s1")
            _sq(s1[:], rw[(0, "im")][:])
            a2 = wide.tile([128, W], BF16, tag="stC", bufs=1, name="a2")
            vec.tensor_add(a2[:], s0[:], s1[:])
            s2 = wide.tile([128, W], BF16, tag="stA", bufs=1, name="s2")
            _sq(s2[:], rw[(1, "re")][:])
            s3 = wide.tile([128, W], BF16, tag="stB", bufs=1, name="s3")
            _sq(s3[:], rw[(1, "im")][:])
            b2 = wide.tile([128, W], BF16, tag="stD", bufs=1, name="b2")
            vec.tensor_add(b2[:], s2[:], s3[:])
            ss = wide.tile([128, W], BF16, tag="stA", bufs=1, name="ss")
            vec.tensor_add(ss[:], a2[:], b2[:])
            f2 = wide.tile([128, h2], BF16, tag="stB", bufs=1, name="f2")
            vec.tensor_add(f2[:], ss[:, 0:h2], ss[:, h2:W])
            vec.tensor_add(stt[:, c.CN:2 * c.CN], f2[:, 0:c.CN],
                           f2[:, c.CN:h2])
            st["stt"] = stt

        def emit_tail_mm(st):
            """Stats matmul + proj_out for the previous chunk."""
            stt, tok_p, rw = st["stt"], st["tok"], st["ret"]
            ps = pstat.tile([1, 2 * c.CN], F32, tag="ps")
            nc.tensor.matmul(ps[:, 0:c.CN], ones_bf[:], stt[:, 0:c.CN],
                             start=True, stop=True)
            nc.tensor.matmul(ps[:, c.CN:2 * c.CN], ones_bf[:],
                             stt[:, c.CN:2 * c.CN], start=True, stop=True)
            obsplit = "obsplit" in pset
            if obsplit:
                sc1 = stc.tile([1, c.CN], F32, tag="sc", name="sc1")
                sca.copy(sc1[:], ps[:, 0:c.CN])
                nc.sync.dma_start(out=stats[0:1, tok_p], in_=sc1[:])
                sc2 = stc.tile([1, c.CN], F32, tag="sc", name="sc2")
                sca.copy(sc2[:], ps[:, c.CN:2 * c.CN])
                nc.sync.dma_start(out=stats[1:2, tok_p], in_=sc2[:])
            else:
                sc = stc.tile([1, 2 * c.CN], F32, tag="sc", name="sc")
                sca.copy(sc[:], ps[:])
                nc.sync.dma_start(out=stats[0:1, tok_p], in_=sc[:, 0:c.CN])
                nc.sync.dma_start(out=stats[1:2, tok_p],
                                  in_=sc[:, c.CN:2 * c.CN])
            if not obsplit:
                ob = obp.tile([128, c.DT, c.CN], BF16, tag="ob", name="ob")
            for d in range(c.DT):
                po = pout.tile([128, c.CN], F32, tag="pout")
                for k in range(kt2_lim):
                    if k < c.CT:
                        h, s, pl = k // SEGS, k % SEGS, "re"
                    else:
                        h, s, pl = (k - c.CT) // SEGS, (k - c.CT) % SEGS, "im"
                    rt = rw[(h, pl)][:, s * c.CN:(s + 1) * c.CN]
                    nc.tensor.matmul(po[:], w2_t[k][:, d * 128:(d + 1) * 128],
                                     rt, start=(k == 0), stop=(k == kt2_lim - 1))
                if obsplit:
                    obd = obp.tile([128, c.CN], BF16, tag="ob", bufs=2,
                                   name=f"ob{d}")
                    if "obdve" in pset:
                        vec.tensor_copy(obd[:], po[:])
                    else:
                        sca.copy(obd[:], po[:])
                    nc.sync.dma_start(out=outp[:, d, tok_p], in_=obd[:])
                elif "obdve" in pset:
                    vec.tensor_copy(ob[:, d, :], po[:])
                else:
                    sca.copy(ob[:, d, :], po[:])
            if not obsplit:
                nc.sync.dma_start(out=outp[:, :, tok_p], in_=ob[:])

        prev = None
        pend_in = {}
        for n in [nn_ for _ in range(reps) for nn_ in range(c.NCHUNK)]:
            t0 = (n % c.CPB) * c.CN
            first_in_batch = t0 == 0
            tok = slice(n * c.CN, (n + 1) * c.CN)

            if prev is not None:
                emit_tail_pre(prev)

            if "obsplit" in pset:
                posb_h = []
                for h in range(NH):
                    pb = stream.tile([128, SEGS, c.CN], F16, tag=f"posb{h}")
                    nc.sync.dma_start(
                        out=pb[:],
                        in_=pos[:, h * SEGS:(h + 1) * SEGS, t0:t0 + c.CN])
                    posb_h.append(pb)
            else:
                posb = stream.tile([128, c.CT, c.CN], F16, tag="posb")
                nc.sync.dma_start(out=posb[:], in_=pos[:, :, t0:t0 + c.CN])

            gnames = {"ph": "th_ph", "mg": "th_mg", "qr": "qre", "qi": "qim"}
            if not pin2 or n % 2 == 0:
                ntin = 2 * c.CN if pin2 else c.CN
                itok = slice(n * c.CN, n * c.CN + ntin)
                sb = 1 if pin2 else None
                xcb = stream.tile([128, c.KT1, ntin], F16, tag="xcb", bufs=sb)
                nc.sync.dma_start(out=xcb[:], in_=xt[:, :, itok])
                if fp8mag:
                    x8cb = stream.tile([128, c.KT1, ntin], F8, tag="x8cb",
                                       bufs=sb)
                    nc.sync.dma_start(out=x8cb[:], in_=x8[:, :, itok])
                ib = 4 if pin2 else None
                cur_in, nxt_in = {}, {}
                for h in range(NH):
                    for nm in ("th_ph", "th_mg", "qre", "qim"):
                        cur_in[(h, nm)] = wide.tile([128, W], F16, tag=nm,
                                                    name=nm, bufs=ib)
                        if pin2:
                            nxt_in[(h, nm)] = wide.tile([128, W], F16, tag=nm,
                                                        name=nm + "b", bufs=ib)
                dmaps = [(cur_in, 0)] + ([(nxt_in, 1)] if pin2 else [])
                if "praw1b" in pset:
                    # one-bank psum tiles: one channel tile per GEMM group
                    assert not pin2
                    for h in range(NH):
                        i0 = h * SEGS
                        for j in range(SEGS):
                            for g in ("ph", "mg", "qr", "qi"):
                                p = praw.tile([128, c.CN], F32, tag="praw")
                                nm = gnames[g]
                                wcols = slice(j * c.CN, (j + 1) * c.CN)
                                dst = cur_in[(h, nm)][:, wcols]
                                if g == "mg" and fp8mag:
                                    nk = c.KT1 // 2
                                    mch = i0 + j
                                    for kk in range(nk):
                                        nc.tensor.matmul(
                                            p[:],
                                            w1m8_sb[:, kk, :,
                                                    mch * 128:
                                                    (mch + 1) * 128],
                                            x8cb[:, 2 * kk:2 * kk + 2, :],
                                            start=(kk == 0),
                                            stop=(kk == nk - 1),
                                            perf_mode=mybir.MatmulPerfMode
                                            .DoubleRow)
                                    sca.activation(dst, p[:], AF.Tanh,
                                                   scale=0.5 / 1024.0)
                                    continue
                                if fp8mag:
                                    gi = {"ph": 0, "qr": 1, "qi": 2}[g]
                                else:
                                    gi = {"ph": 0, "mg": 1, "qr": 2,
                                          "qi": 3}[g]
                                m = gi * c.CT + i0 + j
                                for k in range(kt1_lim):
                                    nc.tensor.matmul(
                                        p[:],
                                        w1_t[k][:, m * 128:(m + 1) * 128],
                                        xcb[:, k, :],
                                        start=(k == 0),
                                        stop=(k == kt1_lim - 1))
                                if g == "ph":
                                    sca.activation(dst, p[:], AF.Tanh)
                                elif g == "mg":
                                    sca.activation(dst, p[:], AF.Tanh,
                                                   scale=0.5)
                                elif "qdve" in pset:
                                    vec.tensor_copy(dst, p[:])
                                else:
                                    sca.copy(dst, p[:])
                    # fall through to the chain loop below
                for h in [] if "praw1b" in pset else range(NH):
                    i0 = h * SEGS
                    for j in range(0, SEGS, 2):
                        for g in ("ph", "mg", "qr", "qi"):
                            p = praw.tile([128, 2 * ntin], F32, tag="praw")
                            pv = p.rearrange("p (c t) -> p c t", c=2)
                            nm = gnames[g]
                            wcols = slice(j * c.CN, (j + 2) * c.CN)
                            if g == "mg" and fp8mag:
                                nk = c.KT1 // 2
                                for half in range(2):
                                    mch = i0 + j + half
                                    for kk in range(nk):
                                        nc.tensor.matmul(
                                            pv[:, half, :],
                                            w1m8_sb[:, kk, :,
                                                    mch * 128:(mch + 1) * 128],
                                            x8cb[:, 2 * kk:2 * kk + 2, :],
                                            start=(kk == 0),
                                            stop=(kk == nk - 1),
                                            perf_mode=mybir.MatmulPerfMode.DoubleRow)
                                for dmap, cc in dmaps:
                                    src = pv[:, :, cc * c.CN:(cc + 1) * c.CN]
                                    sca.activation(dmap[(h, nm)][:, wcols],
                                                   src, AF.Tanh,
                                                   scale=0.5 / 1024.0)
                                continue
                            if fp8mag:
                                gi = {"ph": 0, "qr": 1, "qi": 2}[g]
                            else:
                                gi = {"ph": 0, "mg": 1, "qr": 2, "qi": 3}[g]
                            for half in range(2):
                                m = gi * c.CT + i0 + j + half
                                for k in range(kt1_lim):
                                    nc.tensor.matmul(
                                        pv[:, half, :],
                                        w1_t[k][:, m * 128:(m + 1) * 128],
                                        xcb[:, k, :],
                                        start=(k == 0),
                                        stop=(k == kt1_lim - 1))
                            for dmap, cc in dmaps:
                                src = pv[:, :, cc * c.CN:(cc + 1) * c.CN]
                                dst = dmap[(h, nm)][:, wcols]
                                if g == "ph":
                                    sca.activation(dst, src, AF.Tanh)
                                elif g == "mg":
                                    sca.activation(dst, src, AF.Tanh, scale=0.5)
                                elif "qdve" in pset:
                                    vec.tensor_copy(dst, src)
                                else:
                                    sca.copy(dst, src)
                if pin2:
                    pend_in.clear()
                    pend_in.update(nxt_in)
            else:
                cur_in = dict(pend_in)

            ret_w = {}
            hs = {h: {} for h in range(NH)}

            def lv_theta(h):
                st = hs[h]
                i0 = h * SEGS
                pos_h = (posb_h[h][:] if "obsplit" in pset
                         else posb[:, i0:i0 + SEGS, :])
                theta = wide.tile([128, W], F16, tag="theta", name="theta")
                vec.scalar_tensor_tensor(theta[:], cur_in[(h, "th_ph")][:],
                                         PI / 2, pos_h, ALU.mult, ALU.add)
                st["theta"] = theta

            def lv_sh(h):
                st = hs[h]
                sh = wide.tile([128, W], F16, tag="sh", name="sh")
                sca.activation(sh[:], st["theta"][:], AF.Sin)
                st["sh"] = sh

            def lv_ab(h):
                st = hs[h]
                ab = wide.tile([128, W], F16, tag="ab", name="ab")
                if "abdve" in pset:
                    vec.scalar_tensor_tensor(ab[:], st["theta"][:], -1.0,
                                             st["theta"][:],
                                             ALU.mult, ALU.max)
                else:
                    sca.activation(ab[:], st["theta"][:], AF.Abs)
                st["ab"] = ab

            def lv_ch(h):
                st = hs[h]
                ch = wide.tile([128, W], F16, tag="theta", name="ch")
                sca.activation(ch[:], st["ab"][:], AF.Sin, bias=half_pi[:],
                               scale=-1.0)
                st["ch"] = ch

            def lv_sqh(h):
                st = hs[h]
                sqh = wide.tile([128, W], F16, tag="ab", name="sqh")
                if "sqhdve" in pset:
                    vec.tensor_mul(sqh[:], st["sh"][:], st["sh"][:])
                else:
                    sca.activation(sqh[:], st["sh"][:], AF.Square)
                st["sqh"] = sqh

            def lv_mg2(h):
                st = hs[h]
                mg2 = wide.tile([128, W], F16, tag="mg2", name="mg2")
                if "mg2dve" in pset:
                    vec.tensor_scalar(mg2[:], cur_in[(h, "th_mg")][:], 1.0,
                                      None, ALU.add)
                else:
                    sca.activation(mg2[:], cur_in[(h, "th_mg")][:],
                                   AF.Identity, bias=one_f[:])
                st["mg2"] = mg2

            def lv_kreN(h):
                st = hs[h]
                kreN = wide.tile([128, W], F16,
                                 tag="kreN" if pin2 else "th_ph", name="kreN")
                vec.scalar_tensor_tensor(kreN[:], st["sqh"][:], 0.5,
                                         st["mg2"][:],
                                         ALU.subtract, ALU.mult)
                st["kreN"] = kreN

            def lv_tt(h):
                st = hs[h]
                tt = wide.tile([128, W], F16,
                               tag="tt" if pin2 else "th_mg", name="tt")
                vec.tensor_mul(tt[:], st["sh"][:], st["ch"][:])
                st["tt"] = tt

            def lv_kim(h):
                st = hs[h]
                kim = wide.tile([128, W], F16, tag="sh", name="kim")
                vec.tensor_mul(kim[:], st["tt"][:], st["mg2"][:])
                st["kim"] = kim

            def lv_scan(h):
                st = hs[h]
                mre = wide.tile([128, W], F16, tag="mre", name="mre")
                mim = wide.tile([128, W], F16, tag="mim", name="mim")
                for s in range(SEGS):
                    seg = slice(s * c.CN, (s + 1) * c.CN)
                    init_re = (0.0 if first_in_batch
                               else car[(h, "re")][:, s:s + 1])
                    vec.tensor_tensor_scan(mre[:, seg], st["kreN"][:, seg],
                                           st["kreN"][:, seg], init_re,
                                           ALU.add, ALU.bypass)
                    init_im = (0.0 if first_in_batch
                               else car[(h, "im")][:, s:s + 1])
                    vec.tensor_tensor_scan(mim[:, seg], st["kim"][:, seg],
                                           st["kim"][:, seg], init_im,
                                           ALU.add, ALU.bypass)
                st["mre"], st["mim"] = mre, mim

            def lv_carry(h):
                st = hs[h]
                if (n % c.CPB) != c.CPB - 1:
                    cre = st["mre"].rearrange("p (s t) -> p s t",
                                              s=SEGS)[:, :, c.CN - 1]
                    vec.tensor_copy(car[(h, "re")][:], cre)
                    cim = st["mim"].rearrange("p (s t) -> p s t",
                                              s=SEGS)[:, :, c.CN - 1]
                    vec.tensor_copy(car[(h, "im")][:], cim)

            def lv_retre(h):
                # retrieval (mreN = -Sre):
                #   rre = mim*qim - mreN*qre ; rim = mim*qre + mreN*qim
                st = hs[h]
                qre, qim = cur_in[(h, "qre")], cur_in[(h, "qim")]
                r1 = wide.tile([128, W], F16, tag="theta", name="r1")
                vec.tensor_mul(r1[:], st["mre"][:], qre[:])
                r2 = wide.tile([128, W], F16, tag="ab", name="r2")
                vec.tensor_mul(r2[:], st["mim"][:], qim[:])
                rre = retp.tile([128, W], F16, tag=f"ret_re_{h}",
                                name=f"ret_re_{h}")
                vec.tensor_sub(rre[:], r2[:], r1[:])
                ret_w[(h, "re")] = rre

            def lv_retim(h):
                st = hs[h]
                qre, qim = cur_in[(h, "qre")], cur_in[(h, "qim")]
                r3 = wide.tile([128, W], F16, tag="theta", name="r3")
                vec.tensor_mul(r3[:], st["mim"][:], qre[:])
                r4 = wide.tile([128, W], F16, tag="ab", name="r4")
                vec.tensor_mul(r4[:], st["mre"][:], qim[:])
                rim = retp.tile([128, W], F16, tag=f"ret_im_{h}",
                                name=f"ret_im_{h}")
                vec.tensor_add(rim[:], r3[:], r4[:])
                ret_w[(h, "im")] = rim

            levels = [lv_theta, lv_sh, lv_ab, lv_ch, lv_sqh, lv_mg2,
                      lv_kreN, lv_tt, lv_kim, lv_scan, lv_carry,
                      lv_retre, lv_retim]
            if "ilv" in pset:
                for lv in levels:
                    for h in range(NH):
                        lv(h)
            else:
                for h in range(NH):
                    for lv in levels:
                        lv(h)

            if prev is not None:
                emit_tail_mm(prev)
            prev = {"ret": ret_w, "tok": tok}

        emit_tail_pre(prev)
        emit_tail_mm(prev)

    return nc


def build_program_v3(cfg: Cfg, reps: int = 1, hw_reps: int = 1,
                     probe: str | None = None):
    """v3: all-f16 matmuls (fp8 dropped -- measured no win on HW), ACT chain
    cut to 3 LUT ops/half, Pool (GPSIMD) engine recruited for the squares
    and sin-products, per-chunk emission ordered so every engine queue is
    dependency-ready (ph/mg GEMMs before q GEMMs, chain interleaved).

    Engine budget per 256-token chunk (target: PE-bound):
      PE   proj_in 256 MM + stats 2 + proj_out 128 MM        ~28.5us
      DVE  theta/mg2/kreN/kim, scans, retrieval, folds, ob   ~23us
      ACT  16 psum drains + sh/ab/ch + sc                    ~16.5us
      Pool sq/tt2 + stats squares                            ~17us
    """
    c = cfg
    assert c.CT % 4 == 0
    SEGS = 4
    NH = c.CT // SEGS
    W = SEGS * c.CN
    nc = bass.Bass()

    class _Dup:
        def __init__(self, eng, on):
            self._eng, self._on = eng, on

        def __getattr__(self, n):
            f = getattr(self._eng, n)
            if not self._on:
                return f

            def g(*a, **k):
                r = f(*a, **k)
                f(*a, **k)
                return r
            return g

    pset = set(probe.split(",")) if probe else set()
    vec = _Dup(nc.vector, "dve2" in pset)
    sca = _Dup(nc.scalar, "act2" in pset)
    pool = _Dup(nc.gpsimd, "pool2" in pset)
    kt1_lim = c.KT1 // 2 if "pein_half" in pset else c.KT1
    kt2_lim = c.KT2 // 2 if "peout_half" in pset else c.KT2
    if "sqact" in pset:                              # stats squares
        def emit_sq(dst, src):
            sca.activation(dst, src, AF.Square)
    elif "sqdve" in pset:
        def emit_sq(dst, src):
            vec.tensor_mul(dst, src, src)
    else:
        def emit_sq(dst, src):
            pool.tensor_mul(dst, src, src)
    ch_eng = vec if "poolchain_off" in pset else pool  # sq/tt2 in chain

    w1 = nc.dram_tensor("w1", [128, c.KT1, 4 * c.NCH], F16,
                        kind="ExternalInput")
    w2 = nc.dram_tensor("w2", [128, c.KT2, c.DIM], F16, kind="ExternalInput")
    xt = nc.dram_tensor("xt", [128, c.KT1, c.NTOK], F16, kind="ExternalInput")
    pos = nc.dram_tensor("pos", [128, c.CT, c.T], F16, kind="ExternalInput")
    DD = c.DT // 2
    outp = nc.dram_tensor("outp", [128, DD, 2, c.NTOK], BF16,
                          kind="ExternalOutput")
    stats = nc.dram_tensor("stats", [2, c.NTOK], F32, kind="ExternalOutput")

    from contextlib import ExitStack
    with tile.TileContext(nc) as tc, ExitStack() as es:
        praw_bufs = 4 if "praw4" in pset else 6
        consts = es.enter_context(tc.tile_pool(name="consts", bufs=1))
        stream = es.enter_context(tc.tile_pool(name="stream", bufs=2))
        wide = es.enter_context(tc.tile_pool(name="wide", bufs=2))
        retp = es.enter_context(tc.tile_pool(name="retp", bufs=2))
        obp = es.enter_context(tc.tile_pool(name="obp", bufs=2))
        stb = es.enter_context(tc.tile_pool(name="stb", bufs=2))
        stc = es.enter_context(tc.tile_pool(name="stc", bufs=2))
        praw = es.enter_context(tc.tile_pool(name="praw", bufs=praw_bufs,
                                             space="PSUM"))
        pstat = es.enter_context(tc.tile_pool(name="pstat", bufs=1,
                                              space="PSUM"))
        pout = es.enter_context(tc.tile_pool(name="pout", bufs=1,
                                             space="PSUM"))

        w1_sb = consts.tile([128, c.KT1, 4 * c.NCH], F16, tag="w1_sb")
        nc.sync.dma_start(out=w1_sb[:], in_=w1[:])
        w2_sb = consts.tile([128, c.KT2, c.DIM], F16, tag="w2_sb")
        nc.sync.dma_start(out=w2_sb[:], in_=w2[:])
        w1_t = [w1_sb[:, k, :] for k in range(c.KT1)]
        w2_t = [w2_sb[:, k, :] for k in range(c.KT2)]

        ones_bf = consts.tile([128, 1], BF16, tag="ones")
        vec.memset(ones_bf[:], 1.0)
        half_pi = consts.tile([128, 1], F32, tag="half_pi")
        vec.memset(half_pi[:], PI / 2)
        car = {}
        for h in range(NH):
            for pl in ("re", "im"):
                car[(h, pl)] = consts.tile([128, SEGS], F16,
                                           tag=f"car_{h}_{pl}",
                                           name=f"car_{h}_{pl}")

        if hw_reps > 1:
            es.enter_context(tc.For_i(0, hw_reps))

        h2 = W // 2

        def emit_tail_pre(st):
            """Stats for chunk n-1: Pool squares + DVE fold tree."""
            rw = st["ret"]
            sqs = {}
            for h in range(NH):
                for pl in ("re", "im"):
                    s = wide.tile([128, W], BF16, tag=f"sq_{h}_{pl}", bufs=1,
                                  name=f"s_{h}_{pl}")
                    emit_sq(s[:], rw[(h, pl)][:])
                    sqs[(h, pl)] = s
            a = wide.tile([128, W], F16, tag="stA", bufs=1, name="a")
            vec.tensor_add(a[:], rw[(0, "re")][:], rw[(0, "im")][:])
            b = wide.tile([128, W], F16, tag="stB", bufs=1, name="b")
            vec.tensor_add(b[:], rw[(1, "re")][:], rw[(1, "im")][:])
            rs = wide.tile([128, W], F16, tag="stC", bufs=1, name="rs")
            vec.tensor_add(rs[:], a[:], b[:])
            f = wide.tile([128, h2], F16, tag="stD", bufs=1, name="f")
            vec.tensor_add(f[:], rs[:, 0:h2], rs[:, h2:W])
            stt = stb.tile([128, 2 * c.CN], BF16, tag="stt", name="stt")
            vec.tensor_add(stt[:, 0:c.CN], f[:, 0:c.CN], f[:, c.CN:h2])
            a2 = wide.tile([128, W], BF16, tag="stA", bufs=1, name="a2")
            vec.tensor_add(a2[:], sqs[(0, "re")][:], sqs[(0, "im")][:])
            b2 = wide.tile([128, W], BF16, tag="stB", bufs=1, name="b2")
            vec.tensor_add(b2[:], sqs[(1, "re")][:], sqs[(1, "im")][:])
            ss = wide.tile([128, W], BF16, tag="stC", bufs=1, name="ss")
            vec.tensor_add(ss[:], a2[:], b2[:])
            f2 = wide.tile([128, h2], BF16, tag="stD", bufs=1, name="f2")
            vec.tensor_add(f2[:], ss[:, 0:h2], ss[:, h2:W])
            vec.tensor_add(stt[:, c.CN:2 * c.CN], f2[:, 0:c.CN],
                           f2[:, c.CN:h2])
            st["stt"] = stt

        def emit_tail_mm(st):
            """Stats matmuls + proj_out for chunk n-1."""
            stt, tok_p, rw = st["stt"], st["tok"], st["ret"]
            ps = pstat.tile([1, 2 * c.CN], F32, tag="ps")
            nc.tensor.matmul(ps[:, 0:c.CN], ones_bf[:], stt[:, 0:c.CN],
                             start=True, stop=True)
            nc.tensor.matmul(ps[:, c.CN:2 * c.CN], ones_bf[:],
                             stt[:, c.CN:2 * c.CN], start=True, stop=True)
            sc = stc.tile([1, 2 * c.CN], F32, tag="sc", name="sc")
            sca.copy(sc[:], ps[:])
            nc.sync.dma_start(out=stats[0:1, tok_p], in_=sc[:, 0:c.CN])
            nc.sync.dma_start(out=stats[1:2, tok_p], in_=sc[:, c.CN:2 * c.CN])
            for dd in range(DD):
                po = pout.tile([128, 2, c.CN], F32, tag="pout")
                for di in range(2):
                    d = dd * 2 + di
                    for k in range(kt2_lim):
                        if k < c.CT:
                            h, s, pl = k // SEGS, k % SEGS, "re"
                        else:
                            h, s, pl = ((k - c.CT) // SEGS,
                                        (k - c.CT) % SEGS, "im")
                        rt = rw[(h, pl)][:, s * c.CN:(s + 1) * c.CN]
                        nc.tensor.matmul(po[:, di, :],
                                         w2_t[k][:, d * 128:(d + 1) * 128],
                                         rt, start=(k == 0),
                                         stop=(k == kt2_lim - 1))
                ob = obp.tile([128, 2, c.CN], BF16, tag="ob", name="ob")
                if "obact" in pset:
                    sca.copy(ob[:], po[:])
                else:
                    vec.tensor_copy(ob[:], po[:])
                nc.sync.dma_start(out=outp[:, dd, :, tok_p], in_=ob[:])

        def drain_group(p, g, dst, wcols):
            if g == "ph":
                sca.activation(dst[:, wcols], p[:], AF.Tanh)
            elif g == "mg":
                sca.activation(dst[:, wcols], p[:], AF.Tanh, scale=0.5)
            else:
                sca.copy(dst[:, wcols], p[:])

        prev = None
        for n in [nn_ for _ in range(reps) for nn_ in range(c.NCHUNK)]:
            t0 = (n % c.CPB) * c.CN
            first_in_batch = t0 == 0
            tok = slice(n * c.CN, (n + 1) * c.CN)

            if prev is not None:
                emit_tail_pre(prev)

            posb = stream.tile([128, c.CT, c.CN], F16, tag="posb")
            nc.sync.dma_start(out=posb[:], in_=pos[:, :, t0:t0 + c.CN])
            xcb = stream.tile([128, c.KT1, c.CN], F16, tag="xcb")
            nc.sync.dma_start(out=xcb[:], in_=xt[:, :, tok])

            gidx = {"ph": 0, "mg": 1, "qr": 2, "qi": 3}
            ret_w = {}
            for h in range(NH):
                i0 = h * SEGS
                th_ph = wide.tile([128, W], F16, tag="th_ph", name="th_ph")
                th_mg = wide.tile([128, W], F16, tag="th_mg", name="th_mg")
                qre = wide.tile([128, W], F16, tag="qre", name="qre")
                qim = wide.tile([128, W], F16, tag="qim", name="qim")
                dest = {"ph": th_ph, "mg": th_mg, "qr": qre, "qi": qim}

                def gemm_pass(groups):
                    for j in (0, 2):
                        for g in groups:
                            p = praw.tile([128, 2 * c.CN], F32, tag="praw")
                            for half in range(2):
                                m = gidx[g] * c.CT + i0 + j + half
                                cols = slice(half * c.CN, (half + 1) * c.CN)
                                for k in range(kt1_lim):
                                    nc.tensor.matmul(
                                        p[:, cols],
                                        w1_t[k][:, m * 128:(m + 1) * 128],
                                        xcb[:, k, :],
                                        start=(k == 0),
                                        stop=(k == kt1_lim - 1))
                            drain_group(p, g, dest[g],
                                        slice(j * c.CN, (j + 2) * c.CN))

                # phase/magnitude GEMMs first: the chain head depends on them
                gemm_pass(("ph", "mg"))
                theta = wide.tile([128, W], F16, tag="theta", name="theta")
                vec.scalar_tensor_tensor(theta[:], th_ph[:], PI / 2,
                                         posb[:, i0:i0 + SEGS, :],
                                         ALU.mult, ALU.add)
                mg2 = wide.tile([128, W], F16, tag="mg2", name="mg2")
                vec.tensor_scalar(mg2[:], th_mg[:], 1.0, None, ALU.add)
                sh = wide.tile([128, W], F16, tag="sh", name="sh")
                sca.activation(sh[:], theta[:], AF.Sin)
                ab = wide.tile([128, W], F16, tag="ab", bufs=1, name="ab")
                sca.activation(ab[:], theta[:], AF.Abs)
                ch = wide.tile([128, W], F16, tag="ch", bufs=1, name="ch")
                sca.activation(ch[:], ab[:], AF.Sin, bias=half_pi[:],
                               scale=-1.0)
                sq = wide.tile([128, W], F16, tag="sq", bufs=1, name="sq")
                ch_eng.tensor_mul(sq[:], sh[:], sh[:])
                tt2 = wide.tile([128, W], F16, tag="tt2", bufs=1, name="tt2")
                ch_eng.tensor_mul(tt2[:], sh[:], ch[:])

                # query GEMMs while the chain runs on ACT/DVE/Pool
                gemm_pass(("qr", "qi"))

                kreN = wide.tile([128, W], F16, tag="kreN", bufs=1,
                                 name="kreN")
                vec.scalar_tensor_tensor(kreN[:], sq[:], 0.5, mg2[:],
                                         ALU.subtract, ALU.mult)
                kim = wide.tile([128, W], F16, tag="kim", bufs=1, name="kim")
                vec.tensor_mul(kim[:], tt2[:], mg2[:])

                mre = wide.tile([128, W], F16, tag="mre", name="mre")
                mim = wide.tile([128, W], F16, tag="mim", name="mim")
                for s in range(SEGS):
                    seg = slice(s * c.CN, (s + 1) * c.CN)
                    init_re = (0.0 if first_in_batch
                               else car[(h, "re")][:, s:s + 1])
                    vec.tensor_tensor_scan(mre[:, seg], kreN[:, seg],
                                           kreN[:, seg], init_re,
                                           ALU.add, ALU.bypass)
                    init_im = (0.0 if first_in_batch
                               else car[(h, "im")][:, s:s + 1])
                    vec.tensor_tensor_scan(mim[:, seg], kim[:, seg],
                                           kim[:, seg], init_im,
                                           ALU.add, ALU.bypass)
                if (n % c.CPB) != c.CPB - 1:
                    cre = mre.rearrange("p (s t) -> p s t", s=SEGS)[:, :,
                                                                    c.CN - 1]
                    vec.tensor_copy(car[(h, "re")][:], cre)
                    cim = mim.rearrange("p (s t) -> p s t", s=SEGS)[:, :,
                                                                    c.CN - 1]
                    vec.tensor_copy(car[(h, "im")][:], cim)

                # retrieval (mreN = -Sre):
                #   rre = mim*qim - mreN*qre ; rim = mim*qre + mreN*qim
                r1 = wide.tile([128, W], F16, tag="r1", bufs=1, name="r1")
                vec.tensor_mul(r1[:], mre[:], qre[:])
                r2 = wide.tile([128, W], F16, tag="r2", bufs=1, name="r2")
                vec.tensor_mul(r2[:], mim[:], qim[:])
                rre = retp.tile([128, W], F16, tag=f"ret_re_{h}",
                                name=f"ret_re_{h}")
                vec.tensor_sub(rre[:], r2[:], r1[:])
                r3 = wide.tile([128, W], F16, tag="r1", bufs=1, name="r3")
                vec.tensor_mul(r3[:], mim[:], qre[:])
                r4 = wide.tile([128, W], F16, tag="r2", bufs=1, name="r4")
                vec.tensor_mul(r4[:], mre[:], qim[:])
                rim = retp.tile([128, W], F16, tag=f"ret_im_{h}",
                                name=f"ret_im_{h}")
                vec.tensor_add(rim[:], r3[:], r4[:])
                ret_w[(h, "re")] = rre
                ret_w[(h, "im")] = rim

            if prev is not None:
                emit_tail_mm(prev)
            prev = {"ret": ret_w, "tok": tok}

        emit_tail_pre(prev)
        emit_tail_mm(prev)

    return nc


# --------------------------------------------------------------------------
# Host-side sharding / unsharding
# --------------------------------------------------------------------------
def shard_inputs(cfg, x, W_in, W_out, ln_gamma, ln_beta, pos_phases):
    c = cfg
    HD = N_CORES * c.NCH
    xT = np.ascontiguousarray(x.reshape(c.NTOK, c.DIM).T)          # [DIM, NTOK]
    # [p, k, tok] partition-major so one DMA covers all k-tiles of a chunk
    xt_h = np.ascontiguousarray(
        xT.reshape(c.KT1, 128, c.NTOK).transpose(1, 0, 2)
    ).astype(ml_dtypes.bfloat16)

    pos64 = pos_phases.astype(np.float64)
    cos_p = (0.5 * np.cos(pos64)).astype(np.float16)               # [T, HD]
    sin_p = (0.5 * np.sin(pos64)).astype(np.float16)

    Wg = (W_out * ln_gamma[None, :]).astype(np.float32)            # [DIM, 2HD]

    in_maps = []
    for cid in range(N_CORES):
        h0 = cid * c.NCH
        hs = slice(h0, h0 + c.NCH)
        w_ph = W_in[0 * HD + h0:0 * HD + h0 + c.NCH]               # [NCH, DIM]
        w_mg = W_in[1 * HD + h0:1 * HD + h0 + c.NCH]
        w_qr = W_in[2 * HD + h0:2 * HD + h0 + c.NCH]
        w_qi = W_in[3 * HD + h0:3 * HD + h0 + c.NCH]
        w_all = np.concatenate([w_ph, w_mg, w_qr, w_qi], axis=0)   # [4NCH, DIM]
        w1_h = np.ascontiguousarray(
            w_all.T.reshape(c.KT1, 128, 4 * c.NCH).transpose(1, 0, 2)
        ).astype(ml_dtypes.bfloat16)

        wg_re = Wg[:, 2 * h0:2 * (h0 + c.NCH):2]                   # [DIM, NCH]
        wg_im = Wg[:, 2 * h0 + 1:2 * (h0 + c.NCH):2]
        w2T = np.concatenate([wg_re.T, wg_im.T], axis=0)           # [2NCH, DIM]
        w2_h = np.ascontiguousarray(
            w2T.reshape(c.KT2, 128, c.DIM).transpose(1, 0, 2)
        ).astype(ml_dtypes.bfloat16)

        cp_h = np.ascontiguousarray(
            cos_p[:, hs].T.reshape(c.CT, 128, c.T).transpose(1, 0, 2))
        sp_h = np.ascontiguousarray(
            sin_p[:, hs].T.reshape(c.CT, 128, c.T).transpose(1, 0, 2))

        in_maps.append({
            "w1": w1_h, "w2": w2_h, "xt": xt_h,
            "cp": cp_h, "sp": sp_h,
        })
    return in_maps


def combine_outputs(cfg, results, W_out, ln_gamma, ln_beta, x_dtype):
    c = cfg
    NF = 2 * N_CORES * c.NCH
    P = np.zeros((c.DIM, c.NTOK), np.float64)
    S1 = np.zeros(c.NTOK, np.float64)
    S2 = np.zeros(c.NTOK, np.float64)
    for r in results:
        # outp is [128, DT, NTOK] partition-major of out^T -> [DIM, NTOK]
        op = r["outp"].transpose(1, 0, 2).reshape(c.DIM, c.NTOK)
        P += op.astype(np.float64)
        S1 += r["stats"][0].astype(np.float64)
        S2 += r["stats"][1].astype(np.float64)
    mu = S1 / NF
    var = S2 / NF - mu * mu
    istd = 1.0 / np.sqrt(var + LN_EPS)
    wg_sum = (W_out.astype(np.float64) @ ln_gamma.astype(np.float64))  # [DIM]
    b_out = (W_out.astype(np.float64) @ ln_beta.astype(np.float64))    # [DIM]
    out = istd[:, None] * (P.T - mu[:, None] * wg_sum[None, :]) + b_out[None, :]
    return out.reshape(c.B, c.T, c.DIM).astype(x_dtype)


def shard_inputs_v2(cfg, x, W_in, W_out, ln_gamma, ln_beta, pos_phases):
    c = cfg
    HD = N_CORES * c.NCH
    xT = np.ascontiguousarray(x.reshape(c.NTOK, c.DIM).T)          # [DIM, NTOK]
    xt_h = np.ascontiguousarray(
        xT.reshape(c.KT1, 128, c.NTOK).transpose(1, 0, 2)
    ).astype(np.float16)
    x8_h = np.ascontiguousarray(
        (xT * 16.0).reshape(c.KT1, 128, c.NTOK).transpose(1, 0, 2)
    ).astype(ml_dtypes.float8_e4m3)

    # pos/2, wrapped to [-pi/2, pi/2): theta_half = pi/2*tanh(ph) + pos/2
    pos64 = pos_phases.astype(np.float64)
    pos_half = (0.5 * (np.mod(pos64 + np.pi, 2 * np.pi) - np.pi)
                ).astype(np.float16)                               # [T, HD]

    Wg = (W_out * ln_gamma[None, :]).astype(np.float32)            # [DIM, 2HD]

    in_maps = []
    for cid in range(N_CORES):
        h0 = cid * c.NCH
        hs = slice(h0, h0 + c.NCH)
        w_ph = W_in[0 * HD + h0:0 * HD + h0 + c.NCH]
        w_mg = W_in[1 * HD + h0:1 * HD + h0 + c.NCH]
        w_qr = W_in[2 * HD + h0:2 * HD + h0 + c.NCH]
        w_qi = W_in[3 * HD + h0:3 * HD + h0 + c.NCH]
        w_all = np.concatenate([w_ph, w_mg, w_qr, w_qi], axis=0)   # [4NCH, DIM]
        w1_h = np.ascontiguousarray(
            w_all.T.reshape(c.KT1, 128, 4 * c.NCH).transpose(1, 0, 2)
        ).astype(np.float16)

        wg_re = Wg[:, 2 * h0:2 * (h0 + c.NCH):2]                   # [DIM, NCH]
        wg_im = Wg[:, 2 * h0 + 1:2 * (h0 + c.NCH):2]
        w2T = np.concatenate([wg_re.T, wg_im.T], axis=0)           # [2NCH, DIM]
        w2_h = np.ascontiguousarray(
            w2T.reshape(c.KT2, 128, c.DIM).transpose(1, 0, 2)
        ).astype(np.float16)

        pos_h = np.ascontiguousarray(
            pos_half[:, hs].T.reshape(c.CT, 128, c.T).transpose(1, 0, 2))

        # fp8(e4m3) copies for the magnitude channel (scales folded into
        # the on-chip tanh input scale: 0.5/(16*64)).
        w13_full = np.concatenate([w_ph, w_qr, w_qi], axis=0)
        w13_h = np.ascontiguousarray(
            w13_full.T.reshape(c.KT1, 128, 3 * c.NCH).transpose(1, 0, 2)
        ).astype(np.float16)
        wm8 = (w_mg.astype(np.float32) * 64.0).astype(
            ml_dtypes.float8_e4m3).astype(ml_dtypes.float8_e4m3)
        # layout [128, KT1//2, 2, NCH]: plane i of pair kk is k-tile 2kk+i
        wm8_h = np.ascontiguousarray(
            wm8.T.reshape(c.KT1 // 2, 2, 128, c.NCH).transpose(2, 0, 1, 3))
        in_maps.append({"w1": w1_h, "w2": w2_h, "xt": xt_h, "pos": pos_h,
                        "w13": w13_h, "w1m8": wm8_h, "x8": x8_h})
    return in_maps


def combine_outputs_v2(cfg, results, W_out, ln_gamma, ln_beta, x_dtype):
    c = cfg
    NF = 2 * N_CORES * c.NCH
    P = np.zeros((c.DIM, c.NTOK), np.float64)
    S1 = np.zeros(c.NTOK, np.float64)
    S2 = np.zeros(c.NTOK, np.float64)
    for r in results:
        op = r["outp"].transpose(1, 0, 2).reshape(c.DIM, c.NTOK)
        P += op.astype(np.float64)
        S1 += r["stats"][0].astype(np.float64)
        S2 += r["stats"][1].astype(np.float64)
    mu = S1 / NF
    var = S2 / NF - mu * mu
    istd = 1.0 / np.sqrt(var + LN_EPS)
    wg_sum = (W_out.astype(np.float64) @ ln_gamma.astype(np.float64))
    b_out = (W_out.astype(np.float64) @ ln_beta.astype(np.float64))
    out = istd[:, None] * (P.T - mu[:, None] * wg_sum[None, :]) + b_out[None, :]
    return out.reshape(c.B, c.T, c.DIM).astype(x_dtype)


def shard_inputs_v3(cfg, x, W_in, W_out, ln_gamma, ln_beta, pos_phases):
    c = cfg
    HD = N_CORES * c.NCH
    xT = np.ascontiguousarray(x.reshape(c.NTOK, c.DIM).T)          # [DIM, NTOK]
    xt_h = np.ascontiguousarray(
        xT.reshape(c.KT1, 128, c.NTOK).transpose(1, 0, 2)
    ).astype(np.float16)

    # pos/2, wrapped to [-pi/2, pi/2): theta_half = pi/2*tanh(ph) + pos/2
    pos64 = pos_phases.astype(np.float64)
    pos_half = (0.5 * (np.mod(pos64 + np.pi, 2 * np.pi) - np.pi)
                ).astype(np.float16)                               # [T, HD]

    Wg = (W_out * ln_gamma[None, :]).astype(np.float32)            # [DIM, 2HD]

    in_maps = []
    for cid in range(N_CORES):
        h0 = cid * c.NCH
        hs = slice(h0, h0 + c.NCH)
        w_ph = W_in[0 * HD + h0:0 * HD + h0 + c.NCH]
        w_mg = W_in[1 * HD + h0:1 * HD + h0 + c.NCH]
        w_qr = W_in[2 * HD + h0:2 * HD + h0 + c.NCH]
        w_qi = W_in[3 * HD + h0:3 * HD + h0 + c.NCH]
        w_all = np.concatenate([w_ph, w_mg, w_qr, w_qi], axis=0)   # [4NCH, DIM]
        w1_h = np.ascontiguousarray(
            w_all.T.reshape(c.KT1, 128, 4 * c.NCH).transpose(1, 0, 2)
        ).astype(np.float16)

        wg_re = Wg[:, 2 * h0:2 * (h0 + c.NCH):2]                   # [DIM, NCH]
        wg_im = Wg[:, 2 * h0 + 1:2 * (h0 + c.NCH):2]
        w2T = np.concatenate([wg_re.T, wg_im.T], axis=0)           # [2NCH, DIM]
        w2_h = np.ascontiguousarray(
            w2T.reshape(c.KT2, 128, c.DIM).transpose(1, 0, 2)
        ).astype(np.float16)

        pos_h = np.ascontiguousarray(
            pos_half[:, hs].T.reshape(c.CT, 128, c.T).transpose(1, 0, 2))
        in_maps.append({"w1": w1_h, "w2": w2_h, "xt": xt_h, "pos": pos_h})
    return in_maps


def combine_outputs_v3(cfg, results, W_out, ln_gamma, ln_beta, x_dtype):
    c = cfg
    NF = 2 * N_CORES * c.NCH
    P = np.zeros((c.DIM, c.NTOK), np.float64)
    S1 = np.zeros(c.NTOK, np.float64)
    S2 = np.zeros(c.NTOK, np.float64)
    for r in results:
        # outp [128, DD, 2, NTOK]: out[(dd*2+di)*128 + p, t]
        op = r["outp"].transpose(1, 2, 0, 3).reshape(c.DIM, c.NTOK)
        P += op.astype(np.float64)
        S1 += r["stats"][0].astype(np.float64)
        S2 += r["stats"][1].astype(np.float64)
    mu = S1 / NF
    var = S2 / NF - mu * mu
    istd = 1.0 / np.sqrt(var + LN_EPS)
    wg_sum = (W_out.astype(np.float64) @ ln_gamma.astype(np.float64))
    b_out = (W_out.astype(np.float64) @ ln_beta.astype(np.float64))
    out = istd[:, None] * (P.T - mu[:, None] * wg_sum[None, :]) + b_out[None, :]
    return out.reshape(c.B, c.T, c.DIM).astype(x_dtype)


import os

# Production configuration: v2 with fp8(e4m3) DoubleRow matmuls for the
# sigmoid magnitude channel (validated rel err 0.011 < 2e-2 on hardware).
# Extensive re-benchmarking (robust interleaved-window protocol) confirmed
# this is PE-bound at the measured per-matmul rate (~135-145 ns per
# LDW+MM pair at N=256 f16; fp8-DoubleRow ~2x cheaper per k-plane); the
# explored alternatives (all-f16 v3 with GPSIMD offload, CN=512, pin512,
# explicit-ldweights reuse, interleaved chain emission, ACT->DVE op moves)
# all measured equal or worse under the same protocol.
DEFAULT_PROBE = "fp8mag"


def _active_build(cfg, reps=1, hw_reps=1, probe=None):
    env = os.environ.get("KERNEL_PROBE")
    base = DEFAULT_PROBE if env is None else env
    merged = ",".join(x for x in [base, probe or ""] if x) or None
    return build_program_v2(cfg, reps=reps, hw_reps=hw_reps, probe=merged)


# Active implementation selector (test.py/bench use these too)
BUILD = _active_build
SHARD = shard_inputs_v2
COMBINE = combine_outputs_v2
CN_ACTIVE = 256

_cached = {}


def kernel(x, W_in, W_out, ln_gamma, ln_beta, pos_phases):
    cfg = Cfg(B=x.shape[0], T=x.shape[1], DIM=x.shape[2],
              NCH=pos_phases.shape[1] // N_CORES, CN=CN_ACTIVE)
    key = (cfg.B, cfg.T, cfg.DIM, cfg.NCH)
    if key not in _cached:
        nc = BUILD(cfg)
        split_multiwait(nc)  # walrus workaround; CoreSim path must skip this
        _cached[key] = nc
    nc = _cached[key]
    in_maps = SHARD(cfg, np.asarray(x), np.asarray(W_in),
                    np.asarray(W_out), np.asarray(ln_gamma),
                    np.asarray(ln_beta), np.asarray(pos_phases))
    # the native run path rejects in_map keys the program doesn't declare
    declared = {a.memorylocations[0].name
                for a in nc.m.functions[0].allocations
                if isinstance(a, mybir.MemoryLocationSet)
                and a.kind == "ExternalInput"}
    in_maps = [{k: v for k, v in m.items() if k in declared} for m in in_maps]
    res = run_bass_kernel_spmd(nc, in_maps, list(range(N_CORES)))
    return COMBINE(cfg, res.results, np.asarray(W_out),
                   np.asarray(ln_gamma), np.asarray(ln_beta),
                   np.asarray(x).dtype)



# revision 26
# speedup vs baseline: 7.7758x; 4.5488x over previous
"""Trainium2 Bass kernel for nn_LongAttention (holographic long-attention block).

Computation (see reference):
  raw = x @ W_in.T -> split [c_phase | c_mag | q_re | q_im] per hd channel
  key = sigmoid(c_mag) * exp(i*(pi*tanh(c_phase) + pos_phase))
  state = cumsum_t(key);  ret = state * conj(q)
  ret_real = interleave(Re, Im) -> LayerNorm(2*hd) -> @ W_out.T

Distribution: hd (8192) split across 8 NeuronCores (1024 ch each); every core
handles both batches and all tokens; cores are fully independent. gamma is
folded into W_out on the host and the LayerNorm is algebraically deferred:
each core returns P = ret @ (W_out*gamma).T partials plus per-token
S1 = sum_f ret, S2 = sum_f ret^2; the host combines
out = istd * (sum_c P_c - mu * (W_out @ gamma)) + W_out @ beta.

Active implementation (build_program_v2, CN=256-token chunks):
 - f16 datapath end to end (matmul inputs, elementwise, scan output) --
   same speed as bf16 everywhere but ~8x finer mantissa, plus 2x DVE
   perf-modes on the 16-bit elementwise ops.
 - The magnitude-channel GEMM runs in fp8(e4m3) with perf_mode=DoubleRow
   (2 k-planes per instruction); the quantization scales (x*16, W*64) are
   folded into the on-chip tanh input scale. Sigmoid's 1/4 slope damps the
   fp8 noise; measured end-to-end rel err 0.011 < 2e-2.
 - sin/cos via the half-angle identity: th = (pi/2)*tanh(ph) + pos/2 with
   pos pre-wrapped to [-pi, pi) on the host, so |th| <= pi stays inside
   the ACT Sin LUT range; cos(2th) = 1-2*sin^2(th) gives the real part
   without a second LUT pass over an out-of-range argument.
 - The cumsum runs channel-major on the DVE as a prefix scan along the free
   (time) axis (fp32 internal state), carried across token chunks.
 - Per-token LN stats are folded on DVE/ACT (tree adds + squares) into one
   [128, 2*CN] tile and reduced across partitions by a single pair of
   ones-matmuls -- instead of 32 PE matmuls per chunk.
 - stats + proj_out for chunk n-1 are emitted during chunk n (software
   pipelining) so the in-order PE queue never waits on the chunk's serial
   ACT<->DVE elementwise chain; all hot pools are double-buffered.
"""

import sys
import numpy as np
import ml_dtypes

for _p in ("/opt/trn_rl_repo", "/root/.axon_site/_ro/trn_rl_repo"):
    if _p not in sys.path:
        sys.path.append(_p)

import bass_rust
import concourse.bass as bass
import concourse.tile as tile
import concourse.mybir as mybir
from concourse.bass_utils import run_bass_kernel_spmd

F32 = mybir.dt.float32
F8 = mybir.dt.float8e4
F16 = mybir.dt.float16
BF16 = mybir.dt.bfloat16
AF = mybir.ActivationFunctionType
ALU = mybir.AluOpType
PI = float(np.pi)

N_CORES = 8
LN_EPS = 1e-5


# --------------------------------------------------------------------------
# Workaround: this container's walrus rejects >1 semaphore wait per
# instruction ("Too many sync wait commands"). Split the extras onto
# same-engine NoOps inserted just before (engine FIFO keeps semantics).
# --------------------------------------------------------------------------
_nop_counter = [0]


def split_multiwait(nc):
    n_split = 0
    for f in nc.m.functions:
        for bb in f.blocks:
            il = bb.instructions
            i = 0
            while i < len(il):
                ins = il[i]
                si = ins.sync_info
                waits = list(si.on_wait) if si is not None and si.on_wait else []
                if len(waits) > 1:
                    for w in waits[:-1]:
                        _nop_counter[0] += 1
                        nop = bass_rust.InstNoOp(
                            name=f"mw_nop_{_nop_counter[0]}",
                            engine=ins.engine,
                            ins=[],
                            outs=[],
                        )
                        nop.sync_info = mybir.SyncInfo(on_wait=[w], on_update=[])
                        il.insert(i, nop)
                        i += 1
                    si.on_wait = [waits[-1]]
                    n_split += 1
                i += 1
    return n_split


# --------------------------------------------------------------------------
# Device program (SPMD: identical on all cores; per-core data differs)
# --------------------------------------------------------------------------
class Cfg:
    def __init__(self, B=2, T=2048, DIM=1024, NCH=1024, CN=256):
        self.B, self.T, self.DIM, self.NCH, self.CN = B, T, DIM, NCH, CN
        self.NTOK = B * T
        self.CT = NCH // 128          # channel tiles per core
        self.KT1 = DIM // 128         # contraction tiles for proj_in
        self.KT2 = 2 * self.CT        # contraction tiles for proj_out (re+im)
        self.DT = DIM // 128          # output dim tiles
        self.NCHUNK = self.NTOK // CN
        self.CPB = T // CN            # chunks per batch


def build_program(cfg: Cfg, reps: int = 1, hw_reps: int = 1,
                  probe: str | None = None):
    c = cfg
    assert c.CT % 4 == 0 or c.CT == 2
    SEGS = 4 if c.CT % 4 == 0 else 2   # channel tiles per wide tile
    NH = c.CT // SEGS                  # wide halves per chunk
    W = SEGS * c.CN                    # wide tile width
    nc = bass.Bass()

    class _Dup:
        def __init__(self, eng, on):
            self._eng, self._on = eng, on

        def __getattr__(self, n):
            f = getattr(self._eng, n)
            if not self._on:
                return f

            def g(*a, **k):
                r = f(*a, **k)
                f(*a, **k)
                return r
            return g

    pset = set(probe.split(",")) if probe else set()
    vec = _Dup(nc.vector, "dve2" in pset)
    sca = _Dup(nc.scalar, "act2" in pset)
    kt1_lim = c.KT1 // 2 if "pein_half" in pset else c.KT1
    kt2_lim = c.KT2 // 2 if "peout_half" in pset else c.KT2
    stats_on = "stats_off" not in pset

    w1 = nc.dram_tensor("w1", [128, c.KT1, 4 * c.NCH], BF16, kind="ExternalInput")
    w2 = nc.dram_tensor("w2", [128, c.KT2, c.DIM], BF16, kind="ExternalInput")
    xt = nc.dram_tensor("xt", [128, c.KT1, c.NTOK], BF16, kind="ExternalInput")
    cp = nc.dram_tensor("cp", [128, c.CT, c.T], F16, kind="ExternalInput")
    sp = nc.dram_tensor("sp", [128, c.CT, c.T], F16, kind="ExternalInput")
    outp = nc.dram_tensor("outp", [128, c.DT, c.NTOK], F32, kind="ExternalOutput")
    stats = nc.dram_tensor("stats", [2, c.NTOK], F32, kind="ExternalOutput")

    from contextlib import ExitStack
    with tile.TileContext(nc) as tc, ExitStack() as es:
        consts = es.enter_context(tc.tile_pool(name="consts", bufs=1))
        stream = es.enter_context(tc.tile_pool(name="stream", bufs=2))
        wide = es.enter_context(tc.tile_pool(name="wide", bufs=1))
        retp = es.enter_context(tc.tile_pool(name="retp", bufs=2))
        obp = es.enter_context(tc.tile_pool(name="obp", bufs=1))
        stc = es.enter_context(tc.tile_pool(name="stc", bufs=2))
        praw = es.enter_context(tc.tile_pool(name="praw", bufs=4, space="PSUM"))
        pstat = es.enter_context(tc.tile_pool(name="pstat", bufs=1, space="PSUM"))
        pstat2 = es.enter_context(tc.tile_pool(name="pstat2", bufs=1, space="PSUM"))
        pout = es.enter_context(tc.tile_pool(name="pout", bufs=2, space="PSUM"))

        w1_sb = consts.tile([128, c.KT1, 4 * c.NCH], BF16, tag="w1_sb")
        nc.sync.dma_start(out=w1_sb[:], in_=w1[:])
        w2_sb = consts.tile([128, c.KT2, c.DIM], BF16, tag="w2_sb")
        nc.sync.dma_start(out=w2_sb[:], in_=w2[:])
        w1_t = [w1_sb[:, k, :] for k in range(c.KT1)]
        w2_t = [w2_sb[:, k, :] for k in range(c.KT2)]

        ones_bf = consts.tile([128, 1], BF16, tag="ones")
        vec.memset(ones_bf[:], 1.0)
        half_pi = consts.tile([128, 1], F32, tag="half_pi")
        vec.memset(half_pi[:], PI / 2)
        car = {}
        for h in range(NH):
            for pl in ("re", "im"):
                car[(h, pl)] = consts.tile([128, SEGS], F32, tag=f"car_{h}_{pl}",
                                           name=f"car_{h}_{pl}")

        if hw_reps > 1:
            es.enter_context(tc.For_i(0, hw_reps))

        for n in [nn_ for _ in range(reps) for nn_ in range(c.NCHUNK)]:
            t0 = (n % c.CPB) * c.CN
            first_in_batch = t0 == 0
            tok = slice(n * c.CN, (n + 1) * c.CN)

            xcb = stream.tile([128, c.KT1, c.CN], BF16, tag="xcb")
            nc.sync.dma_start(out=xcb[:], in_=xt[:, :, tok])
            xc = [xcb[:, k, :] for k in range(c.KT1)]
            cpb = stream.tile([128, c.CT, c.CN], F16, tag="cpb")
            nc.sync.dma_start(out=cpb[:], in_=cp[:, :, t0:t0 + c.CN])
            spb = stream.tile([128, c.CT, c.CN], F16, tag="spb")
            nc.sync.dma_start(out=spb[:], in_=sp[:, :, t0:t0 + c.CN])

            ret_w = {}
            for h in range(NH):
                i0 = h * SEGS
                # ---- proj_in: 4 groups x SEGS channel tiles -> psum pairs ----
                # psum tile [128, 2*CN] holds channel tiles (j, j+1) of a group
                th_ph = wide.tile([128, W], F32, tag="th_ph", name="th_ph")
                th_mg = wide.tile([128, W], F32, tag="th_mg", name="th_mg")
                qre = wide.tile([128, W], F32, tag="qre", name="qre")
                qim = wide.tile([128, W], F32, tag="qim", name="qim")
                dest = {"ph": th_ph, "mg": th_mg, "qr": qre, "qi": qim}
                for j in range(0, SEGS, 2):
                    for gi, g in enumerate(("ph", "mg", "qr", "qi")):
                        p = praw.tile([128, 2 * c.CN], F32, tag="praw")
                        for half in range(2):
                            m = gi * c.CT + i0 + j + half
                            cols = slice(half * c.CN, (half + 1) * c.CN)
                            for k in range(kt1_lim):
                                nc.tensor.matmul(
                                    p[:, cols],
                                    w1_t[k][:, m * 128:(m + 1) * 128], xc[k],
                                    start=(k == 0), stop=(k == kt1_lim - 1))
                        wcols = slice(j * c.CN, (j + 2) * c.CN)
                        if g == "ph" or g == "mg":
                            sc = 1.0 if g == "ph" else 0.5
                            sca.activation(dest[g][:, wcols], p[:],
                                                 AF.Tanh, scale=sc)
                        elif "qdve" in pset:
                            vec.tensor_copy(dest[g][:, wcols], p[:])
                        else:
                            sca.copy(dest[g][:, wcols], p[:])

                # ---- content phasor (wide) ----
                sinp = wide.tile([128, W], F32, tag="sinp", name="sinp")
                sca.activation(sinp[:], th_ph[:], AF.Sin, scale=PI)
                tabs = wide.tile([128, W], F32, tag="tabs", name="tabs")
                sca.activation(tabs[:], th_ph[:], AF.Abs)
                cosp = wide.tile([128, W], F32, tag="th_ph", name="cosp")
                sca.activation(cosp[:], tabs[:], AF.Sin,
                                     bias=half_pi[:], scale=-PI)
                # 2*sigma = th_mg + 1 ; the 0.5 is folded into cp/sp on host
                ssin = wide.tile([128, W], F32, tag="tabs", name="ssin")
                vec.scalar_tensor_tensor(ssin[:], th_mg[:], 1.0, sinp[:],
                                               ALU.add, ALU.mult)
                scos = wide.tile([128, W], F32, tag="sinp", name="scos")
                vec.scalar_tensor_tensor(scos[:], th_mg[:], 1.0, cosp[:],
                                               ALU.add, ALU.mult)

                # ---- key = content * pos phasor (wide, cp/sp pre-halved) ----
                cps = cpb[:, i0:i0 + SEGS, :]
                sps = spb[:, i0:i0 + SEGS, :]
                ta = wide.tile([128, W], F32, tag="tmp1", name="ta")
                vec.tensor_mul(ta[:], scos[:], cps)
                tb = wide.tile([128, W], F32, tag="tmp2", name="tb")
                vec.tensor_mul(tb[:], ssin[:], sps)
                kre = wide.tile([128, W], F32, tag="kre", name="kre")
                vec.tensor_sub(kre[:], ta[:], tb[:])
                tc_ = wide.tile([128, W], F32, tag="tmp1", name="tc_")
                vec.tensor_mul(tc_[:], ssin[:], cps)
                td = wide.tile([128, W], F32, tag="tmp2", name="td")
                vec.tensor_mul(td[:], scos[:], sps)
                kim = wide.tile([128, W], F32, tag="kim", name="kim")
                vec.tensor_add(kim[:], tc_[:], td[:])

                # ---- prefix scan per channel tile segment ----
                mre = wide.tile([128, W], F32, tag="mre", name="mre")
                mim = wide.tile([128, W], F32, tag="mim", name="mim")
                for s in range(SEGS):
                    seg = slice(s * c.CN, (s + 1) * c.CN)
                    init_re = 0.0 if first_in_batch else car[(h, "re")][:, s:s + 1]
                    vec.tensor_tensor_scan(mre[:, seg], kre[:, seg],
                                                 kre[:, seg], init_re,
                                                 ALU.add, ALU.bypass)
                    init_im = 0.0 if first_in_batch else car[(h, "im")][:, s:s + 1]
                    vec.tensor_tensor_scan(mim[:, seg], kim[:, seg],
                                                 kim[:, seg], init_im,
                                                 ALU.add, ALU.bypass)
                if (n % c.CPB) != c.CPB - 1:
                    cre = mre.rearrange("p (s t) -> p s t", s=SEGS)[:, :, c.CN - 1]
                    vec.tensor_copy(car[(h, "re")][:], cre)
                    cim = mim.rearrange("p (s t) -> p s t", s=SEGS)[:, :, c.CN - 1]
                    vec.tensor_copy(car[(h, "im")][:], cim)

                # ---- retrieval = state * conj(q) (wide) ----
                r1 = wide.tile([128, W], F32, tag="tmp1", name="r1")
                vec.tensor_mul(r1[:], mre[:], qre[:])
                r2 = wide.tile([128, W], F32, tag="tmp2", name="r2")
                vec.tensor_mul(r2[:], mim[:], qim[:])
                rre = retp.tile([128, W], BF16, tag=f"ret_re_{h}",
                                name=f"ret_re_{h}")
                vec.tensor_add(rre[:], r1[:], r2[:])
                r3 = wide.tile([128, W], F32, tag="tmp1", name="r3")
                vec.tensor_mul(r3[:], mim[:], qre[:])
                r4 = wide.tile([128, W], F32, tag="tmp2", name="r4")
                vec.tensor_mul(r4[:], mre[:], qim[:])
                rim = retp.tile([128, W], BF16, tag=f"ret_im_{h}",
                                name=f"ret_im_{h}")
                vec.tensor_sub(rim[:], r3[:], r4[:])
                ret_w[(h, "re")] = rre
                ret_w[(h, "im")] = rim

            # ---- per-token stats via ones-matmuls ----
            ps1 = pstat.tile([1, c.CN], F32, tag="ps1")
            ps2 = pstat2.tile([1, c.CN], F32, tag="ps2")
            n_st = 2 * c.CT
            idx = 0
            for h in range(NH):
                for pl in ("re", "im"):
                    rw = ret_w[(h, pl)]
                    sq = wide.tile([128, W], BF16, tag="sq", name="sq")
                    vec.tensor_mul(sq[:], rw[:], rw[:])
                    if not stats_on:
                        continue
                    for s in range(SEGS):
                        seg = slice(s * c.CN, (s + 1) * c.CN)
                        nc.tensor.matmul(ps1[:], ones_bf[:], rw[:, seg],
                                         start=(idx == 0), stop=(idx == n_st - 1))
                        nc.tensor.matmul(ps2[:], ones_bf[:], sq[:, seg],
                                         start=(idx == 0), stop=(idx == n_st - 1))
                        idx += 1
            if not stats_on:
                nc.tensor.matmul(ps1[:], ones_bf[:], ret_w[(0, "re")][:, 0:c.CN],
                                 start=True, stop=True)
                nc.tensor.matmul(ps2[:], ones_bf[:], ret_w[(0, "im")][:, 0:c.CN],
                                 start=True, stop=True)
            s1c = stc.tile([1, c.CN], F32, tag="s1c", name="s1c")
            sca.copy(s1c[:], ps1[:])
            nc.sync.dma_start(out=stats[0:1, tok], in_=s1c[:])
            s2c = stc.tile([1, c.CN], F32, tag="s2c", name="s2c")
            sca.copy(s2c[:], ps2[:])
            nc.sync.dma_start(out=stats[1:2, tok], in_=s2c[:])

            # ---- proj_out partial (accumulate over all chpl tiles) ----
            ob = obp.tile([128, c.DT, c.CN], F32, tag="ob", name="ob")
            for d in range(c.DT):
                po = pout.tile([128, c.CN], F32, tag="pout")
                for k in range(kt2_lim):
                    if k < c.CT:
                        h, s, pl = k // SEGS, k % SEGS, "re"
                    else:
                        h, s, pl = (k - c.CT) // SEGS, (k - c.CT) % SEGS, "im"
                    rt = ret_w[(h, pl)][:, s * c.CN:(s + 1) * c.CN]
                    nc.tensor.matmul(po[:], w2_t[k][:, d * 128:(d + 1) * 128],
                                     rt, start=(k == 0), stop=(k == kt2_lim - 1))
                sca.copy(ob[:, d, :], po[:])
            nc.sync.dma_start(out=outp[:, :, tok], in_=ob[:])

    return nc


def build_program_v2(cfg: Cfg, reps: int = 1, hw_reps: int = 1,
                     probe: str | None = None):
    """v2: f16 datapath, CN=512, double-angle sin/cos (one pos tensor),
    stats folded on DVE (single ones-matmul per chunk), engine-balanced.

    key = sigma(mg)*exp(i*theta), theta = pi*tanh(ph) + pos.
    With th = theta/2 = (pi/2)*tanh(ph) + pos/2 (|th| <= pi, LUT-valid):
      sh = sin(th), ch = cos(th) = sin(pi/2 - |th|)
      kreN = (sh^2 - 0.5)*mg2 = -sigma*cos(theta)   (mg2 = tanh(mg/2)+1 = 2*sigma)
      kim  = sh*ch*mg2        =  sigma*sin(theta)
    The negated real part flows through the scan (SreN = -Sre); retrieval
    compensates: rre = mim*qim - mreN*qre ; rim = mim*qre + mreN*qim.
    """
    c = cfg
    assert c.CT % 4 == 0
    SEGS = 4
    NH = c.CT // SEGS
    W = SEGS * c.CN
    nc = bass.Bass()

    class _Dup:
        def __init__(self, eng, on):
            self._eng, self._on = eng, on

        def __getattr__(self, n):
            f = getattr(self._eng, n)
            if not self._on:
                return f

            def g(*a, **k):
                r = f(*a, **k)
                f(*a, **k)
                return r
            return g

    pset = set(probe.split(",")) if probe else set()
    vec = _Dup(nc.vector, "dve2" in pset)
    sca = _Dup(nc.scalar, "act2" in pset)
    kt1_lim = c.KT1 // 2 if "pein_half" in pset else c.KT1
    kt2_lim = c.KT2 // 2 if "peout_half" in pset else c.KT2

    fp8mag = "fp8mag" in pset
    pin2 = "pin512" in pset
    if fp8mag:
        w13 = nc.dram_tensor("w13", [128, c.KT1, 3 * c.NCH], F16,
                             kind="ExternalInput")
        w1m8 = nc.dram_tensor("w1m8", [128, c.KT1 // 2, 2, c.NCH], F8,
                              kind="ExternalInput")
        x8 = nc.dram_tensor("x8", [128, c.KT1, c.NTOK], F8,
                            kind="ExternalInput")
    else:
        w1 = nc.dram_tensor("w1", [128, c.KT1, 4 * c.NCH], F16,
                            kind="ExternalInput")
    w2 = nc.dram_tensor("w2", [128, c.KT2, c.DIM], F16, kind="ExternalInput")
    xt = nc.dram_tensor("xt", [128, c.KT1, c.NTOK], F16, kind="ExternalInput")
    pos = nc.dram_tensor("pos", [128, c.CT, c.T], F16, kind="ExternalInput")
    outp = nc.dram_tensor("outp", [128, c.DT, c.NTOK], BF16,
                          kind="ExternalOutput")
    stats = nc.dram_tensor("stats", [2, c.NTOK], F32, kind="ExternalOutput")

    from contextlib import ExitStack
    with tile.TileContext(nc) as tc, ExitStack() as es:
        small = c.CN <= 256
        praw_bufs = 4 if ((small and not pin2) or "praw1b" in pset) else 2
        pout_bufs = 2
        if "praw3" in pset:
            praw_bufs, pout_bufs = (6, 2) if small else (3, 1)
        consts = es.enter_context(tc.tile_pool(name="consts", bufs=1))
        stream = es.enter_context(tc.tile_pool(name="stream", bufs=2))
        wide_bufs = 2 if small else 1
        for p_ in pset:
            if p_.startswith("wb"):
                wide_bufs = int(p_[2:])
        wide = es.enter_context(tc.tile_pool(name="wide", bufs=wide_bufs))
        retp = es.enter_context(tc.tile_pool(name="retp", bufs=2 if small else 1))
        obp = es.enter_context(tc.tile_pool(name="obp",
                                            bufs=1 if (pin2 or not small) else 2))
        small_stage = pin2 or "obsplit" in pset
        stb = es.enter_context(tc.tile_pool(name="stb",
                                            bufs=1 if small_stage else 2))
        stc = es.enter_context(tc.tile_pool(name="stc",
                                            bufs=1 if small_stage else 2))
        praw = es.enter_context(tc.tile_pool(name="praw", bufs=praw_bufs,
                                             space="PSUM"))
        pstat = es.enter_context(tc.tile_pool(name="pstat", bufs=1, space="PSUM"))
        pout = es.enter_context(tc.tile_pool(name="pout", bufs=pout_bufs,
                                             space="PSUM"))

        if fp8mag:
            w1_sb = consts.tile([128, c.KT1, 3 * c.NCH], F16, tag="w1_sb")
            nc.sync.dma_start(out=w1_sb[:], in_=w13[:])
            w1m8_sb = consts.tile([128, c.KT1 // 2, 2, c.NCH], F8,
                                  tag="w1m8_sb")
            nc.sync.dma_start(out=w1m8_sb[:], in_=w1m8[:])
        else:
            w1_sb = consts.tile([128, c.KT1, 4 * c.NCH], F16, tag="w1_sb")
            nc.sync.dma_start(out=w1_sb[:], in_=w1[:])
        w2_sb = consts.tile([128, c.KT2, c.DIM], F16, tag="w2_sb")
        nc.sync.dma_start(out=w2_sb[:], in_=w2[:])
        w1_t = [w1_sb[:, k, :] for k in range(c.KT1)]
        w2_t = [w2_sb[:, k, :] for k in range(c.KT2)]

        ones_bf = consts.tile([128, 1], BF16, tag="ones")
        vec.memset(ones_bf[:], 1.0)
        one_f = consts.tile([128, 1], F32, tag="one_f")
        vec.memset(one_f[:], 1.0)
        half_pi = consts.tile([128, 1], F32, tag="half_pi")
        vec.memset(half_pi[:], PI / 2)
        car = {}
        for h in range(NH):
            for pl in ("re", "im"):
                car[(h, pl)] = consts.tile([128, SEGS], F16, tag=f"car_{h}_{pl}",
                                           name=f"car_{h}_{pl}")

        if hw_reps > 1:
            es.enter_context(tc.For_i(0, hw_reps))

        h2 = W // 2

        po512 = "po512" in pset

        def _rv(x):
            # ret entries are tiles normally, [128, SEGS, CN] AP views in
            # po512 mode
            return x[:] if not po512 else x

        def _3d(tile_):
            return tile_.rearrange("p (s t) -> p s t", s=SEGS)

        def emit_tail_pre(st):
            """DVE folds + ACT squares for the previous chunk's stats."""
            rw = st["ret"]
            a = wide.tile([128, W], F16, tag="stA", bufs=1, name="a")
            vec.tensor_add(_3d(a) if po512 else a[:],
                           _rv(rw[(0, "re")]), _rv(rw[(0, "im")]))
            b = wide.tile([128, W], F16, tag="stB", bufs=1, name="b")
            vec.tensor_add(_3d(b) if po512 else b[:],
                           _rv(rw[(1, "re")]), _rv(rw[(1, "im")]))
            rs = wide.tile([128, W], F16, tag="stC", bufs=1, name="rs")
            vec.tensor_add(rs[:], a[:], b[:])
            f = wide.tile([128, h2], F16, tag="stD", bufs=1, name="f")
            vec.tensor_add(f[:], rs[:, 0:h2], rs[:, h2:W])
            stt = stb.tile([128, 2 * c.CN], BF16, tag="stt", name="stt")
            vec.tensor_add(stt[:, 0:c.CN], f[:, 0:c.CN], f[:, c.CN:h2])
            def _sq(dst, src):
                if "sqdve" in pset:
                    vec.tensor_mul(dst, src, src)
                else:
                    sca.activation(dst, src, AF.Square)
            s0 = wide.tile([128, W], BF16, tag="stA", bufs=1, name="s0")
            _sq(_3d(s0) if po512 else s0[:], _rv(rw[(0, "re")]))
            s1 = wide.tile([128, W], BF16, tag="stB", bufs=1, name="s1")
            _sq(_3d(s1) if po512 else s1[:], _rv(rw[(0, "im")]))
            a2 = wide.tile([128, W], BF16, tag="stC", bufs=1, name="a2")
            vec.tensor_add(a2[:], s0[:], s1[:])
            s2 = wide.tile([128, W], BF16, tag="stA", bufs=1, name="s2")
            _sq(_3d(s2) if po512 else s2[:], _rv(rw[(1, "re")]))
            s3 = wide.tile([128, W], BF16, tag="stB", bufs=1, name="s3")
            _sq(_3d(s3) if po512 else s3[:], _rv(rw[(1, "im")]))
            b2 = wide.tile([128, W], BF16, tag="stD", bufs=1, name="b2")
            vec.tensor_add(b2[:], s2[:], s3[:])
            ss = wide.tile([128, W], BF16, tag="stA", bufs=1, name="ss")
            vec.tensor_add(ss[:], a2[:], b2[:])
            f2 = wide.tile([128, h2], BF16, tag="stB", bufs=1, name="f2")
            vec.tensor_add(f2[:], ss[:, 0:h2], ss[:, h2:W])
            vec.tensor_add(stt[:, c.CN:2 * c.CN], f2[:, 0:c.CN],
                           f2[:, c.CN:h2])
            st["stt"] = stt

        def emit_tail_mm(st):
            """Stats matmul + proj_out for the previous chunk."""
            stt, tok_p, rw = st["stt"], st["tok"], st["ret"]
            ps = pstat.tile([1, 2 * c.CN], F32, tag="ps")
            nc.tensor.matmul(ps[:, 0:c.CN], ones_bf[:], stt[:, 0:c.CN],
                             start=True, stop=True)
            nc.tensor.matmul(ps[:, c.CN:2 * c.CN], ones_bf[:],
                             stt[:, c.CN:2 * c.CN], start=True, stop=True)
            obsplit = "obsplit" in pset
            if obsplit:
                sc1 = stc.tile([1, c.CN], F32, tag="sc", name="sc1")
                sca.copy(sc1[:], ps[:, 0:c.CN])
                nc.sync.dma_start(out=stats[0:1, tok_p], in_=sc1[:])
                sc2 = stc.tile([1, c.CN], F32, tag="sc", name="sc2")
                sca.copy(sc2[:], ps[:, c.CN:2 * c.CN])
                nc.sync.dma_start(out=stats[1:2, tok_p], in_=sc2[:])
            else:
                sc = stc.tile([1, 2 * c.CN], F32, tag="sc", name="sc")
                sca.copy(sc[:], ps[:])
                nc.sync.dma_start(out=stats[0:1, tok_p], in_=sc[:, 0:c.CN])
                nc.sync.dma_start(out=stats[1:2, tok_p],
                                  in_=sc[:, c.CN:2 * c.CN])
            if po512:
                return
            if not obsplit:
                ob = obp.tile([128, c.DT, c.CN], BF16, tag="ob", name="ob")
            for d in range(c.DT):
                po = pout.tile([128, c.CN], F32, tag="pout")
                for k in range(kt2_lim):
                    if k < c.CT:
                        h, s, pl = k // SEGS, k % SEGS, "re"
                    else:
                        h, s, pl = (k - c.CT) // SEGS, (k - c.CT) % SEGS, "im"
                    rt = rw[(h, pl)][:, s * c.CN:(s + 1) * c.CN]
                    nc.tensor.matmul(po[:], w2_t[k][:, d * 128:(d + 1) * 128],
                                     rt, start=(k == 0), stop=(k == kt2_lim - 1))
                if obsplit:
                    obd = obp.tile([128, c.CN], BF16, tag="ob", bufs=2,
                                   name=f"ob{d}")
                    if "obdve" in pset:
                        vec.tensor_copy(obd[:], po[:])
                    else:
                        sca.copy(obd[:], po[:])
                    nc.sync.dma_start(out=outp[:, d, tok_p], in_=obd[:])
                elif "obdve" in pset:
                    vec.tensor_copy(ob[:, d, :], po[:])
                else:
                    sca.copy(ob[:, d, :], po[:])
            if not obsplit:
                nc.sync.dma_start(out=outp[:, :, tok_p], in_=ob[:])

        def emit_po_pair(pd):
            """proj_out over a 2-chunk pair: N=512 matmuls from the paired
            ret layout [128, SEGS, 2, CN] (tokens of both chunks contiguous
            per channel segment)."""
            pair, tok2 = pd["pair"], pd["tok2"]
            for d in range(c.DT):
                po = pout.tile([128, 2 * c.CN], F32, tag="pout")
                for k in range(kt2_lim):
                    if k < c.CT:
                        h, s, pl = k // SEGS, k % SEGS, "re"
                    else:
                        h, s, pl = (k - c.CT) // SEGS, (k - c.CT) % SEGS, "im"
                    rt = pair[(h, pl)][:, s, :, :]
                    nc.tensor.matmul(po[:], w2_t[k][:, d * 128:(d + 1) * 128],
                                     rt, start=(k == 0),
                                     stop=(k == kt2_lim - 1))
                obd = obp.tile([128, 2 * c.CN], BF16, tag="ob", bufs=2,
                               name=f"obp{d}")
                if "obdve" in pset:
                    vec.tensor_copy(obd[:], po[:])
                else:
                    sca.copy(obd[:], po[:])
                nc.sync.dma_start(out=outp[:, d, tok2], in_=obd[:])

        prev = None
        pair_cur = None
        pair_done = None
        pend_in = {}
        for n in [nn_ for _ in range(reps) for nn_ in range(c.NCHUNK)]:
            t0 = (n % c.CPB) * c.CN
            first_in_batch = t0 == 0
            tok = slice(n * c.CN, (n + 1) * c.CN)

            if prev is not None:
                emit_tail_pre(prev)
            if po512 and n % 2 == 0:
                pair_cur = {
                    (h, pl): retp.tile([128, SEGS, 2, c.CN], F16,
                                       tag=f"pair_{h}_{pl}",
                                       name=f"pair_{h}_{pl}")
                    for h in range(NH) for pl in ("re", "im")}

            if "obsplit" in pset:
                posb_h = []
                for h in range(NH):
                    pb = stream.tile([128, SEGS, c.CN], F16, tag=f"posb{h}")
                    nc.sync.dma_start(
                        out=pb[:],
                        in_=pos[:, h * SEGS:(h + 1) * SEGS, t0:t0 + c.CN])
                    posb_h.append(pb)
            else:
                posb = stream.tile([128, c.CT, c.CN], F16, tag="posb")
                nc.sync.dma_start(out=posb[:], in_=pos[:, :, t0:t0 + c.CN])

            gnames = {"ph": "th_ph", "mg": "th_mg", "qr": "qre", "qi": "qim"}
            if not pin2 or n % 2 == 0:
                ntin = 2 * c.CN if pin2 else c.CN
                itok = slice(n * c.CN, n * c.CN + ntin)
                sb = 1 if pin2 else None
                xcb = stream.tile([128, c.KT1, ntin], F16, tag="xcb", bufs=sb)
                nc.sync.dma_start(out=xcb[:], in_=xt[:, :, itok])
                if fp8mag:
                    x8cb = stream.tile([128, c.KT1, ntin], F8, tag="x8cb",
                                       bufs=sb)
                    nc.sync.dma_start(out=x8cb[:], in_=x8[:, :, itok])
                ib = 4 if pin2 else None
                cur_in, nxt_in = {}, {}
                for h in range(NH):
                    for nm in ("th_ph", "th_mg", "qre", "qim"):
                        cur_in[(h, nm)] = wide.tile([128, W], F16, tag=nm,
                                                    name=nm, bufs=ib)
                        if pin2:
                            nxt_in[(h, nm)] = wide.tile([128, W], F16, tag=nm,
                                                        name=nm + "b", bufs=ib)
                dmaps = [(cur_in, 0)] + ([(nxt_in, 1)] if pin2 else [])
                if "praw1b" in pset:
                    # one-bank psum tiles: one channel tile per GEMM group
                    assert not pin2
                    for h in range(NH):
                        i0 = h * SEGS
                        for j in range(SEGS):
                            for g in ("ph", "mg", "qr", "qi"):
                                p = praw.tile([128, c.CN], F32, tag="praw")
                                nm = gnames[g]
                                wcols = slice(j * c.CN, (j + 1) * c.CN)
                                dst = cur_in[(h, nm)][:, wcols]
                                if g == "mg" and fp8mag:
                                    nk = c.KT1 // 2
                                    mch = i0 + j
                                    for kk in range(nk):
                                        nc.tensor.matmul(
                                            p[:],
                                            w1m8_sb[:, kk, :,
                                                    mch * 128:
                                                    (mch + 1) * 128],
                                            x8cb[:, 2 * kk:2 * kk + 2, :],
                                            start=(kk == 0),
                                            stop=(kk == nk - 1),
                                            perf_mode=mybir.MatmulPerfMode
                                            .DoubleRow)
                                    sca.activation(dst, p[:], AF.Tanh,
                                                   scale=0.5 / 1024.0)
                                    continue
                                if fp8mag:
                                    gi = {"ph": 0, "qr": 1, "qi": 2}[g]
                                else:
                                    gi = {"ph": 0, "mg": 1, "qr": 2,
                                          "qi": 3}[g]
                                m = gi * c.CT + i0 + j
                                for k in range(kt1_lim):
                                    nc.tensor.matmul(
                                        p[:],
                                        w1_t[k][:, m * 128:(m + 1) * 128],
                                        xcb[:, k, :],
                                        start=(k == 0),
                                        stop=(k == kt1_lim - 1))
                                if g == "ph":
                                    sca.activation(dst, p[:], AF.Tanh)
                                elif g == "mg":
                                    sca.activation(dst, p[:], AF.Tanh,
                                                   scale=0.5)
                                elif "qdve" in pset:
                                    vec.tensor_copy(dst, p[:])
                                else:
                                    sca.copy(dst, p[:])
                    # fall through to the chain loop below
                for h in [] if "praw1b" in pset else range(NH):
                    i0 = h * SEGS
                    for j in range(0, SEGS, 2):
                        for g in ("ph", "mg", "qr", "qi"):
                            p = praw.tile([128, 2 * ntin], F32, tag="praw")
                            pv = p.rearrange("p (c t) -> p c t", c=2)
                            nm = gnames[g]
                            wcols = slice(j * c.CN, (j + 2) * c.CN)
                            if g == "mg" and fp8mag:
                                nk = c.KT1 // 2
                                for half in range(2):
                                    mch = i0 + j + half
                                    for kk in range(nk):
                                        nc.tensor.matmul(
                                            pv[:, half, :],
                                            w1m8_sb[:, kk, :,
                                                    mch * 128:(mch + 1) * 128],
                                            x8cb[:, 2 * kk:2 * kk + 2, :],
                                            start=(kk == 0),
                                            stop=(kk == nk - 1),
                                            perf_mode=mybir.MatmulPerfMode.DoubleRow)
                                for dmap, cc in dmaps:
                                    src = pv[:, :, cc * c.CN:(cc + 1) * c.CN]
                                    sca.activation(dmap[(h, nm)][:, wcols],
                                                   src, AF.Tanh,
                                                   scale=0.5 / 1024.0)
                                continue
                            if fp8mag:
                                gi = {"ph": 0, "qr": 1, "qi": 2}[g]
                            else:
                                gi = {"ph": 0, "mg": 1, "qr": 2, "qi": 3}[g]
                            for half in range(2):
                                m = gi * c.CT + i0 + j + half
                                for k in range(kt1_lim):
                                    nc.tensor.matmul(
                                        pv[:, half, :],
                                        w1_t[k][:, m * 128:(m + 1) * 128],
                                        xcb[:, k, :],
                                        start=(k == 0),
                                        stop=(k == kt1_lim - 1))
                            for dmap, cc in dmaps:
                                src = pv[:, :, cc * c.CN:(cc + 1) * c.CN]
                                dst = dmap[(h, nm)][:, wcols]
                                if g == "ph":
                                    sca.activation(dst, src, AF.Tanh)
                                elif g == "mg":
                                    sca.activation(dst, src, AF.Tanh, scale=0.5)
                                elif "qdve" in pset:
                                    vec.tensor_copy(dst, src)
                                else:
                                    sca.copy(dst, src)
                if pin2:
                    pend_in.clear()
                    pend_in.update(nxt_in)
            else:
                cur_in = dict(pend_in)

            ret_w = {}
            hs = {h: {} for h in range(NH)}

            def lv_theta(h):
                st = hs[h]
                i0 = h * SEGS
                pos_h = (posb_h[h][:] if "obsplit" in pset
                         else posb[:, i0:i0 + SEGS, :])
                theta = wide.tile([128, W], F16, tag="theta", name="theta")
                vec.scalar_tensor_tensor(theta[:], cur_in[(h, "th_ph")][:],
                                         PI / 2, pos_h, ALU.mult, ALU.add)
                st["theta"] = theta

            def lv_sh(h):
                st = hs[h]
                sh = wide.tile([128, W], F16, tag="sh", name="sh")
                sca.activation(sh[:], st["theta"][:], AF.Sin)
                st["sh"] = sh

            def lv_ab(h):
                st = hs[h]
                ab = wide.tile([128, W], F16, tag="ab", name="ab")
                if "abdve" in pset:
                    vec.scalar_tensor_tensor(ab[:], st["theta"][:], -1.0,
                                             st["theta"][:],
                                             ALU.mult, ALU.max)
                else:
                    sca.activation(ab[:], st["theta"][:], AF.Abs)
                st["ab"] = ab

            def lv_ch(h):
                st = hs[h]
                ch = wide.tile([128, W], F16, tag="theta", name="ch")
                sca.activation(ch[:], st["ab"][:], AF.Sin, bias=half_pi[:],
                               scale=-1.0)
                st["ch"] = ch

            def lv_sqh(h):
                st = hs[h]
                sqh = wide.tile([128, W], F16, tag="ab", name="sqh")
                if "sqhdve" in pset:
                    vec.tensor_mul(sqh[:], st["sh"][:], st["sh"][:])
                else:
                    sca.activation(sqh[:], st["sh"][:], AF.Square)
                st["sqh"] = sqh

            def lv_mg2(h):
                st = hs[h]
                mg2 = wide.tile([128, W], F16, tag="mg2", name="mg2")
                if "mg2dve" in pset:
                    vec.tensor_scalar(mg2[:], cur_in[(h, "th_mg")][:], 1.0,
                                      None, ALU.add)
                else:
                    sca.activation(mg2[:], cur_in[(h, "th_mg")][:],
                                   AF.Identity, bias=one_f[:])
                st["mg2"] = mg2

            def lv_kreN(h):
                st = hs[h]
                kreN = wide.tile([128, W], F16,
                                 tag="kreN" if pin2 else "th_ph", name="kreN")
                vec.scalar_tensor_tensor(kreN[:], st["sqh"][:], 0.5,
                                         st["mg2"][:],
                                         ALU.subtract, ALU.mult)
                st["kreN"] = kreN

            def lv_tt(h):
                st = hs[h]
                tt = wide.tile([128, W], F16,
                               tag="tt" if pin2 else "th_mg", name="tt")
                vec.tensor_mul(tt[:], st["sh"][:], st["ch"][:])
                st["tt"] = tt

            def lv_kim(h):
                st = hs[h]
                kim = wide.tile([128, W], F16, tag="sh", name="kim")
                vec.tensor_mul(kim[:], st["tt"][:], st["mg2"][:])
                st["kim"] = kim

            def lv_scan(h):
                st = hs[h]
                mre = wide.tile([128, W], F16, tag="mre", name="mre")
                mim = wide.tile([128, W], F16, tag="mim", name="mim")
                for s in range(SEGS):
                    seg = slice(s * c.CN, (s + 1) * c.CN)
                    init_re = (0.0 if first_in_batch
                               else car[(h, "re")][:, s:s + 1])
                    vec.tensor_tensor_scan(mre[:, seg], st["kreN"][:, seg],
                                           st["kreN"][:, seg], init_re,
                                           ALU.add, ALU.bypass)
                    init_im = (0.0 if first_in_batch
                               else car[(h, "im")][:, s:s + 1])
                    vec.tensor_tensor_scan(mim[:, seg], st["kim"][:, seg],
                                           st["kim"][:, seg], init_im,
                                           ALU.add, ALU.bypass)
                st["mre"], st["mim"] = mre, mim

            def lv_carry(h):
                st = hs[h]
                if (n % c.CPB) != c.CPB - 1:
                    cre = st["mre"].rearrange("p (s t) -> p s t",
                                              s=SEGS)[:, :, c.CN - 1]
                    vec.tensor_copy(car[(h, "re")][:], cre)
                    cim = st["mim"].rearrange("p (s t) -> p s t",
                                              s=SEGS)[:, :, c.CN - 1]
                    vec.tensor_copy(car[(h, "im")][:], cim)

            def lv_retre(h):
                # retrieval (mreN = -Sre):
                #   rre = mim*qim - mreN*qre ; rim = mim*qre + mreN*qim
                st = hs[h]
                qre, qim = cur_in[(h, "qre")], cur_in[(h, "qim")]
                r1 = wide.tile([128, W], F16, tag="theta", name="r1")
                vec.tensor_mul(r1[:], st["mre"][:], qre[:])
                r2 = wide.tile([128, W], F16, tag="ab", name="r2")
                vec.tensor_mul(r2[:], st["mim"][:], qim[:])
                if po512:
                    dst = pair_cur[(h, "re")][:, :, n % 2, :]
                    vec.tensor_sub(dst, _3d(r2), _3d(r1))
                    ret_w[(h, "re")] = dst
                else:
                    rre = retp.tile([128, W], F16, tag=f"ret_re_{h}",
                                    name=f"ret_re_{h}")
                    vec.tensor_sub(rre[:], r2[:], r1[:])
                    ret_w[(h, "re")] = rre

            def lv_retim(h):
                st = hs[h]
                qre, qim = cur_in[(h, "qre")], cur_in[(h, "qim")]
                r3 = wide.tile([128, W], F16, tag="theta", name="r3")
                vec.tensor_mul(r3[:], st["mim"][:], qre[:])
                r4 = wide.tile([128, W], F16, tag="ab", name="r4")
                vec.tensor_mul(r4[:], st["mre"][:], qim[:])
                if po512:
                    dst = pair_cur[(h, "im")][:, :, n % 2, :]
                    vec.tensor_add(dst, _3d(r3), _3d(r4))
                    ret_w[(h, "im")] = dst
                else:
                    rim = retp.tile([128, W], F16, tag=f"ret_im_{h}",
                                    name=f"ret_im_{h}")
                    vec.tensor_add(rim[:], r3[:], r4[:])
                    ret_w[(h, "im")] = rim

            levels = [lv_theta, lv_sh, lv_ab, lv_ch, lv_sqh, lv_mg2,
                      lv_kreN, lv_tt, lv_kim, lv_scan, lv_carry,
                      lv_retre, lv_retim]
            if "ilv" in pset:
                for lv in levels:
                    for h in range(NH):
                        lv(h)
            else:
                for h in range(NH):
                    for lv in levels:
                        lv(h)

            if prev is not None:
                emit_tail_mm(prev)
                if po512 and n % 2 == 0 and pair_done is not None:
                    emit_po_pair(pair_done)
            if po512 and n % 2 == 1:
                pair_done = {"pair": pair_cur,
                             "tok2": slice((n - 1) * c.CN, (n + 1) * c.CN)}
            prev = {"ret": ret_w, "tok": tok}

        emit_tail_pre(prev)
        emit_tail_mm(prev)
        if po512:
            emit_po_pair(pair_done)

    return nc


def build_program_v3(cfg: Cfg, reps: int = 1, hw_reps: int = 1,
                     probe: str | None = None):
    """v3: all-f16 matmuls (fp8 dropped -- measured no win on HW), ACT chain
    cut to 3 LUT ops/half, Pool (GPSIMD) engine recruited for the squares
    and sin-products, per-chunk emission ordered so every engine queue is
    dependency-ready (ph/mg GEMMs before q GEMMs, chain interleaved).

    Engine budget per 256-token chunk (target: PE-bound):
      PE   proj_in 256 MM + stats 2 + proj_out 128 MM        ~28.5us
      DVE  theta/mg2/kreN/kim, scans, retrieval, folds, ob   ~23us
      ACT  16 psum drains + sh/ab/ch + sc                    ~16.5us
      Pool sq/tt2 + stats squares                            ~17us
    """
    c = cfg
    assert c.CT % 4 == 0
    SEGS = 4
    NH = c.CT // SEGS
    W = SEGS * c.CN
    nc = bass.Bass()

    class _Dup:
        def __init__(self, eng, on):
            self._eng, self._on = eng, on

        def __getattr__(self, n):
            f = getattr(self._eng, n)
            if not self._on:
                return f

            def g(*a, **k):
                r = f(*a, **k)
                f(*a, **k)
                return r
            return g

    pset = set(probe.split(",")) if probe else set()
    vec = _Dup(nc.vector, "dve2" in pset)
    sca = _Dup(nc.scalar, "act2" in pset)
    pool = _Dup(nc.gpsimd, "pool2" in pset)
    kt1_lim = c.KT1 // 2 if "pein_half" in pset else c.KT1
    kt2_lim = c.KT2 // 2 if "peout_half" in pset else c.KT2
    if "sqact" in pset:                              # stats squares
        def emit_sq(dst, src):
            sca.activation(dst, src, AF.Square)
    elif "sqdve" in pset:
        def emit_sq(dst, src):
            vec.tensor_mul(dst, src, src)
    else:
        def emit_sq(dst, src):
            pool.tensor_mul(dst, src, src)
    ch_eng = vec if "poolchain_off" in pset else pool  # sq/tt2 in chain

    w1 = nc.dram_tensor("w1", [128, c.KT1, 4 * c.NCH], F16,
                        kind="ExternalInput")
    w2 = nc.dram_tensor("w2", [128, c.KT2, c.DIM], F16, kind="ExternalInput")
    xt = nc.dram_tensor("xt", [128, c.KT1, c.NTOK], F16, kind="ExternalInput")
    pos = nc.dram_tensor("pos", [128, c.CT, c.T], F16, kind="ExternalInput")
    DD = c.DT // 2
    outp = nc.dram_tensor("outp", [128, DD, 2, c.NTOK], BF16,
                          kind="ExternalOutput")
    stats = nc.dram_tensor("stats", [2, c.NTOK], F32, kind="ExternalOutput")

    from contextlib import ExitStack
    with tile.TileContext(nc) as tc, ExitStack() as es:
        praw_bufs = 4 if "praw4" in pset else 6
        consts = es.enter_context(tc.tile_pool(name="consts", bufs=1))
        stream = es.enter_context(tc.tile_pool(name="stream", bufs=2))
        wide = es.enter_context(tc.tile_pool(name="wide", bufs=2))
        retp = es.enter_context(tc.tile_pool(name="retp", bufs=2))
        obp = es.enter_context(tc.tile_pool(name="obp", bufs=2))
        stb = es.enter_context(tc.tile_pool(name="stb", bufs=2))
        stc = es.enter_context(tc.tile_pool(name="stc", bufs=2))
        praw = es.enter_context(tc.tile_pool(name="praw", bufs=praw_bufs,
                                             space="PSUM"))
        pstat = es.enter_context(tc.tile_pool(name="pstat", bufs=1,
                                              space="PSUM"))
        pout = es.enter_context(tc.tile_pool(name="pout", bufs=1,
                                             space="PSUM"))

        w1_sb = consts.tile([128, c.KT1, 4 * c.NCH], F16, tag="w1_sb")
        nc.sync.dma_start(out=w1_sb[:], in_=w1[:])
        w2_sb = consts.tile([128, c.KT2, c.DIM], F16, tag="w2_sb")
        nc.sync.dma_start(out=w2_sb[:], in_=w2[:])
        w1_t = [w1_sb[:, k, :] for k in range(c.KT1)]
        w2_t = [w2_sb[:, k, :] for k in range(c.KT2)]

        ones_bf = consts.tile([128, 1], BF16, tag="ones")
        vec.memset(ones_bf[:], 1.0)
        half_pi = consts.tile([128, 1], F32, tag="half_pi")
        vec.memset(half_pi[:], PI / 2)
        car = {}
        for h in range(NH):
            for pl in ("re", "im"):
                car[(h, pl)] = consts.tile([128, SEGS], F16,
                                           tag=f"car_{h}_{pl}",
                                           name=f"car_{h}_{pl}")

        if hw_reps > 1:
            es.enter_context(tc.For_i(0, hw_reps))

        h2 = W // 2

        def emit_tail_pre(st):
            """Stats for chunk n-1: Pool squares + DVE fold tree."""
            rw = st["ret"]
            sqs = {}
            for h in range(NH):
                for pl in ("re", "im"):
                    s = wide.tile([128, W], BF16, tag=f"sq_{h}_{pl}", bufs=1,
                                  name=f"s_{h}_{pl}")
                    emit_sq(s[:], rw[(h, pl)][:])
                    sqs[(h, pl)] = s
            a = wide.tile([128, W], F16, tag="stA", bufs=1, name="a")
            vec.tensor_add(a[:], rw[(0, "re")][:], rw[(0, "im")][:])
            b = wide.tile([128, W], F16, tag="stB", bufs=1, name="b")
            vec.tensor_add(b[:], rw[(1, "re")][:], rw[(1, "im")][:])
            rs = wide.tile([128, W], F16, tag="stC", bufs=1, name="rs")
            vec.tensor_add(rs[:], a[:], b[:])
            f = wide.tile([128, h2], F16, tag="stD", bufs=1, name="f")
            vec.tensor_add(f[:], rs[:, 0:h2], rs[:, h2:W])
            stt = stb.tile([128, 2 * c.CN], BF16, tag="stt", name="stt")
            vec.tensor_add(stt[:, 0:c.CN], f[:, 0:c.CN], f[:, c.CN:h2])
            a2 = wide.tile([128, W], BF16, tag="stA", bufs=1, name="a2")
            vec.tensor_add(a2[:], sqs[(0, "re")][:], sqs[(0, "im")][:])
            b2 = wide.tile([128, W], BF16, tag="stB", bufs=1, name="b2")
            vec.tensor_add(b2[:], sqs[(1, "re")][:], sqs[(1, "im")][:])
            ss = wide.tile([128, W], BF16, tag="stC", bufs=1, name="ss")
            vec.tensor_add(ss[:], a2[:], b2[:])
            f2 = wide.tile([128, h2], BF16, tag="stD", bufs=1, name="f2")
            vec.tensor_add(f2[:], ss[:, 0:h2], ss[:, h2:W])
            vec.tensor_add(stt[:, c.CN:2 * c.CN], f2[:, 0:c.CN],
                           f2[:, c.CN:h2])
            st["stt"] = stt

        def emit_tail_mm(st):
            """Stats matmuls + proj_out for chunk n-1."""
            stt, tok_p, rw = st["stt"], st["tok"], st["ret"]
            ps = pstat.tile([1, 2 * c.CN], F32, tag="ps")
            nc.tensor.matmul(ps[:, 0:c.CN], ones_bf[:], stt[:, 0:c.CN],
                             start=True, stop=True)
            nc.tensor.matmul(ps[:, c.CN:2 * c.CN], ones_bf[:],
                             stt[:, c.CN:2 * c.CN], start=True, stop=True)
            sc = stc.tile([1, 2 * c.CN], F32, tag="sc", name="sc")
            sca.copy(sc[:], ps[:])
            nc.sync.dma_start(out=stats[0:1, tok_p], in_=sc[:, 0:c.CN])
            nc.sync.dma_start(out=stats[1:2, tok_p], in_=sc[:, c.CN:2 * c.CN])
            for dd in range(DD):
                po = pout.tile([128, 2, c.CN], F32, tag="pout")
                for di in range(2):
                    d = dd * 2 + di
                    for k in range(kt2_lim):
                        if k < c.CT:
                            h, s, pl = k // SEGS, k % SEGS, "re"
                        else:
                            h, s, pl = ((k - c.CT) // SEGS,
                                        (k - c.CT) % SEGS, "im")
                        rt = rw[(h, pl)][:, s * c.CN:(s + 1) * c.CN]
                        nc.tensor.matmul(po[:, di, :],
                                         w2_t[k][:, d * 128:(d + 1) * 128],
                                         rt, start=(k == 0),
                                         stop=(k == kt2_lim - 1))
                ob = obp.tile([128, 2, c.CN], BF16, tag="ob", name="ob")
                if "obact" in pset:
                    sca.copy(ob[:], po[:])
                else:
                    vec.tensor_copy(ob[:], po[:])
                nc.sync.dma_start(out=outp[:, dd, :, tok_p], in_=ob[:])

        def drain_group(p, g, dst, wcols):
            if g == "ph":
                sca.activation(dst[:, wcols], p[:], AF.Tanh)
            elif g == "mg":
                sca.activation(dst[:, wcols], p[:], AF.Tanh, scale=0.5)
            else:
                sca.copy(dst[:, wcols], p[:])

        prev = None
        for n in [nn_ for _ in range(reps) for nn_ in range(c.NCHUNK)]:
            t0 = (n % c.CPB) * c.CN
            first_in_batch = t0 == 0
            tok = slice(n * c.CN, (n + 1) * c.CN)

            if prev is not None:
                emit_tail_pre(prev)

            posb = stream.tile([128, c.CT, c.CN], F16, tag="posb")
            nc.sync.dma_start(out=posb[:], in_=pos[:, :, t0:t0 + c.CN])
            xcb = stream.tile([128, c.KT1, c.CN], F16, tag="xcb")
            nc.sync.dma_start(out=xcb[:], in_=xt[:, :, tok])

            gidx = {"ph": 0, "mg": 1, "qr": 2, "qi": 3}
            ret_w = {}
            for h in range(NH):
                i0 = h * SEGS
                th_ph = wide.tile([128, W], F16, tag="th_ph", name="th_ph")
                th_mg = wide.tile([128, W], F16, tag="th_mg", name="th_mg")
                qre = wide.tile([128, W], F16, tag="qre", name="qre")
                qim = wide.tile([128, W], F16, tag="qim", name="qim")
                dest = {"ph": th_ph, "mg": th_mg, "qr": qre, "qi": qim}

                def gemm_pass(groups):
                    for j in (0, 2):
                        for g in groups:
                            p = praw.tile([128, 2 * c.CN], F32, tag="praw")
                            for half in range(2):
                                m = gidx[g] * c.CT + i0 + j + half
                                cols = slice(half * c.CN, (half + 1) * c.CN)
                                for k in range(kt1_lim):
                                    nc.tensor.matmul(
                                        p[:, cols],
                                        w1_t[k][:, m * 128:(m + 1) * 128],
                                        xcb[:, k, :],
                                        start=(k == 0),
                                        stop=(k == kt1_lim - 1))
                            drain_group(p, g, dest[g],
                                        slice(j * c.CN, (j + 2) * c.CN))

                # phase/magnitude GEMMs first: the chain head depends on them
                gemm_pass(("ph", "mg"))
                theta = wide.tile([128, W], F16, tag="theta", name="theta")
                vec.scalar_tensor_tensor(theta[:], th_ph[:], PI / 2,
                                         posb[:, i0:i0 + SEGS, :],
                                         ALU.mult, ALU.add)
                mg2 = wide.tile([128, W], F16, tag="mg2", name="mg2")
                vec.tensor_scalar(mg2[:], th_mg[:], 1.0, None, ALU.add)
                sh = wide.tile([128, W], F16, tag="sh", name="sh")
                sca.activation(sh[:], theta[:], AF.Sin)
                ab = wide.tile([128, W], F16, tag="ab", bufs=1, name="ab")
                sca.activation(ab[:], theta[:], AF.Abs)
                ch = wide.tile([128, W], F16, tag="ch", bufs=1, name="ch")
                sca.activation(ch[:], ab[:], AF.Sin, bias=half_pi[:],
                               scale=-1.0)
                sq = wide.tile([128, W], F16, tag="sq", bufs=1, name="sq")
                ch_eng.tensor_mul(sq[:], sh[:], sh[:])
                tt2 = wide.tile([128, W], F16, tag="tt2", bufs=1, name="tt2")
                ch_eng.tensor_mul(tt2[:], sh[:], ch[:])

                # query GEMMs while the chain runs on ACT/DVE/Pool
                gemm_pass(("qr", "qi"))

                kreN = wide.tile([128, W], F16, tag="kreN", bufs=1,
                                 name="kreN")
                vec.scalar_tensor_tensor(kreN[:], sq[:], 0.5, mg2[:],
                                         ALU.subtract, ALU.mult)
                kim = wide.tile([128, W], F16, tag="kim", bufs=1, name="kim")
                vec.tensor_mul(kim[:], tt2[:], mg2[:])

                mre = wide.tile([128, W], F16, tag="mre", name="mre")
                mim = wide.tile([128, W], F16, tag="mim", name="mim")
                for s in range(SEGS):
                    seg = slice(s * c.CN, (s + 1) * c.CN)
                    init_re = (0.0 if first_in_batch
                               else car[(h, "re")][:, s:s + 1])
                    vec.tensor_tensor_scan(mre[:, seg], kreN[:, seg],
                                           kreN[:, seg], init_re,
                                           ALU.add, ALU.bypass)
                    init_im = (0.0 if first_in_batch
                               else car[(h, "im")][:, s:s + 1])
                    vec.tensor_tensor_scan(mim[:, seg], kim[:, seg],
                                           kim[:, seg], init_im,
                                           ALU.add, ALU.bypass)
                if (n % c.CPB) != c.CPB - 1:
                    cre = mre.rearrange("p (s t) -> p s t", s=SEGS)[:, :,
                                                                    c.CN - 1]
                    vec.tensor_copy(car[(h, "re")][:], cre)
                    cim = mim.rearrange("p (s t) -> p s t", s=SEGS)[:, :,
                                                                    c.CN - 1]
                    vec.tensor_copy(car[(h, "im")][:], cim)

                # retrieval (mreN = -Sre):
                #   rre = mim*qim - mreN*qre ; rim = mim*qre + mreN*qim
                r1 = wide.tile([128, W], F16, tag="r1", bufs=1, name="r1")
                vec.tensor_mul(r1[:], mre[:], qre[:])
                r2 = wide.tile([128, W], F16, tag="r2", bufs=1, name="r2")
                vec.tensor_mul(r2[:], mim[:], qim[:])
                rre = retp.tile([128, W], F16, tag=f"ret_re_{h}",
                                name=f"ret_re_{h}")
                vec.tensor_sub(rre[:], r2[:], r1[:])
                r3 = wide.tile([128, W], F16, tag="r1", bufs=1, name="r3")
                vec.tensor_mul(r3[:], mim[:], qre[:])
                r4 = wide.tile([128, W], F16, tag="r2", bufs=1, name="r4")
                vec.tensor_mul(r4[:], mre[:], qim[:])
                rim = retp.tile([128, W], F16, tag=f"ret_im_{h}",
                                name=f"ret_im_{h}")
                vec.tensor_add(rim[:], r3[:], r4[:])
                ret_w[(h, "re")] = rre
                ret_w[(h, "im")] = rim

            if prev is not None:
                emit_tail_mm(prev)
            prev = {"ret": ret_w, "tok": tok}

        emit_tail_pre(prev)
        emit_tail_mm(prev)

    return nc


# --------------------------------------------------------------------------
# Host-side sharding / unsharding
# --------------------------------------------------------------------------
def shard_inputs(cfg, x, W_in, W_out, ln_gamma, ln_beta, pos_phases):
    c = cfg
    HD = N_CORES * c.NCH
    xT = np.ascontiguousarray(x.reshape(c.NTOK, c.DIM).T)          # [DIM, NTOK]
    # [p, k, tok] partition-major so one DMA covers all k-tiles of a chunk
    xt_h = np.ascontiguousarray(
        xT.reshape(c.KT1, 128, c.NTOK).transpose(1, 0, 2)
    ).astype(ml_dtypes.bfloat16)

    pos64 = pos_phases.astype(np.float64)
    cos_p = (0.5 * np.cos(pos64)).astype(np.float16)               # [T, HD]
    sin_p = (0.5 * np.sin(pos64)).astype(np.float16)

    Wg = (W_out * ln_gamma[None, :]).astype(np.float32)            # [DIM, 2HD]

    in_maps = []
    for cid in range(N_CORES):
        h0 = cid * c.NCH
        hs = slice(h0, h0 + c.NCH)
        w_ph = W_in[0 * HD + h0:0 * HD + h0 + c.NCH]               # [NCH, DIM]
        w_mg = W_in[1 * HD + h0:1 * HD + h0 + c.NCH]
        w_qr = W_in[2 * HD + h0:2 * HD + h0 + c.NCH]
        w_qi = W_in[3 * HD + h0:3 * HD + h0 + c.NCH]
        w_all = np.concatenate([w_ph, w_mg, w_qr, w_qi], axis=0)   # [4NCH, DIM]
        w1_h = np.ascontiguousarray(
            w_all.T.reshape(c.KT1, 128, 4 * c.NCH).transpose(1, 0, 2)
        ).astype(ml_dtypes.bfloat16)

        wg_re = Wg[:, 2 * h0:2 * (h0 + c.NCH):2]                   # [DIM, NCH]
        wg_im = Wg[:, 2 * h0 + 1:2 * (h0 + c.NCH):2]
        w2T = np.concatenate([wg_re.T, wg_im.T], axis=0)           # [2NCH, DIM]
        w2_h = np.ascontiguousarray(
            w2T.reshape(c.KT2, 128, c.DIM).transpose(1, 0, 2)
        ).astype(ml_dtypes.bfloat16)

        cp_h = np.ascontiguousarray(
            cos_p[:, hs].T.reshape(c.CT, 128, c.T).transpose(1, 0, 2))
        sp_h = np.ascontiguousarray(
            sin_p[:, hs].T.reshape(c.CT, 128, c.T).transpose(1, 0, 2))

        in_maps.append({
            "w1": w1_h, "w2": w2_h, "xt": xt_h,
            "cp": cp_h, "sp": sp_h,
        })
    return in_maps


def combine_outputs(cfg, results, W_out, ln_gamma, ln_beta, x_dtype):
    c = cfg
    NF = 2 * N_CORES * c.NCH
    P = np.zeros((c.DIM, c.NTOK), np.float64)
    S1 = np.zeros(c.NTOK, np.float64)
    S2 = np.zeros(c.NTOK, np.float64)
    for r in results:
        # outp is [128, DT, NTOK] partition-major of out^T -> [DIM, NTOK]
        op = r["outp"].transpose(1, 0, 2).reshape(c.DIM, c.NTOK)
        P += op.astype(np.float64)
        S1 += r["stats"][0].astype(np.float64)
        S2 += r["stats"][1].astype(np.float64)
    mu = S1 / NF
    var = S2 / NF - mu * mu
    istd = 1.0 / np.sqrt(var + LN_EPS)
    wg_sum = (W_out.astype(np.float64) @ ln_gamma.astype(np.float64))  # [DIM]
    b_out = (W_out.astype(np.float64) @ ln_beta.astype(np.float64))    # [DIM]
    out = istd[:, None] * (P.T - mu[:, None] * wg_sum[None, :]) + b_out[None, :]
    return out.reshape(c.B, c.T, c.DIM).astype(x_dtype)


def shard_inputs_v2(cfg, x, W_in, W_out, ln_gamma, ln_beta, pos_phases):
    c = cfg
    HD = N_CORES * c.NCH
    xT = np.ascontiguousarray(x.reshape(c.NTOK, c.DIM).T)          # [DIM, NTOK]
    xt_h = np.ascontiguousarray(
        xT.reshape(c.KT1, 128, c.NTOK).transpose(1, 0, 2)
    ).astype(np.float16)
    x8_h = np.ascontiguousarray(
        (xT * 16.0).reshape(c.KT1, 128, c.NTOK).transpose(1, 0, 2)
    ).astype(ml_dtypes.float8_e4m3)

    # pos/2, wrapped to [-pi/2, pi/2): theta_half = pi/2*tanh(ph) + pos/2
    pos64 = pos_phases.astype(np.float64)
    pos_half = (0.5 * (np.mod(pos64 + np.pi, 2 * np.pi) - np.pi)
                ).astype(np.float16)                               # [T, HD]

    Wg = (W_out * ln_gamma[None, :]).astype(np.float32)            # [DIM, 2HD]

    in_maps = []
    for cid in range(N_CORES):
        h0 = cid * c.NCH
        hs = slice(h0, h0 + c.NCH)
        w_ph = W_in[0 * HD + h0:0 * HD + h0 + c.NCH]
        w_mg = W_in[1 * HD + h0:1 * HD + h0 + c.NCH]
        w_qr = W_in[2 * HD + h0:2 * HD + h0 + c.NCH]
        w_qi = W_in[3 * HD + h0:3 * HD + h0 + c.NCH]
        w_all = np.concatenate([w_ph, w_mg, w_qr, w_qi], axis=0)   # [4NCH, DIM]
        w1_h = np.ascontiguousarray(
            w_all.T.reshape(c.KT1, 128, 4 * c.NCH).transpose(1, 0, 2)
        ).astype(np.float16)

        wg_re = Wg[:, 2 * h0:2 * (h0 + c.NCH):2]                   # [DIM, NCH]
        wg_im = Wg[:, 2 * h0 + 1:2 * (h0 + c.NCH):2]
        w2T = np.concatenate([wg_re.T, wg_im.T], axis=0)           # [2NCH, DIM]
        w2_h = np.ascontiguousarray(
            w2T.reshape(c.KT2, 128, c.DIM).transpose(1, 0, 2)
        ).astype(np.float16)

        pos_h = np.ascontiguousarray(
            pos_half[:, hs].T.reshape(c.CT, 128, c.T).transpose(1, 0, 2))

        # fp8(e4m3) copies for the magnitude channel (scales folded into
        # the on-chip tanh input scale: 0.5/(16*64)).
        w13_full = np.concatenate([w_ph, w_qr, w_qi], axis=0)
        w13_h = np.ascontiguousarray(
            w13_full.T.reshape(c.KT1, 128, 3 * c.NCH).transpose(1, 0, 2)
        ).astype(np.float16)
        wm8 = (w_mg.astype(np.float32) * 64.0).astype(
            ml_dtypes.float8_e4m3).astype(ml_dtypes.float8_e4m3)
        # layout [128, KT1//2, 2, NCH]: plane i of pair kk is k-tile 2kk+i
        wm8_h = np.ascontiguousarray(
            wm8.T.reshape(c.KT1 // 2, 2, 128, c.NCH).transpose(2, 0, 1, 3))
        in_maps.append({"w1": w1_h, "w2": w2_h, "xt": xt_h, "pos": pos_h,
                        "w13": w13_h, "w1m8": wm8_h, "x8": x8_h})
    return in_maps


def combine_outputs_v2(cfg, results, W_out, ln_gamma, ln_beta, x_dtype):
    c = cfg
    NF = 2 * N_CORES * c.NCH
    P = np.zeros((c.DIM, c.NTOK), np.float64)
    S1 = np.zeros(c.NTOK, np.float64)
    S2 = np.zeros(c.NTOK, np.float64)
    for r in results:
        op = r["outp"].transpose(1, 0, 2).reshape(c.DIM, c.NTOK)
        P += op.astype(np.float64)
        S1 += r["stats"][0].astype(np.float64)
        S2 += r["stats"][1].astype(np.float64)
    mu = S1 / NF
    var = S2 / NF - mu * mu
    istd = 1.0 / np.sqrt(var + LN_EPS)
    wg_sum = (W_out.astype(np.float64) @ ln_gamma.astype(np.float64))
    b_out = (W_out.astype(np.float64) @ ln_beta.astype(np.float64))
    out = istd[:, None] * (P.T - mu[:, None] * wg_sum[None, :]) + b_out[None, :]
    return out.reshape(c.B, c.T, c.DIM).astype(x_dtype)


def shard_inputs_v3(cfg, x, W_in, W_out, ln_gamma, ln_beta, pos_phases):
    c = cfg
    HD = N_CORES * c.NCH
    xT = np.ascontiguousarray(x.reshape(c.NTOK, c.DIM).T)          # [DIM, NTOK]
    xt_h = np.ascontiguousarray(
        xT.reshape(c.KT1, 128, c.NTOK).transpose(1, 0, 2)
    ).astype(np.float16)

    # pos/2, wrapped to [-pi/2, pi/2): theta_half = pi/2*tanh(ph) + pos/2
    pos64 = pos_phases.astype(np.float64)
    pos_half = (0.5 * (np.mod(pos64 + np.pi, 2 * np.pi) - np.pi)
                ).astype(np.float16)                               # [T, HD]

    Wg = (W_out * ln_gamma[None, :]).astype(np.float32)            # [DIM, 2HD]

    in_maps = []
    for cid in range(N_CORES):
        h0 = cid * c.NCH
        hs = slice(h0, h0 + c.NCH)
        w_ph = W_in[0 * HD + h0:0 * HD + h0 + c.NCH]
        w_mg = W_in[1 * HD + h0:1 * HD + h0 + c.NCH]
        w_qr = W_in[2 * HD + h0:2 * HD + h0 + c.NCH]
        w_qi = W_in[3 * HD + h0:3 * HD + h0 + c.NCH]
        w_all = np.concatenate([w_ph, w_mg, w_qr, w_qi], axis=0)   # [4NCH, DIM]
        w1_h = np.ascontiguousarray(
            w_all.T.reshape(c.KT1, 128, 4 * c.NCH).transpose(1, 0, 2)
        ).astype(np.float16)

        wg_re = Wg[:, 2 * h0:2 * (h0 + c.NCH):2]                   # [DIM, NCH]
        wg_im = Wg[:, 2 * h0 + 1:2 * (h0 + c.NCH):2]
        w2T = np.concatenate([wg_re.T, wg_im.T], axis=0)           # [2NCH, DIM]
        w2_h = np.ascontiguousarray(
            w2T.reshape(c.KT2, 128, c.DIM).transpose(1, 0, 2)
        ).astype(np.float16)

        pos_h = np.ascontiguousarray(
            pos_half[:, hs].T.reshape(c.CT, 128, c.T).transpose(1, 0, 2))
        in_maps.append({"w1": w1_h, "w2": w2_h, "xt": xt_h, "pos": pos_h})
    return in_maps


def combine_outputs_v3(cfg, results, W_out, ln_gamma, ln_beta, x_dtype):
    c = cfg
    NF = 2 * N_CORES * c.NCH
    P = np.zeros((c.DIM, c.NTOK), np.float64)
    S1 = np.zeros(c.NTOK, np.float64)
    S2 = np.zeros(c.NTOK, np.float64)
    for r in results:
        # outp [128, DD, 2, NTOK]: out[(dd*2+di)*128 + p, t]
        op = r["outp"].transpose(1, 2, 0, 3).reshape(c.DIM, c.NTOK)
        P += op.astype(np.float64)
        S1 += r["stats"][0].astype(np.float64)
        S2 += r["stats"][1].astype(np.float64)
    mu = S1 / NF
    var = S2 / NF - mu * mu
    istd = 1.0 / np.sqrt(var + LN_EPS)
    wg_sum = (W_out.astype(np.float64) @ ln_gamma.astype(np.float64))
    b_out = (W_out.astype(np.float64) @ ln_beta.astype(np.float64))
    out = istd[:, None] * (P.T - mu[:, None] * wg_sum[None, :]) + b_out[None, :]
    return out.reshape(c.B, c.T, c.DIM).astype(x_dtype)


import os

# Production configuration: v2 with fp8(e4m3) DoubleRow matmuls for the
# sigmoid magnitude channel (validated rel err 0.011 < 2e-2 on hardware),
# plus po512: proj_out runs over 2-chunk pairs as N=512 matmuls from a
# paired ret layout [128, SEGS, 2, CN] -- halves the proj_out LDW+MM
# instruction count at the better measured wide-N rate (0.466 vs 0.561
# ns/col), numerically identical to per-chunk proj_out.
# The kernel is PE-bound (~135-145 ns per LDW+MM pair at N=256 f16;
# fp8-DoubleRow ~2x cheaper per k-plane); other explored alternatives
# (all-f16 v3 with GPSIMD offload, CN=512, pin512, explicit-ldweights
# reuse, interleaved chain emission, ACT->DVE op moves) all measured
# equal or worse under a noise-robust interleaved A/B protocol.
DEFAULT_PROBE = "fp8mag,po512"


def _active_build(cfg, reps=1, hw_reps=1, probe=None):
    env = os.environ.get("KERNEL_PROBE")
    base = DEFAULT_PROBE if env is None else env
    merged = ",".join(x for x in [base, probe or ""] if x) or None
    return build_program_v2(cfg, reps=reps, hw_reps=hw_reps, probe=merged)


# Active implementation selector (test.py/bench use these too)
BUILD = _active_build
SHARD = shard_inputs_v2
COMBINE = combine_outputs_v2
CN_ACTIVE = 256

_cached = {}


def kernel(x, W_in, W_out, ln_gamma, ln_beta, pos_phases):
    cfg = Cfg(B=x.shape[0], T=x.shape[1], DIM=x.shape[2],
              NCH=pos_phases.shape[1] // N_CORES, CN=CN_ACTIVE)
    key = (cfg.B, cfg.T, cfg.DIM, cfg.NCH)
    if key not in _cached:
        nc = BUILD(cfg)
        split_multiwait(nc)  # walrus workaround; CoreSim path must skip this
        _cached[key] = nc
    nc = _cached[key]
    in_maps = SHARD(cfg, np.asarray(x), np.asarray(W_in),
                    np.asarray(W_out), np.asarray(ln_gamma),
                    np.asarray(ln_beta), np.asarray(pos_phases))
    # the native run path rejects in_map keys the program doesn't declare
    declared = {a.memorylocations[0].name
                for a in nc.m.functions[0].allocations
                if isinstance(a, mybir.MemoryLocationSet)
                and a.kind == "ExternalInput"}
    in_maps = [{k: v for k, v in m.items() if k in declared} for m in in_maps]
    res = run_bass_kernel_spmd(nc, in_maps, list(range(N_CORES)))
    return COMBINE(cfg, res.results, np.asarray(W_out),
                   np.asarray(ln_gamma), np.asarray(ln_beta),
                   np.asarray(x).dtype)

